# revision 1
# baseline (speedup 1.0000x reference)
"""Trainium2 Bass kernel for CachedMultiHeadedAttention (tensor-parallel over heads).

Sharding: 8 cores x 4 heads. Each core computes Q projection + attention for
its 4 heads, then a partial output projection against its 512 rows of Wo.
Host sums the 8 partial outputs (the "all-reduce" done at unshard time) and
adds bo.

Device-side layouts are chosen so NO on-chip transposes are needed:
  - x is passed pre-transposed (xT [D, S]) so contraction dims land on
    SBUF partitions for every matmul.
  - k_cache is passed pre-transposed per head (kT [DK, pos]).
  - The reference's softmax quirk (softmax over the QUERY axis) maps to
    scores^T tiles [l_part, s_free]: one fused ACT pass does exp + row-sum.
    The 1/sum normalization is folded into V rows (8x less data than the
    weight matrix).
Precision: streamed operands (x, Wq, k/v caches, Wo, qT, ctxT) are f16
(10 mantissa bits, ~5e-4 relative — full PE rate and half the DMA bytes of
f32r); softmax weights and scaled V run as float32r (full PE rate, unlike
plain fp32's 1/4 rate); all accumulation is f32 in PSUM, and the 8 partial
outputs are summed on the host in f64. The rank-1 k_new/v_new projections
run in bf16 — they only affect one of the 4096 cache rows. Measured
end-to-end relative error: ~6e-4.

Scheduling notes (cost-model-profiled):
  - Each dma_start costs ~625ns on the single serialized HWDGE queue, so
    DMAs are consolidated into ~130 large transfers (a naive version with
    557 DMAs spent 348us in HWDGE alone).
  - Engine queues execute in order, so the ACT-bound softmax loops carry
    "ride" work: the next head's Q-projection matmuls and (in head 0) the
    kv_new projections are emitted inside the S loop, paced per l-tile, with
    ctx matmuls lagged one iteration behind the exp that feeds them.
  - PSUM is exactly 8 banks: scores 2x[128,1024] (4) + ctx [128,1024] (2) +
    single-bank two-pass Q and kv_new accumulators (1+1).
"""

import math

import numpy as np
import ml_dtypes

import concourse.bass as bass
import concourse.mybir as mybir
import concourse.tile as tile
from concourse import bacc
from concourse.bass_utils import run_bass_kernel_spmd

F32 = mybir.dt.float32
F32R = mybir.dt.float32r
BF16 = mybir.dt.bfloat16
F16 = mybir.dt.float16
AF = mybir.ActivationFunctionType

H, D, DK, S = 32, 4096, 128, 1024
NCORES = 8
HP = H // NCORES          # heads per core
DC = D // 128             # contraction chunks for d_model


def build(pos: int):
    L = pos + 1
    LC = (L + 127) // 128          # number of 128-wide l tiles
    LG = (LC + 7) // 8             # l-tile groups of 8 (1024 l per group)
    INV = 1.0 / math.sqrt(DK)

    nc = bacc.Bacc("TRN2", target_bir_lowering=False, debug=False,
                   num_devices=NCORES)

    xT_d = nc.dram_tensor("xT", [D, S], F16, kind="ExternalInput").ap()
    wq_d = nc.dram_tensor("wq", [HP, D, DK], F16, kind="ExternalInput").ap()
    wkv_d = nc.dram_tensor("wkv", [D, 2 * HP * DK], BF16, kind="ExternalInput").ap()
    xl_d = nc.dram_tensor("xl", [128, DC], BF16, kind="ExternalInput").ap()
    bq_d = nc.dram_tensor("bq", [HP, DK, 1], F32, kind="ExternalInput").ap()
    bkv_d = nc.dram_tensor("bkv", [1, 2 * HP * DK], F32, kind="ExternalInput").ap()
    kT_d = nc.dram_tensor("kT", [HP, DK, pos], F16, kind="ExternalInput").ap()
    v_d = nc.dram_tensor("v", [HP, pos, DK], F16, kind="ExternalInput").ap()
    wo_d = nc.dram_tensor("wo", [HP * DK, D], F16, kind="ExternalInput").ap()
    out_d = nc.dram_tensor("out", [S, D], F16, kind="ExternalOutput").ap()

    with tile.TileContext(nc) as tc:
        # Pools are released LIFO; ctxT survives into the output projection,
        # so it sits at the bottom of the SBUF pool stack.
        ctxT_pool = tc.alloc_tile_pool(name="ctxT", bufs=1)
        wo_pool = tc.alloc_tile_pool(name="wop", bufs=1)
        stage_pool = tc.alloc_tile_pool(name="stagep", bufs=1)
        xT_pool = tc.alloc_tile_pool(name="xT", bufs=1)
        qT_pool = tc.alloc_tile_pool(name="qT", bufs=2)
        small = tc.alloc_tile_pool(name="smallp", bufs=1)
        wq_pool = tc.alloc_tile_pool(name="wqp", bufs=8)
        wkv_pool = tc.alloc_tile_pool(name="wkvp", bufs=3)
        kt_pool = tc.alloc_tile_pool(name="ktp", bufs=3)
        v_pool = tc.alloc_tile_pool(name="vp", bufs=3)
        wt_pool = tc.alloc_tile_pool(name="wtp", bufs=4)
        vs_pool = tc.alloc_tile_pool(name="vsp", bufs=4)
        ss_pool = tc.alloc_tile_pool(name="ssp", bufs=8)

        # PSUM budget (8 banks): psq 1 + kv 1 + pss 4 + psc 2.
        # Q projections and the kv_new projections run in TWO s-half /
        # k-v passes so their accumulators are single-bank.
        psq = tc.alloc_tile_pool(name="psq", bufs=1, space="PSUM")
        kv_pool = tc.alloc_tile_pool(name="kvp", bufs=1, space="PSUM")
        pss = tc.alloc_tile_pool(name="pss", bufs=2, space="PSUM")
        psc = tc.alloc_tile_pool(name="psc", bufs=1, space="PSUM")

        ctxTs = [ctxT_pool.tile([128, S], F16, name=f"cT{h}", tag=f"cT{h}")
                 for h in range(HP)]

        # small constants first (tiny DMAs, ahead of the big streams)
        kvrow = small.tile([1, 2 * HP * DK], F16, name="kvrow", tag="kvrow")
        bkv_t = small.tile([1, 2 * HP * DK], F32, name="bkvt", tag="bkvt")
        nc.sync.dma_start(bkv_t[:], bkv_d[:])
        xl_t = small.tile([128, DC], BF16, name="xlt", tag="xlt")
        nc.sync.dma_start(xl_t[:], xl_d[:])

        # resident xT tiles (8 big tiles of 4 chunks), interleaved with head
        # 0's Q weight groups so the first Q matmuls start after ~2.5MB, not
        # after the full 17MB of x.
        xbig = []
        wq0_groups = []
        for gx in range(DC // 4):
            wqt = wq_pool.tile([128, 4 * DK], F16, name=f"wq0_{gx}", tag="wq")
            nc.sync.dma_start(
                wqt[:], wq_d[0, gx * 512:(gx + 1) * 512, :].rearrange(
                    "(i p) k -> p i k", p=128))
            wq0_groups.append(wqt)
            xt = xT_pool.tile([128, 4 * S], F16, name=f"xt{gx}", tag=f"xt{gx}")
            nc.sync.dma_start(
                xt[:], xT_d[gx * 512:(gx + 1) * 512, :].rearrange(
                    "(i p) s -> p i s", p=128))
            xbig.append(xt)

        def xsl(c, lo, sz):
            return xbig[c // 4][:, (c % 4) * S + lo:(c % 4) * S + lo + sz]

        def emit_wq_dma(h, gw, tag="wq"):
            wqt = wq_pool.tile([128, 4 * DK], F16,
                               name=f"wq{h}_{gw}", tag=tag)
            nc.sync.dma_start(
                wqt[:], wq_d[h, gw * 512:(gw + 1) * 512, :].rearrange(
                    "(i p) k -> p i k", p=128))
            return wqt

        def q_half_mm(psq_t, wqt, c, half):
            lhs = wqt[:, (c % 4) * DK:(c % 4 + 1) * DK]
            nc.tensor.matmul(psq_t[:], lhs, xsl(c, half * 512, 512),
                             start=(c == 0), stop=(c == DC - 1))

        def q_half_add(h, qT_t, psq_t, half, bq_t):
            nc.vector.tensor_scalar_add(qT_t[:, half * 512:(half + 1) * 512],
                                        psq_t[:], bq_t[:])

        kv_cur = {}

        def kv_mm(kv_t, c, which):
            # which: 0 = k_new, 1 = v_new. Weight chunks are DMA'd two at a
            # time — each dma_start costs ~625ns of serialized HWDGE.
            if c % 4 == 0:
                wkvt = wkv_pool.tile([128, 4 * HP * DK], BF16,
                                     name=f"wkv{which}_{c}", tag="wkv")
                nc.sync.dma_start(
                    wkvt[:], wkv_d[c * 128:(c + 4) * 128,
                                   which * HP * DK:(which + 1) * HP * DK]
                    .rearrange("(i p) k -> p i k", p=128))
                kv_cur["t"] = wkvt
            wkvt = kv_cur["t"]
            nc.tensor.matmul(kv_t[0:1, :], xl_t[:, c:c + 1],
                             wkvt[:, (c % 4) * HP * DK:(c % 4 + 1) * HP * DK],
                             start=(c == 0), stop=(c == DC - 1))

        def kv_add(kv_t, which):
            nc.vector.tensor_add(
                kvrow[0:1, which * HP * DK:(which + 1) * HP * DK], kv_t[:],
                bkv_t[0:1, which * HP * DK:(which + 1) * HP * DK])

        def load_group(h, g):
            """Cache-only loads of l-group g (the new-entry writes are
            emitted separately, after kvrow's writes in trace order)."""
            g0 = g * 1024
            gl = min(1024, L - g0)            # valid l in group
            gc = max(0, min(1024, pos - g0))  # of which from cache
            kt8 = kt_pool.tile([128, 1024], F16, name=f"kt{h}_{g}", tag="kt")
            if gc > 0:
                nc.sync.dma_start(kt8[:, 0:gc], kT_d[h, :, g0:g0 + gc])
            if gl < 1024:
                nc.vector.memset(kt8[:, gl:1024], 0.0)
            v8 = v_pool.tile([128, 1024], F16, name=f"v{h}_{g}", tag="v")
            if gl < 1024:
                # zero whole padded chunks first (full partition range — DVE
                # requires 32-aligned partition bases); valid rows are DMA'd
                # over the zeros below.
                nc.vector.memset(v8[:, (gl // 128) * 128:1024], 0.0)
            fc = gc // 128
            if fc > 0:
                nc.sync.dma_start(
                    v8[:, 0:fc * 128],
                    v_d[h, g0:g0 + fc * 128, :].rearrange(
                        "(i p) k -> p i k", p=128))
            rem = gc - fc * 128
            if rem > 0:
                nc.sync.dma_start(v8[0:rem, fc * 128:(fc + 1) * 128],
                                  v_d[h, g0 + fc * 128:g0 + gc, :])
            return kt8, v8

        def new_entry_writes(h, kt8, v8):
            # column/row for l == pos from the biased kvrow
            gp = pos % 1024
            nc.sync.dma_start(kt8[:, gp:gp + 1],
                              kvrow[0:1, h * DK:(h + 1) * DK])
            nc.sync.dma_start(
                v8[gp % 128:gp % 128 + 1, (gp // 128) * 128:(gp // 128 + 1) * 128],
                kvrow[0:1, HP * DK + h * DK:HP * DK + (h + 1) * DK])

        npos_g = pos // 1024            # l-group holding the new entry
        npos_lt = pos // 128            # l-tile index holding the new entry
        # riding is only possible when the S loop is long enough for the
        # 2-instruction-per-lt passes to finish before the new entry is used
        ride_kv = LC >= DC and npos_lt >= 8
        ride_q = LC >= DC

        # ---------- head 0 Q projection (phase A, DMA-paced) ----------
        bq_t = ss_pool.tile([128, 1], F32, name="bq0", tag="bq", bufs=2)
        nc.sync.dma_start(bq_t[:], bq_d[0])
        qT_t = qT_pool.tile([128, S], F16, name="qT0", tag="qT")
        # both s-halves accumulate concurrently (pass B borrows the idle kv
        # bank) so the whole projection rides the x-arrival gaps instead of
        # serializing 6.8us of pass-B matmuls after the stream ends
        psq_a = psq.tile([128, 512], F32, name="psq0_0", tag="psq")
        psq_b = kv_pool.tile([128, 512], F32, name="psq0_1", tag="kv")
        for c in range(DC):
            q_half_mm(psq_a, wq0_groups[c // 4], c, 0)
            q_half_mm(psq_b, wq0_groups[c // 4], c, 1)
        q_half_add(0, qT_t, psq_a, 0, bq_t)
        q_half_add(0, qT_t, psq_b, 1, bq_t)

        if not ride_kv:
            # fallback: dense kv_new before the S loops
            for which in range(2):
                kv_t = kv_pool.tile([1, HP * DK], F32, name=f"kv{which}", tag="kv")
                for c in range(DC):
                    kv_mm(kv_t, c, which)
                kv_add(kv_t, which)

        for h in range(HP):
            # per-lt ride items emitted right after the scores matmuls
            rides = [[] for _ in range(LC)]
            if h + 1 < HP and ride_q:
                bq1 = ss_pool.tile([128, 1], F32, name=f"bq{h+1}", tag="bq",
                                   bufs=2)
                nc.sync.dma_start(bq1[:], bq_d[h + 1])
                qT_next = qT_pool.tile([128, S], F16, name=f"qT{h+1}", tag="qT")
                state = {}

                def mk_q(lt, h1=h + 1, qn=qT_next, bqt=bq1, st=state):
                    def emit():
                        half, c0 = divmod(2 * lt, DC)
                        if c0 == 0 and half == 0:
                            st["wqts"] = {}
                        if c0 == 0:
                            st["psq"] = psq.tile([128, 512], F32,
                                                 name=f"psq{h1}_{half}", tag="psq")
                        for c in (c0, c0 + 1):
                            gw = c // 4
                            if half == 0 and c % 4 == 0:
                                # pass B reuses these resident tiles (8 slots)
                                st["wqts"][gw] = emit_wq_dma(h1, gw)
                            q_half_mm(st["psq"], st["wqts"][gw], c, half)
                        if c0 + 1 == DC - 1:
                            q_half_add(h1, qn, st["psq"], half, bqt)
                    return emit

                for lt in range(DC):
                    rides[lt].append(mk_q(lt))
            if h == 0 and ride_kv:
                # kv_new work items, paced so both passes (and their kvrow
                # writes) are emitted strictly before lt == npos_lt
                kv_work = ([("mm", 0, c) for c in range(DC)] + [("add", 0, 0)]
                           + [("mm", 1, c) for c in range(DC)] + [("add", 1, 0)])
                kvstate = {}

                def kv_emit_one(item, st=kvstate):
                    kind, which, c = item
                    if kind == "add":
                        kv_add(st["kv"], which)
                        return
                    if c == 0:
                        st["kv"] = kv_pool.tile([1, HP * DK], F32,
                                                name=f"kv{which}", tag="kv")
                    kv_mm(st["kv"], c, which)

                n_slots = npos_lt - 1          # ride slots: lt 0..npos_lt-2
                n_pre = max(0, len(kv_work) - 2 * n_slots)
                for item in kv_work[:n_pre]:
                    kv_emit_one(item)
                rest = kv_work[n_pre:]
                for k, item in enumerate(rest):
                    rides[k // 2].append(
                        (lambda it=item: kv_emit_one(it)))

            o_staged = {}
            o_post = []
            if h == HP - 1 and LC >= DC:
                # S_3 has no Q to ride; its psq/kv PSUM banks are dead. Ride
                # the first-3-chunk partials of 16 output tiles there, staged
                # to SBUF; the O phase finishes them with one matmul + add.
                wos = [wo_pool.tile([128, D], F16, name=f"wo{c}", tag=f"wo{c}")
                       for c in range(HP)]

                def mk_wo(c):
                    return lambda: nc.sync.dma_start(
                        wos[c][:], wo_d[c * 128:(c + 1) * 128, :])

                o_tiles = [(s_t, mg) for s_t in (6, 7) for mg in range(D // 512)]
                o_state = {}

                def mk_o(item, st=o_state):
                    t, k = item
                    s_t, mg = o_tiles[t]

                    def emit():
                        if k == 0:
                            pool = kv_pool if t % 2 == 0 else psq
                            st["ps"] = pool.tile(
                                [128, 512], F32, name=f"ops{t}",
                                tag="kv" if t % 2 == 0 else "psq")
                        if k < 3:
                            nc.tensor.matmul(
                                st["ps"][:],
                                ctxTs[k][:, s_t * 128:(s_t + 1) * 128],
                                wos[k][:, mg * 512:(mg + 1) * 512],
                                start=(k == 0), stop=(k == 2))
                        else:
                            sg = stage_pool.tile([128, 512], F16,
                                                 name=f"sg{t}", tag=f"sg{t}")
                            nc.vector.tensor_copy(sg[:], st["ps"][:])
                            o_staged[(s_t, mg)] = sg
                    return emit

                # wo0/wo1 load right after S_3's first K/V group; wo2/wo3
                # trail via the ride slots they're needed in
                o_post.extend([mk_wo(0), mk_wo(1)])
                rides[2].append(mk_wo(2))
                rides[10].append(mk_wo(3))
                o_work = [(t, k) for t in range(len(o_tiles)) for k in range(4)]
                for idx, item in enumerate(o_work):
                    rides[6 + idx // 3].append(mk_o(item))

            psc_t = psc.tile([128, S], F32, name=f"psc{h}", tag="psc")
            cur = load_group(h, 0)
            for fn_ in o_post:
                fn_()
            if not (h == 0 and ride_kv) and npos_g == 0 and npos_lt < LC:
                new_entry_writes(h, *cur)
            nxt = None
            pend = None              # lag-1 ctx: (lt, wt, vst)
            for lt in range(LC):
                g, j = lt // 8, lt % 8
                if j == 0 and g > 0:
                    cur = nxt
                if j == 0 and g + 1 < (LC + 7) // 8:
                    nxt = load_group(h, g + 1)
                    if not (h == 0 and ride_kv) and npos_g == g + 1:
                        new_entry_writes(h, *nxt)
                kt8, v8 = cur
                if h == 0 and ride_kv and lt == npos_lt:
                    # kvrow writes were emitted at lt <= npos_lt - 1
                    new_entry_writes(h, kt8, v8) if npos_g == g else None
                    if npos_g == g + 1 and nxt is not None:
                        new_entry_writes(h, *nxt)

                ps = pss.tile([128, 1024], F32, name=f"ps_{h}_{lt}", tag="pss")
                ksl = kt8[:, j * 128:(j + 1) * 128]
                nc.tensor.matmul(ps[:, 0:512], ksl, qT_t[:, 0:512])
                nc.tensor.matmul(ps[:, 512:1024], ksl, qT_t[:, 512:1024])

                for emit in rides[lt]:
                    emit()

                wt = wt_pool.tile([128, 1024], F32R, name=f"wt_{h}_{lt}", tag="wt")
                ssum = ss_pool.tile([128, 1], F32, name=f"ss_{h}_{lt}", tag="ssum")
                nc.scalar.activation(wt[:], ps[:], AF.Exp, scale=INV, accum_out=ssum[:])
                rec = ss_pool.tile([128, 1], F32, name=f"rc_{h}_{lt}", tag="rec")
                nc.vector.reciprocal(rec[:], ssum[:])
                vst = vs_pool.tile([128, DK], F32R, name=f"vs{h}_{lt}", tag="vs")
                nc.vector.tensor_scalar_mul(vst[:], v8[:, j * 128:(j + 1) * 128], rec[:])

                if pend is not None:
                    plt, pwt, pvst = pend
                    nc.tensor.matmul(psc_t[:, 0:512], pvst[:], pwt[:, 0:512],
                                     start=(plt == 0), stop=False)
                    nc.tensor.matmul(psc_t[:, 512:1024], pvst[:], pwt[:, 512:1024],
                                     start=(plt == 0), stop=False)
                pend = (lt, wt, vst)
            plt, pwt, pvst = pend
            nc.tensor.matmul(psc_t[:, 0:512], pvst[:], pwt[:, 0:512],
                             start=(plt == 0), stop=True)
            nc.tensor.matmul(psc_t[:, 512:1024], pvst[:], pwt[:, 512:1024],
                             start=(plt == 0), stop=True)
            nc.vector.tensor_copy(ctxTs[h][:], psc_t[:])
            if h + 1 < HP and not ride_q:
                # dense fallback Q projection for the next head
                bq1 = ss_pool.tile([128, 1], F32, name=f"bq{h+1}", tag="bq",
                                   bufs=2)
                nc.sync.dma_start(bq1[:], bq_d[h + 1])
                qT_next = qT_pool.tile([128, S], F16, name=f"qT{h+1}", tag="qT")
                wqts_fb = {}
                for half in range(2):
                    psq_t = psq.tile([128, 512], F32,
                                     name=f"psq{h+1}_{half}", tag="psq")
                    for c in range(DC):
                        if half == 0 and c % 4 == 0:
                            wqts_fb[c // 4] = emit_wq_dma(h + 1, c // 4)
                        q_half_mm(psq_t, wqts_fb[c // 4], c, half)
                    q_half_add(h + 1, qT_next, psq_t, half, bq1)
            if h + 1 < HP:
                qT_t = qT_next

        # release attention-phase pools before the output projection (LIFO)
        for p in (psc, pss, kv_pool, psq,
                  ss_pool, vs_pool, wt_pool, v_pool, kt_pool,
                  wkv_pool, wq_pool, small, qT_pool, xT_pool):
            p.release()

        # ---------- output projection: out[s, m] partial ----------
        # Wo fully resident in the space freed by xT; one 16KB-burst output
        # DMA per s-tile.
        ob_pool = tc.alloc_tile_pool(name="obp", bufs=2)
        pso = tc.alloc_tile_pool(name="pso", bufs=4, space="PSUM")
        if not o_staged:
            # fallback path (short sequences): load Wo here
            wos = []
            for c in range(HP):
                wot = wo_pool.tile([128, D], F16, name=f"wo{c}", tag=f"wo{c}")
                nc.sync.dma_start(wot[:], wo_d[c * 128:(c + 1) * 128, :])
                wos.append(wot)
        for s_t in range(S // 128):
            ob = ob_pool.tile([128, D], F16, name=f"ob{s_t}", tag="ob")
            for mg in range(D // 512):
                sg = o_staged.get((s_t, mg))
                pso_t = pso.tile([128, 512], F32, name=f"po{s_t}_{mg}", tag="pso")
                if sg is not None:
                    nc.tensor.matmul(pso_t[:],
                                     ctxTs[HP - 1][:, s_t * 128:(s_t + 1) * 128],
                                     wos[HP - 1][:, mg * 512:(mg + 1) * 512])
                    nc.vector.tensor_add(ob[:, mg * 512:(mg + 1) * 512],
                                         sg[:], pso_t[:])
                else:
                    for c in range(HP):
                        nc.tensor.matmul(pso_t[:],
                                         ctxTs[c][:, s_t * 128:(s_t + 1) * 128],
                                         wos[c][:, mg * 512:(mg + 1) * 512],
                                         start=(c == 0), stop=(c == HP - 1))
                    nc.vector.tensor_copy(ob[:, mg * 512:(mg + 1) * 512], pso_t[:])
            if s_t == S // 128 - 1:
                # stream the final tile's output per mg-pair: the exposed
                # post-compute transfer shrinks to a quarter row-band
                for q in range(8):
                    nc.sync.dma_start(
                        out_d[s_t * 128:(s_t + 1) * 128,
                              q * (D // 8):(q + 1) * (D // 8)],
                        ob[:, q * (D // 8):(q + 1) * (D // 8)])
            else:
                nc.sync.dma_start(out_d[s_t * 128:(s_t + 1) * 128, :], ob[:])
        for p in (pso, ob_pool, stage_pool, wo_pool, ctxT_pool):
            p.release()

    nc.compile()
    return nc


_CACHE = {}
LAST_EXEC_NS = None


def kernel(x, k_cache, v_cache, Wq, bq, Wk, bk, Wv, bv, Wo, bo, pos):
    global LAST_EXEC_NS
    pos = int(pos)

    def f32(a):
        return np.ascontiguousarray(np.asarray(a), dtype=np.float32)

    x = f32(x)
    k_cache, v_cache = f32(k_cache), f32(v_cache)
    Wq, Wk, Wv, Wo = f32(Wq), f32(Wk), f32(Wv), f32(Wo)
    bq, bk, bv, bo = f32(bq), f32(bk), f32(bv), f32(bo)

    xT = np.ascontiguousarray(x[0].T.astype(np.float16))   # [D, S]
    xl = np.ascontiguousarray(
        x[0, -1].reshape(DC, 128).T.astype(ml_dtypes.bfloat16))
    in_maps = []
    for i in range(NCORES):
        hs = slice(i * HP, (i + 1) * HP)
        in_maps.append({
            "xT": xT,
            "wq": np.ascontiguousarray(Wq[hs].astype(np.float16)),
            "wkv": np.ascontiguousarray(np.concatenate([
                Wk[hs].transpose(1, 0, 2).reshape(D, HP * DK),
                Wv[hs].transpose(1, 0, 2).reshape(D, HP * DK)],
                axis=1).astype(ml_dtypes.bfloat16)),
            "xl": xl,
            "bq": np.ascontiguousarray(bq[hs].reshape(HP, DK, 1)),
            "bkv": np.ascontiguousarray(np.concatenate(
                [bk[hs].reshape(-1), bv[hs].reshape(-1)])[None, :]),
            "kT": np.ascontiguousarray(
                k_cache[hs, :pos, :].transpose(0, 2, 1).astype(np.float16)),
            "v": np.ascontiguousarray(v_cache[hs, :pos, :].astype(np.float16)),
            "wo": np.ascontiguousarray(
                Wo[i * HP * DK:(i + 1) * HP * DK].astype(np.float16)),
        })

    if pos not in _CACHE:
        _CACHE[pos] = build(pos)
    nc = _CACHE[pos]

    res = run_bass_kernel_spmd(nc, in_maps, core_ids=list(range(NCORES)))
    LAST_EXEC_NS = res.exec_time_ns

    acc = np.zeros((S, D), np.float64)
    for r in res.results:
        acc += r["out"]
    out = (acc + bo.astype(np.float64)).astype(np.float32)
    return out[None]



# revision 33
# speedup vs baseline: 1.0812x; 1.0812x over previous
"""Trainium2 Bass kernel for CachedMultiHeadedAttention (tensor-parallel over heads).

Sharding: 8 cores x 4 heads. Each core computes Q projection + attention for
its 4 heads, then a partial output projection against its 512 rows of Wo.
Host sums the 8 partial outputs (the "all-reduce" done at unshard time) and
adds bo.

Device-side layouts are chosen so NO on-chip transposes are needed:
  - x is passed pre-transposed (xT [D, S]) so contraction dims land on
    SBUF partitions for every matmul.
  - k_cache is passed pre-transposed per head (kT [DK, pos]).
  - The reference's softmax quirk (softmax over the QUERY axis) maps to
    scores^T tiles [l_part, s_free]: one fused ACT pass does exp + row-sum.
    The 1/sum normalization is folded into V rows (8x less data than the
    weight matrix).
Precision: streamed operands (x, Wq, k/v caches, Wo, qT, ctxT) are f16
(10 mantissa bits, ~5e-4 relative — full PE rate and half the DMA bytes of
f32r); softmax weights and scaled V run as float32r (full PE rate, unlike
plain fp32's 1/4 rate); all accumulation is f32 in PSUM, and the 8 partial
outputs are summed on the host in f64. The rank-1 k_new/v_new projections
run in bf16 — they only affect one of the 4096 cache rows. Measured
end-to-end relative error: ~6e-4.

Scheduling notes (cost-model-profiled):
  - Each dma_start costs ~625ns on the single serialized HWDGE queue, so
    DMAs are consolidated into ~130 large transfers (a naive version with
    557 DMAs spent 348us in HWDGE alone).
  - Engine queues execute in order, so the ACT-bound softmax loops carry
    "ride" work: the next head's Q-projection matmuls and (in head 0) the
    kv_new projections are emitted inside the S loop, paced per l-tile, with
    ctx matmuls lagged one iteration behind the exp that feeds them.
  - PSUM is exactly 8 banks: scores 2x[128,1024] (4) + ctx [128,1024] (2) +
    single-bank two-pass Q and kv_new accumulators (1+1).
"""

import math

import numpy as np
import ml_dtypes

import concourse.bass as bass
import concourse.mybir as mybir
import concourse.tile as tile
from concourse import bacc
from concourse.bass_utils import run_bass_kernel_spmd

F32 = mybir.dt.float32
F32R = mybir.dt.float32r
BF16 = mybir.dt.bfloat16
F16 = mybir.dt.float16
AF = mybir.ActivationFunctionType

H, D, DK, S = 32, 4096, 128, 1024
NCORES = 8
HP = H // NCORES          # heads per core
DC = D // 128             # contraction chunks for d_model


def build(pos: int):
    L = pos + 1
    LC = (L + 127) // 128          # number of 128-wide l tiles
    LG = (LC + 7) // 8             # l-tile groups of 8 (1024 l per group)
    INV = 1.0 / math.sqrt(DK)

    nc = bacc.Bacc("TRN2", target_bir_lowering=False, debug=False,
                   num_devices=NCORES)

    xT_d = nc.dram_tensor("xT", [D, S], F16, kind="ExternalInput").ap()
    wq_d = nc.dram_tensor("wq", [HP, D, DK], F16, kind="ExternalInput").ap()
    wkv_d = nc.dram_tensor("wkv", [D, 2 * HP * DK], BF16, kind="ExternalInput").ap()
    xl_d = nc.dram_tensor("xl", [128, DC], BF16, kind="ExternalInput").ap()
    bq_d = nc.dram_tensor("bq", [HP, DK, 1], F32, kind="ExternalInput").ap()
    bkv_d = nc.dram_tensor("bkv", [1, 2 * HP * DK], F32, kind="ExternalInput").ap()
    kT_d = nc.dram_tensor("kT", [HP, DK, pos], F16, kind="ExternalInput").ap()
    v_d = nc.dram_tensor("v", [HP, pos, DK], F16, kind="ExternalInput").ap()
    wo_d = nc.dram_tensor("wo", [HP * DK, D], F16, kind="ExternalInput").ap()
    out_d = nc.dram_tensor("out", [S, D], F16, kind="ExternalOutput").ap()

    with tile.TileContext(nc) as tc:
        # Pools are released LIFO; ctxT survives into the output projection,
        # so it sits at the bottom of the SBUF pool stack.
        ctxT_pool = tc.alloc_tile_pool(name="ctxT", bufs=1)
        wo_pool = tc.alloc_tile_pool(name="wop", bufs=1)
        stage_pool = tc.alloc_tile_pool(name="stagep", bufs=1)
        xT_pool = tc.alloc_tile_pool(name="xT", bufs=1)
        qT_pool = tc.alloc_tile_pool(name="qT", bufs=2)
        small = tc.alloc_tile_pool(name="smallp", bufs=1)
        wq_pool = tc.alloc_tile_pool(name="wqp", bufs=8)
        wkv_pool = tc.alloc_tile_pool(name="wkvp", bufs=3)
        kt_pool = tc.alloc_tile_pool(name="ktp", bufs=3)
        v_pool = tc.alloc_tile_pool(name="vp", bufs=3)
        wt_pool = tc.alloc_tile_pool(name="wtp", bufs=4)
        vs_pool = tc.alloc_tile_pool(name="vsp", bufs=4)
        ss_pool = tc.alloc_tile_pool(name="ssp", bufs=8)

        # PSUM budget (8 banks): psq 1 + kv 1 + pss 4 + psc 2.
        # Q projections and the kv_new projections run in TWO s-half /
        # k-v passes so their accumulators are single-bank.
        psq = tc.alloc_tile_pool(name="psq", bufs=1, space="PSUM")
        kv_pool = tc.alloc_tile_pool(name="kvp", bufs=1, space="PSUM")
        pss = tc.alloc_tile_pool(name="pss", bufs=2, space="PSUM")
        psc = tc.alloc_tile_pool(name="psc", bufs=1, space="PSUM")

        ctxTs = [ctxT_pool.tile([128, S], F16, name=f"cT{h}", tag=f"cT{h}")
                 for h in range(HP)]

        # small constants first (tiny DMAs, ahead of the big streams)
        kvrow = small.tile([1, 2 * HP * DK], F16, name="kvrow", tag="kvrow")
        bkv_t = small.tile([1, 2 * HP * DK], F32, name="bkvt", tag="bkvt")
        nc.sync.dma_start(bkv_t[:], bkv_d[:])
        xl_t = small.tile([128, DC], BF16, name="xlt", tag="xlt")
        nc.sync.dma_start(xl_t[:], xl_d[:])

        # resident xT tiles (8 big tiles of 4 chunks), interleaved with head
        # 0's Q weight groups so the first Q matmuls start after ~2.5MB, not
        # after the full 17MB of x.
        xbig = []
        wq0_groups = []
        for gx in range(DC // 4):
            wqt = wq_pool.tile([128, 4 * DK], F16, name=f"wq0_{gx}", tag="wq")
            nc.sync.dma_start(
                wqt[:], wq_d[0, gx * 512:(gx + 1) * 512, :].rearrange(
                    "(i p) k -> p i k", p=128))
            wq0_groups.append(wqt)
            xt = xT_pool.tile([128, 4 * S], F16, name=f"xt{gx}", tag=f"xt{gx}")
            nc.sync.dma_start(
                xt[:], xT_d[gx * 512:(gx + 1) * 512, :].rearrange(
                    "(i p) s -> p i s", p=128))
            xbig.append(xt)

        def xsl(c, lo, sz):
            return xbig[c // 4][:, (c % 4) * S + lo:(c % 4) * S + lo + sz]

        def emit_wq_dma(h, gw, tag="wq"):
            wqt = wq_pool.tile([128, 4 * DK], F16,
                               name=f"wq{h}_{gw}", tag=tag)
            nc.sync.dma_start(
                wqt[:], wq_d[h, gw * 512:(gw + 1) * 512, :].rearrange(
                    "(i p) k -> p i k", p=128))
            return wqt

        def q_half_mm(psq_t, wqt, c, half):
            lhs = wqt[:, (c % 4) * DK:(c % 4 + 1) * DK]
            nc.tensor.matmul(psq_t[:], lhs, xsl(c, half * 512, 512),
                             start=(c == 0), stop=(c == DC - 1))

        def q_half_add(h, qT_t, psq_t, half, bq_t):
            nc.vector.tensor_scalar_add(qT_t[:, half * 512:(half + 1) * 512],
                                        psq_t[:], bq_t[:])

        kv_cur = {}

        def kv_mm(kv_t, c, which):
            # which: 0 = k_new, 1 = v_new. Weight chunks are DMA'd two at a
            # time — each dma_start costs ~625ns of serialized HWDGE.
            if c % 4 == 0:
                wkvt = wkv_pool.tile([128, 4 * HP * DK], BF16,
                                     name=f"wkv{which}_{c}", tag="wkv")
                nc.sync.dma_start(
                    wkvt[:], wkv_d[c * 128:(c + 4) * 128,
                                   which * HP * DK:(which + 1) * HP * DK]
                    .rearrange("(i p) k -> p i k", p=128))
                kv_cur["t"] = wkvt
            wkvt = kv_cur["t"]
            nc.tensor.matmul(kv_t[0:1, :], xl_t[:, c:c + 1],
                             wkvt[:, (c % 4) * HP * DK:(c % 4 + 1) * HP * DK],
                             start=(c == 0), stop=(c == DC - 1))

        def kv_add(kv_t, which):
            nc.vector.tensor_add(
                kvrow[0:1, which * HP * DK:(which + 1) * HP * DK], kv_t[:],
                bkv_t[0:1, which * HP * DK:(which + 1) * HP * DK])

        def load_group(h, g):
            """Cache-only loads of l-group g (the new-entry writes are
            emitted separately, after kvrow's writes in trace order)."""
            g0 = g * 1024
            gl = min(1024, L - g0)            # valid l in group
            gc = max(0, min(1024, pos - g0))  # of which from cache
            kt8 = kt_pool.tile([128, 1024], F16, name=f"kt{h}_{g}", tag="kt")
            if gc > 0:
                nc.sync.dma_start(kt8[:, 0:gc], kT_d[h, :, g0:g0 + gc])
            if gl < 1024:
                nc.vector.memset(kt8[:, gl:1024], 0.0)
            v8 = v_pool.tile([128, 1024], F16, name=f"v{h}_{g}", tag="v")
            if gl < 1024:
                # zero whole padded chunks first (full partition range — DVE
                # requires 32-aligned partition bases); valid rows are DMA'd
                # over the zeros below.
                nc.vector.memset(v8[:, (gl // 128) * 128:1024], 0.0)
            fc = gc // 128
            if fc > 0:
                nc.sync.dma_start(
                    v8[:, 0:fc * 128],
                    v_d[h, g0:g0 + fc * 128, :].rearrange(
                        "(i p) k -> p i k", p=128))
            rem = gc - fc * 128
            if rem > 0:
                nc.sync.dma_start(v8[0:rem, fc * 128:(fc + 1) * 128],
                                  v_d[h, g0 + fc * 128:g0 + gc, :])
            return kt8, v8

        def new_entry_writes(h, kt8, v8):
            # column/row for l == pos from the biased kvrow
            gp = pos % 1024
            nc.sync.dma_start(kt8[:, gp:gp + 1],
                              kvrow[0:1, h * DK:(h + 1) * DK])
            nc.sync.dma_start(
                v8[gp % 128:gp % 128 + 1, (gp // 128) * 128:(gp // 128 + 1) * 128],
                kvrow[0:1, HP * DK + h * DK:HP * DK + (h + 1) * DK])

        npos_g = pos // 1024            # l-group holding the new entry
        npos_lt = pos // 128            # l-tile index holding the new entry
        # riding is only possible when the S loop is long enough for the
        # 2-instruction-per-lt passes to finish before the new entry is used
        ride_kv = LC >= DC and npos_lt >= 8
        ride_q = LC >= DC

        # ---------- head 0 Q projection (phase A, DMA-paced) ----------
        bq_t = ss_pool.tile([128, 1], F32, name="bq0", tag="bq", bufs=2)
        nc.sync.dma_start(bq_t[:], bq_d[0])
        qT_t = qT_pool.tile([128, S], F16, name="qT0", tag="qT")
        # both s-halves accumulate concurrently (pass B borrows the idle kv
        # bank) so the whole projection rides the x-arrival gaps instead of
        # serializing 6.8us of pass-B matmuls after the stream ends
        psq_a = psq.tile([128, 512], F32, name="psq0_0", tag="psq")
        psq_b = kv_pool.tile([128, 512], F32, name="psq0_1", tag="kv")
        for c in range(DC):
            q_half_mm(psq_a, wq0_groups[c // 4], c, 0)
            q_half_mm(psq_b, wq0_groups[c // 4], c, 1)
        q_half_add(0, qT_t, psq_a, 0, bq_t)
        q_half_add(0, qT_t, psq_b, 1, bq_t)

        if not ride_kv:
            # fallback: dense kv_new before the S loops
            for which in range(2):
                kv_t = kv_pool.tile([1, HP * DK], F32, name=f"kv{which}", tag="kv")
                for c in range(DC):
                    kv_mm(kv_t, c, which)
                kv_add(kv_t, which)

        for h in range(HP):
            # per-lt ride items emitted right after the scores matmuls
            rides = [[] for _ in range(LC)]
            if h + 1 < HP and ride_q:
                bq1 = ss_pool.tile([128, 1], F32, name=f"bq{h+1}", tag="bq",
                                   bufs=2)
                nc.sync.dma_start(bq1[:], bq_d[h + 1])
                qT_next = qT_pool.tile([128, S], F16, name=f"qT{h+1}", tag="qT")
                state = {}

                def mk_q(lt, h1=h + 1, qn=qT_next, bqt=bq1, st=state):
                    def emit():
                        half, c0 = divmod(2 * lt, DC)
                        if c0 == 0 and half == 0:
                            st["wqts"] = {}
                        if c0 == 0:
                            st["psq"] = psq.tile([128, 512], F32,
                                                 name=f"psq{h1}_{half}", tag="psq")
                        for c in (c0, c0 + 1):
                            gw = c // 4
                            if half == 0 and c % 4 == 0:
                                # pass B reuses these resident tiles (8 slots)
                                st["wqts"][gw] = emit_wq_dma(h1, gw)
                            q_half_mm(st["psq"], st["wqts"][gw], c, half)
                        if c0 + 1 == DC - 1:
                            q_half_add(h1, qn, st["psq"], half, bqt)
                    return emit

                for lt in range(DC):
                    rides[lt].append(mk_q(lt))
            if h == 0 and ride_kv:
                # kv_new work items, paced so both passes (and their kvrow
                # writes) are emitted strictly before lt == npos_lt
                kv_work = ([("mm", 0, c) for c in range(DC)] + [("add", 0, 0)]
                           + [("mm", 1, c) for c in range(DC)] + [("add", 1, 0)])
                kvstate = {}

                def kv_emit_one(item, st=kvstate):
                    kind, which, c = item
                    if kind == "add":
                        kv_add(st["kv"], which)
                        return
                    if c == 0:
                        st["kv"] = kv_pool.tile([1, HP * DK], F32,
                                                name=f"kv{which}", tag="kv")
                    kv_mm(st["kv"], c, which)

                n_slots = npos_lt - 1          # ride slots: lt 0..npos_lt-2
                n_pre = max(0, len(kv_work) - 2 * n_slots)
                for item in kv_work[:n_pre]:
                    kv_emit_one(item)
                rest = kv_work[n_pre:]
                for k, item in enumerate(rest):
                    rides[k // 2].append(
                        (lambda it=item: kv_emit_one(it)))

            o_staged = {}
            o_post = []
            if h == HP - 1 and LC >= DC:
                # S_3 has no Q to ride; its psq/kv PSUM banks are dead. Ride
                # the first-3-chunk partials of 16 output tiles there, staged
                # to SBUF; the O phase finishes them with one matmul + add.
                wos = [wo_pool.tile([128, D], F16, name=f"wo{c}", tag=f"wo{c}")
                       for c in range(HP)]

                def mk_wo(c):
                    return lambda: nc.sync.dma_start(
                        wos[c][:], wo_d[c * 128:(c + 1) * 128, :])

                o_tiles = [(s_t, mg) for s_t in (6, 7) for mg in range(D // 512)]
                o_state = {}

                def mk_o(item, st=o_state):
                    t, k = item
                    s_t, mg = o_tiles[t]

                    def emit():
                        if k == 0:
                            pool = kv_pool if t % 2 == 0 else psq
                            st["ps"] = pool.tile(
                                [128, 512], F32, name=f"ops{t}",
                                tag="kv" if t % 2 == 0 else "psq")
                        if k < 3:
                            nc.tensor.matmul(
                                st["ps"][:],
                                ctxTs[k][:, s_t * 128:(s_t + 1) * 128],
                                wos[k][:, mg * 512:(mg + 1) * 512],
                                start=(k == 0), stop=(k == 2))
                        else:
                            sg = stage_pool.tile([128, 512], F16,
                                                 name=f"sg{t}", tag=f"sg{t}")
                            nc.vector.tensor_copy(sg[:], st["ps"][:])
                            o_staged[(s_t, mg)] = sg
                    return emit

                # wo0/wo1 load right after S_3's first K/V group; wo2/wo3
                # trail via the ride slots they're needed in
                o_post.extend([mk_wo(0), mk_wo(1)])
                rides[2].append(mk_wo(2))
                rides[10].append(mk_wo(3))
                o_work = [(t, k) for t in range(len(o_tiles)) for k in range(4)]
                for idx, item in enumerate(o_work):
                    rides[6 + idx // 3].append(mk_o(item))

            psc_t = psc.tile([128, S], F32, name=f"psc{h}", tag="psc")
            cur = load_group(h, 0)
            for fn_ in o_post:
                fn_()
            if not (h == 0 and ride_kv) and npos_g == 0 and npos_lt < LC:
                new_entry_writes(h, *cur)
            nxt = None
            pend = None              # lag-1 ctx: (lt, wt, vst)
            for lt in range(LC):
                g, j = lt // 8, lt % 8
                if j == 0 and g > 0:
                    cur = nxt
                if j == 0 and g + 1 < (LC + 7) // 8:
                    nxt = load_group(h, g + 1)
                    if not (h == 0 and ride_kv) and npos_g == g + 1:
                        new_entry_writes(h, *nxt)
                kt8, v8 = cur
                if h == 0 and ride_kv and lt == npos_lt:
                    # kvrow writes were emitted at lt <= npos_lt - 1
                    new_entry_writes(h, kt8, v8) if npos_g == g else None
                    if npos_g == g + 1 and nxt is not None:
                        new_entry_writes(h, *nxt)

                ps = pss.tile([128, 1024], F32, name=f"ps_{h}_{lt}", tag="pss")
                ksl = kt8[:, j * 128:(j + 1) * 128]
                nc.tensor.matmul(ps[:, 0:512], ksl, qT_t[:, 0:512])
                nc.tensor.matmul(ps[:, 512:1024], ksl, qT_t[:, 512:1024])

                for emit in rides[lt]:
                    emit()

                wt = wt_pool.tile([128, 1024], F32R, name=f"wt_{h}_{lt}", tag="wt")
                ssum = ss_pool.tile([128, 1], F32, name=f"ss_{h}_{lt}", tag="ssum")
                nc.scalar.activation(wt[:], ps[:], AF.Exp, scale=INV, accum_out=ssum[:])
                rec = ss_pool.tile([128, 1], F32, name=f"rc_{h}_{lt}", tag="rec")
                nc.vector.reciprocal(rec[:], ssum[:])
                vst = vs_pool.tile([128, DK], F32R, name=f"vs{h}_{lt}", tag="vs")
                nc.vector.tensor_scalar_mul(vst[:], v8[:, j * 128:(j + 1) * 128], rec[:])

                if pend is not None:
                    plt, pwt, pvst = pend
                    nc.tensor.matmul(psc_t[:, 0:512], pvst[:], pwt[:, 0:512],
                                     start=(plt == 0), stop=False)
                    nc.tensor.matmul(psc_t[:, 512:1024], pvst[:], pwt[:, 512:1024],
                                     start=(plt == 0), stop=False)
                pend = (lt, wt, vst)
            plt, pwt, pvst = pend
            nc.tensor.matmul(psc_t[:, 0:512], pvst[:], pwt[:, 0:512],
                             start=(plt == 0), stop=True)
            nc.tensor.matmul(psc_t[:, 512:1024], pvst[:], pwt[:, 512:1024],
                             start=(plt == 0), stop=True)
            nc.vector.tensor_copy(ctxTs[h][:], psc_t[:])
            if h + 1 < HP and not ride_q:
                # dense fallback Q projection for the next head
                bq1 = ss_pool.tile([128, 1], F32, name=f"bq{h+1}", tag="bq",
                                   bufs=2)
                nc.sync.dma_start(bq1[:], bq_d[h + 1])
                qT_next = qT_pool.tile([128, S], F16, name=f"qT{h+1}", tag="qT")
                wqts_fb = {}
                for half in range(2):
                    psq_t = psq.tile([128, 512], F32,
                                     name=f"psq{h+1}_{half}", tag="psq")
                    for c in range(DC):
                        if half == 0 and c % 4 == 0:
                            wqts_fb[c // 4] = emit_wq_dma(h + 1, c // 4)
                        q_half_mm(psq_t, wqts_fb[c // 4], c, half)
                    q_half_add(h + 1, qT_next, psq_t, half, bq1)
            if h + 1 < HP:
                qT_t = qT_next

        # release attention-phase pools before the output projection (LIFO)
        for p in (psc, pss, kv_pool, psq,
                  ss_pool, vs_pool, wt_pool, v_pool, kt_pool,
                  wkv_pool, wq_pool, small, qT_pool, xT_pool):
            p.release()

        # ---------- output projection: out[s, m] partial ----------
        # Wo fully resident in the space freed by xT; one 16KB-burst output
        # DMA per s-tile.
        ob_pool = tc.alloc_tile_pool(name="obp", bufs=3)
        pso = tc.alloc_tile_pool(name="pso", bufs=4, space="PSUM")
        if not o_staged:
            # fallback path (short sequences): load Wo here
            wos = []
            for c in range(HP):
                wot = wo_pool.tile([128, D], F16, name=f"wo{c}", tag=f"wo{c}")
                nc.sync.dma_start(wot[:], wo_d[c * 128:(c + 1) * 128, :])
                wos.append(wot)
        for s_t in range(S // 128):
            ob = ob_pool.tile([128, D], F16, name=f"ob{s_t}", tag="ob")
            for mg in range(D // 512):
                sg = o_staged.get((s_t, mg))
                pso_t = pso.tile([128, 512], F32, name=f"po{s_t}_{mg}", tag="pso")
                if sg is not None:
                    nc.tensor.matmul(pso_t[:],
                                     ctxTs[HP - 1][:, s_t * 128:(s_t + 1) * 128],
                                     wos[HP - 1][:, mg * 512:(mg + 1) * 512])
                    nc.vector.tensor_add(ob[:, mg * 512:(mg + 1) * 512],
                                         sg[:], pso_t[:])
                else:
                    for c in range(HP):
                        nc.tensor.matmul(pso_t[:],
                                         ctxTs[c][:, s_t * 128:(s_t + 1) * 128],
                                         wos[c][:, mg * 512:(mg + 1) * 512],
                                         start=(c == 0), stop=(c == HP - 1))
                    nc.vector.tensor_copy(ob[:, mg * 512:(mg + 1) * 512], pso_t[:])
            if s_t == S // 128 - 1:
                # stream the final tile's output per mg-pair: the exposed
                # post-compute transfer shrinks to a quarter row-band
                for q in range(8):
                    nc.sync.dma_start(
                        out_d[s_t * 128:(s_t + 1) * 128,
                              q * (D // 8):(q + 1) * (D // 8)],
                        ob[:, q * (D // 8):(q + 1) * (D // 8)])
            else:
                nc.sync.dma_start(out_d[s_t * 128:(s_t + 1) * 128, :], ob[:])
        for p in (pso, ob_pool, stage_pool, wo_pool, ctxT_pool):
            p.release()

    nc.compile()
    return nc


# e5m2: Wk/Wv entries (sigma ~1/64) stay normal (min normal 2^-14), so no
# pre-scaling is needed and the bias-add stays a plain tensor_add. The new
# cache entry is 1 of 4096 rows, so its ~7% quantization error contributes
# ~0.1% to the context.
FP8 = mybir.dt.float8e5


def build_fast(pos: int):
    """Specialized build for the harness regime (pos=4095, L=4096=DC*128).

    Structural changes vs ``build``:
      - Phase A projects heads 0..2 concurrently (6 PSUM banks), paced by the
        interleaved wq/x DMA stream, with x group 0 split into 4 chunk DMAs so
        the first matmul starts ~2us in. Head 3's Q projection rides inside
        head 0's S loop as before.
      - kv_new runs TRANSPOSED: stationary = fp8 wkv [128d x 128col] chunks,
        moving = fp8 x_last chunk [128,1], out = one PSUM column per
        (head, k/v). 256 rank-1 matmuls cost ~0.4ns each in PE time vs the
        13.6us the 64 row-form N=512 matmuls cost.
      - No staged-O riding: the output projection runs as a single PE-bound
        phase at the end with the psum->SBUF copies on the ACT engine (idle
        there), so PE never waits on DVE.
      - Last output tile DMA'd in 4 chunks so only ~1 chunk is exposed.
    """
    L = pos + 1
    LC = L // 128
    NG = LC // 8
    npos_g = pos // 1024
    INV = 1.0 / math.sqrt(DK)
    assert LC == DC and S == 1024 and npos_g == NG - 1

    nc = bacc.Bacc("TRN2", target_bir_lowering=False, debug=False,
                   num_devices=NCORES)

    xT_d = nc.dram_tensor("xT", [D, S], F16, kind="ExternalInput").ap()
    wq_d = nc.dram_tensor("wq", [HP, D, DK], F16, kind="ExternalInput").ap()
    wkv_d = nc.dram_tensor("wkv", [D, 2 * HP * DK], FP8, kind="ExternalInput").ap()
    xl_d = nc.dram_tensor("xl", [128, DC], FP8, kind="ExternalInput").ap()
    bq_d = nc.dram_tensor("bq", [DK, HP], F32, kind="ExternalInput").ap()
    bkv_d = nc.dram_tensor("bkv", [DK, 2 * HP], F32, kind="ExternalInput").ap()
    kT_d = nc.dram_tensor("kT", [HP, DK, pos], F16, kind="ExternalInput").ap()
    v_d = nc.dram_tensor("v", [HP, pos, DK], F16, kind="ExternalInput").ap()
    wo_d = nc.dram_tensor("wo", [HP * DK, D], F16, kind="ExternalInput").ap()
    out_d = nc.dram_tensor("out", [S, D], F16, kind="ExternalOutput").ap()

    with tile.TileContext(nc) as tc:
        ctxT_pool = tc.alloc_tile_pool(name="ctxT", bufs=1)
        wo_pool = tc.alloc_tile_pool(name="wop", bufs=1)
        xT_pool = tc.alloc_tile_pool(name="xT", bufs=1)
        qT_pool = tc.alloc_tile_pool(name="qT", bufs=4)
        small = tc.alloc_tile_pool(name="smallp", bufs=1)
        sg_pool = tc.alloc_tile_pool(name="sgp", bufs=1)
        wq_pool = tc.alloc_tile_pool(name="wqp", bufs=12)
        kt_pool = tc.alloc_tile_pool(name="ktp", bufs=4)
        v_pool = tc.alloc_tile_pool(name="vp", bufs=4)
        wt_pool = tc.alloc_tile_pool(name="wtp", bufs=6)
        vs_pool = tc.alloc_tile_pool(name="vsp", bufs=7)
        ss_pool = tc.alloc_tile_pool(name="ssp", bufs=8)

        # PSUM (8 banks): psq 1 + kv 1 + pss 2x[128,1024] (4) + psc 2.
        # Phase A borrows pss slot0/slot1 + psc for the three Q accumulators.
        psq = tc.alloc_tile_pool(name="psq", bufs=1, space="PSUM")
        kv_pool = tc.alloc_tile_pool(name="kvp", bufs=1, space="PSUM")
        pss = tc.alloc_tile_pool(name="pss", bufs=2, space="PSUM")
        psc = tc.alloc_tile_pool(name="psc", bufs=1, space="PSUM")
        # top of stack: released after the kv rides (start of h2's S loop)
        # to make room for the staged-O sg tiles
        wkv_pool = tc.alloc_tile_pool(name="wkvp", bufs=5)

        ctxTs = [ctxT_pool.tile([128, S], F16, name=f"cT{h}", tag=f"cT{h}")
                 for h in range(HP)]

        # ---------- phase A: DMA emissions ----------
        wq_tiles = {}

        def emit_wq(h, g):
            t = wq_pool.tile([128, 4 * DK], F16, name=f"wq{h}_{g}", tag="wq")
            nc.sync.dma_start(
                t[:], wq_d[h, g * 512:(g + 1) * 512, :].rearrange(
                    "(i p) k -> p i k", p=128))
            wq_tiles[(h, g)] = t

        xbig = [xT_pool.tile([128, 4 * S], F16, name=f"xt{g}", tag=f"xt{g}")
                for g in range(DC // 4)]
        emit_wq(0, 0)
        nc.sync.dma_start(xbig[0][:, 0:512], xT_d[0:128, 0:512])
        nc.sync.dma_start(xbig[0][:, 512:S], xT_d[0:128, 512:S])
        for i in range(1, 4):  # group 0 split per chunk for a fast start
            nc.sync.dma_start(xbig[0][:, i * S:(i + 1) * S],
                              xT_d[i * 128:(i + 1) * 128, :])
        emit_wq(1, 0)
        emit_wq(2, 0)
        for h in range(3):
            emit_wq(h, 1)
        bq_t = small.tile([DK, HP], F32, name="bqall", tag="bqall")
        bkv_t = small.tile([DK, 2 * HP], F32, name="bkvt", tag="bkvt")
        xl_t = small.tile([128, DC], FP8, name="xlt", tag="xlt")
        for g in range(1, DC // 4):
            # x leads its segment (PE unblocks on it); the wq tiles for
            # group g+1 ride behind, arriving a full segment early
            nc.sync.dma_start(
                xbig[g][:, 0:2 * S],
                xT_d[g * 512:g * 512 + 256, :].rearrange(
                    "(i p) s -> p i s", p=128))
            nc.sync.dma_start(
                xbig[g][:, 2 * S:4 * S],
                xT_d[g * 512 + 256:(g + 1) * 512, :].rearrange(
                    "(i p) s -> p i s", p=128))
            if g + 1 < DC // 4:
                for h in range(3):
                    emit_wq(h, g + 1)
            if g == 3:
                # small constants ride behind the early x groups; needed
                # first at the phase-A bias adds / S-h0 kv rides
                nc.sync.dma_start(bq_t[:], bq_d[:])
                nc.sync.dma_start(bkv_t[:], bkv_d[:])
                nc.sync.dma_start(xl_t[:], xl_d[:])

        def xsl(c, lo, sz):
            return xbig[c // 4][:, (c % 4) * S + lo:(c % 4) * S + lo + sz]

        # ---------- phase A: compute emissions ----------
        qAcc = [pss.tile([128, S], F32, name="qacc0", tag="pss"),
                pss.tile([128, S], F32, name="qacc1", tag="pss"),
                psc.tile([128, S], F32, name="qacc2", tag="psc")]
        qTs = [qT_pool.tile([128, S], F16, name=f"qT{h}", tag="qT")
               for h in range(HP)]
        for g in range(DC // 4):
            for h in range(3):
                for c in range(4 * g, 4 * g + 4):
                    lhs = wq_tiles[(h, g)][:, (c % 4) * DK:(c % 4 + 1) * DK]
                    nc.tensor.matmul(qAcc[h][:, 0:512], lhs, xsl(c, 0, 512),
                                     start=(c == 0), stop=(c == DC - 1))
                    nc.tensor.matmul(qAcc[h][:, 512:1024], lhs, xsl(c, 512, 512),
                                     start=(c == 0), stop=(c == DC - 1))
        for h in range(3):
            for hf in range(2):
                nc.vector.tensor_scalar_add(
                    qTs[h][:, hf * 512:(hf + 1) * 512],
                    qAcc[h][:, hf * 512:(hf + 1) * 512], bq_t[:, h:h + 1])

        # ---------- post-phase-A DMA block (queue order matters) ----------
        def load_group(h, g):
            g0 = g * 1024
            gl = min(1024, L - g0)
            gc = max(0, min(1024, pos - g0))
            kt8 = kt_pool.tile([128, 1024], F16, name=f"kt{h}_{g}", tag="kt")
            if gc > 0:
                nc.sync.dma_start(kt8[:, 0:gc], kT_d[h, :, g0:g0 + gc])
            if gl < 1024:
                nc.vector.memset(kt8[:, gl:1024], 0.0)
            v8 = v_pool.tile([128, 1024], F16, name=f"v{h}_{g}", tag="v")
            if gl < 1024:
                nc.vector.memset(v8[:, (gl // 128) * 128:1024], 0.0)
            fc = gc // 128
            if fc > 0:
                nc.sync.dma_start(
                    v8[:, 0:fc * 128],
                    v_d[h, g0:g0 + fc * 128, :].rearrange(
                        "(i p) k -> p i k", p=128))
            rem = gc - fc * 128
            if rem > 0:
                nc.sync.dma_start(v8[0:rem, fc * 128:(fc + 1) * 128],
                                  v_d[h, g0 + fc * 128:g0 + gc, :])
            return kt8, v8

        groups0 = [load_group(0, 0)]
        for g in range(DC // 4):
            emit_wq(3, g)
        for g in range(1, NG):
            groups0.append(load_group(0, g))
        wkv_tiles = []
        for jt in range(8):
            t = wkv_pool.tile([128, 4 * 2 * HP * DK], FP8,
                              name=f"wkv{jt}", tag="wkv")
            nc.sync.dma_start(
                t[:], wkv_d[jt * 512:(jt + 1) * 512, :].rearrange(
                    "(i p) k -> p i k", p=128))
            wkv_tiles.append(t)
        wos = []
        for cblk in range(HP):
            wot = wo_pool.tile([128, D], F16, name=f"wo{cblk}", tag=f"wo{cblk}")
            nc.sync.dma_start(wot[:], wo_d[cblk * 128:(cblk + 1) * 128, :])
            wos.append(wot)

        # ---------- S loops ----------
        kvf = small.tile([DK, 2 * HP], F16, name="kvf", tag="kvf")
        gp = pos % 1024

        def new_entry_writes(h, kt8, v8):
            nc.sync.dma_start(kt8[:, gp:gp + 1], kvf[:, h:h + 1])
            nc.sync.dma_start(
                v8[gp % 128:gp % 128 + 1,
                   (gp // 128) * 128:(gp // 128 + 1) * 128],
                kvf[:, HP + h:HP + h + 1])

        kvacc = {}
        q3state = {}
        o_staged = {}        # (s_t, mg) -> (sg_tile, chunks_staged)
        o2state = {}

        def mk_o(loop_h, t, ck, nck):
            # staged-O ride: accumulate the first `nck` Wo chunks for tile t
            # (s_t, mg) into a spare PSUM bank, stage to SBUF; the O phase
            # finishes the remaining chunks and adds. h2 stages 2 chunks
            # (ctx0/1 known), h3 stages 3.
            base = 0 if loop_h == 2 else 16
            s_t, mg = divmod(base + t, 8)

            def emit():
                if ck == 0:
                    pool, tag = ((psq, "psq") if t % 2 == 0 else
                                 (kv_pool, "kv"))
                    o2state[(loop_h, t)] = pool.tile(
                        [128, 512], F32, name=f"o{loop_h}_{t}", tag=tag)
                acc = o2state[(loop_h, t)]
                nc.tensor.matmul(acc[:],
                                 ctxTs[ck][:, s_t * 128:(s_t + 1) * 128],
                                 wos[ck][:, mg * 512:(mg + 1) * 512],
                                 start=(ck == 0), stop=(ck == nck - 1))
                if ck == nck - 1:
                    sg = sg_pool.tile([128, 512], F16, name=f"sg{s_t}_{mg}",
                                      tag=f"sg{s_t}_{mg}")
                    nc.vector.tensor_copy(sg[:], acc[:])
                    o_staged[(s_t, mg)] = (sg, nck)
            return emit

        def mk_q3(half, c):
            # one chunk-matmul of head 3's Q per ride slot: half 0 rides in
            # h0's loop, half 1 in h1's — both loops stay just above ACT pace
            def emit():
                if c == 0:
                    q3state["psq"] = psq.tile([128, 512], F32,
                                              name=f"psq3_{half}", tag="psq")
                lhs = wq_tiles[(3, c // 4)][:, (c % 4) * DK:(c % 4 + 1) * DK]
                nc.tensor.matmul(q3state["psq"][:], lhs, xsl(c, half * 512, 512),
                                 start=(c == 0), stop=(c == DC - 1))
                if c == DC - 1:
                    nc.vector.tensor_scalar_add(
                        qTs[3][:, half * 512:(half + 1) * 512],
                        q3state["psq"][:], bq_t[:, 3:4])
            return emit

        for h in range(HP):
            rides = [[] for _ in range(LC)]
            if h in (0, 1):
                for lt in range(DC):
                    rides[lt].append(mk_q3(h, lt))
            if h == 2:
                # wkv is dead after h0's kv rides
                wkv_pool.release()
                # light staged-O riding: ~100ns/lt of DVE headroom under the
                # ACT pace allows one sg copy every ~6 lt
                for t in range(5):
                    for ck in range(2):
                        rides[6 * t + 2 * ck].append(mk_o(2, t, ck, 2))
            if h == 3:
                for t in range(5):
                    for ck in range(3):
                        rides[6 * t + 2 * ck].append(mk_o(3, t, ck, 3))
            if h == 0:

                def mk_kv(ci):
                    def emit():
                        if ci == 0:
                            kvacc["t"] = kv_pool.tile(
                                [128, 2 * HP], F32, name="kvt", tag="kv")
                        kt = kvacc["t"]
                        wt_ = wkv_tiles[ci]
                        for cc in range(4):
                            c = 4 * ci + cc
                            for jj in range(2 * HP):
                                nc.tensor.matmul(
                                    kt[:, jj:jj + 1],
                                    wt_[:, cc * 1024 + jj * DK:
                                        cc * 1024 + (jj + 1) * DK],
                                    xl_t[:, c:c + 1],
                                    start=(c == 0), stop=(c == DC - 1))
                    return emit

                for ci in range(8):
                    rides[2 + ci].append(mk_kv(ci))

                def kv_finish():
                    nc.vector.tensor_add(kvf[:], kvacc["t"][:], bkv_t[:])
                rides[10].append(kv_finish)

            psc_t = psc.tile([128, S], F32, name=f"psc{h}", tag="psc")
            cur = groups0[0] if h == 0 else load_group(h, 0)
            qT_t = qTs[h]
            nxt = None
            pends = []

            def ctx_mm(item, stop):
                plt, pwt, pvst = item
                nc.tensor.matmul(psc_t[:, 0:512], pvst[:], pwt[:, 0:512],
                                 start=(plt == 0), stop=stop)
                nc.tensor.matmul(psc_t[:, 512:1024], pvst[:], pwt[:, 512:1024],
                                 start=(plt == 0), stop=stop)

            for lt in range(LC):
                g, j = lt // 8, lt % 8
                if j == 0 and g > 0:
                    cur = nxt
                if j == 0 and g + 1 < NG:
                    nxt = groups0[g + 1] if h == 0 else load_group(h, g + 1)
                    if npos_g == g + 1:
                        new_entry_writes(h, *nxt)
                kt8, v8 = cur

                ps = pss.tile([128, 1024], F32, name=f"ps_{h}_{lt}", tag="pss")
                ksl = kt8[:, j * 128:(j + 1) * 128]
                nc.tensor.matmul(ps[:, 0:512], ksl, qT_t[:, 0:512])
                nc.tensor.matmul(ps[:, 512:1024], ksl, qT_t[:, 512:1024])

                for emit in rides[lt]:
                    emit()

                wt = wt_pool.tile([128, 1024], F16, name=f"wt_{h}_{lt}", tag="wt")
                ssum = ss_pool.tile([128, 1], F32, name=f"ss_{h}_{lt}", tag="ssum")
                # Row-sum fully on DVE (pairwise f16 add-tree + short
                # reduce, ~0.94us/lt): keeps ACT at a jitter-free steady
                # 1.04us/lt exp cadence — an ACT accum_out would make its
                # service time exceed the loop period and cascade stalls
                # through the 2-deep score-bank rotation.
                nc.scalar.activation(wt[:], ps[:], AF.Exp, scale=INV)
                r1 = ss_pool.tile([128, 512], F16, name=f"r1_{h}_{lt}",
                                  tag="r1", bufs=2)
                nc.vector.tensor_add(r1[:], wt[:, 0:512], wt[:, 512:1024])
                r2 = ss_pool.tile([128, 256], F16, name=f"r2_{h}_{lt}",
                                  tag="r2", bufs=2)
                nc.vector.tensor_add(r2[:], r1[:, 0:256], r1[:, 256:512])
                r3 = ss_pool.tile([128, 128], F16, name=f"r3_{h}_{lt}",
                                  tag="r3", bufs=2)
                nc.vector.tensor_add(r3[:], r2[:, 0:128], r2[:, 128:256])
                nc.vector.tensor_reduce(ssum[:], r3[:], mybir.AxisListType.X,
                                        mybir.AluOpType.add)
                rec = ss_pool.tile([128, 1], F32, name=f"rc_{h}_{lt}", tag="rec")
                nc.vector.reciprocal(rec[:], ssum[:])
                vst = vs_pool.tile([128, DK], F16, name=f"vs{h}_{lt}", tag="vs")
                nc.vector.tensor_scalar_mul(vst[:], v8[:, j * 128:(j + 1) * 128],
                                            rec[:])

                # lag-3: the exp -> add-tree -> reduce -> recip -> scale
                # chain is ~2.1us, over two loop periods; a shorter lag
                # would stall PE on vst
                pends.append((lt, wt, vst))
                if len(pends) > 4:
                    ctx_mm(pends.pop(0), stop=False)
            while len(pends) > 1:
                ctx_mm(pends.pop(0), stop=False)
            plt, pwt, pvst = pends.pop(0)
            nc.tensor.matmul(psc_t[:, 0:512], pvst[:], pwt[:, 0:512],
                             start=(plt == 0), stop=True)
            nc.vector.tensor_copy(ctxTs[h][:, 0:512], psc_t[:, 0:512])
            nc.tensor.matmul(psc_t[:, 512:1024], pvst[:], pwt[:, 512:1024],
                             start=(plt == 0), stop=True)
            nc.vector.tensor_copy(ctxTs[h][:, 512:1024], psc_t[:, 512:1024])

        # release attention-phase pools before the output projection (LIFO;
        # wkv was already released at the start of h2's loop)
        for p in (psc, pss, kv_pool, psq,
                  ss_pool, vs_pool, wt_pool, v_pool, kt_pool, wq_pool):
            p.release()

        # ---------- output projection ----------
        ob_pool = tc.alloc_tile_pool(name="obp", bufs=3)
        pso = tc.alloc_tile_pool(name="pso", bufs=6, space="PSUM")
        last_t = S // 128 - 1

        def emit_o_tile(s_t, mg, ob):
            obsl = ob[:, mg * 512:(mg + 1) * 512]
            staged = o_staged.get((s_t, mg))
            pso_t = pso.tile([128, 512], F32, name=f"po{s_t}_{mg}", tag="pso")
            c0 = staged[1] if staged else 0
            for cblk in range(c0, HP):
                nc.tensor.matmul(pso_t[:],
                                 ctxTs[cblk][:, s_t * 128:(s_t + 1) * 128],
                                 wos[cblk][:, mg * 512:(mg + 1) * 512],
                                 start=(cblk == c0), stop=(cblk == HP - 1))
            if staged:
                # DVE is idle in the O phase; ACT carries the plain copies
                nc.vector.tensor_add(obsl, staged[0][:], pso_t[:])
            elif s_t == last_t and mg >= 6:
                # last tiles: copy on DVE so the final copy runs parallel
                # to ACT's mg6 copy, shortening the end-of-kernel chain
                nc.vector.tensor_copy(obsl, pso_t[:])
            else:
                nc.scalar.activation(obsl, pso_t[:], AF.Copy)

        for s_t in range(S // 128):
            ob = ob_pool.tile([128, D], F16, name=f"ob{s_t}", tag="ob")
            for mg in range(D // 512):
                emit_o_tile(s_t, mg, ob)
                if s_t == last_t and mg >= 2:
                    # stream the final tile as it completes (HWDGE executes
                    # in order, so the big head chunk is emitted first and
                    # only the last 128KB chunk's transfer is exposed)
                    lo = 0 if mg == 2 else mg * 512
                    nc.sync.dma_start(
                        out_d[s_t * 128:(s_t + 1) * 128, lo:(mg + 1) * 512],
                        ob[:, lo:(mg + 1) * 512])
            if s_t != last_t:
                nc.sync.dma_start(out_d[s_t * 128:(s_t + 1) * 128, :], ob[:])
        for p in (pso, ob_pool, sg_pool, small, qT_pool, xT_pool,
                  wo_pool, ctxT_pool):
            p.release()

    nc.compile()
    return nc


_CACHE = {}
LAST_EXEC_NS = None


def kernel(x, k_cache, v_cache, Wq, bq, Wk, bk, Wv, bv, Wo, bo, pos):
    global LAST_EXEC_NS
    pos = int(pos)

    def f32(a):
        return np.ascontiguousarray(np.asarray(a), dtype=np.float32)

    x = f32(x)
    k_cache, v_cache = f32(k_cache), f32(v_cache)
    Wq, Wk, Wv, Wo = f32(Wq), f32(Wk), f32(Wv), f32(Wo)
    bq, bk, bv, bo = f32(bq), f32(bk), f32(bv), f32(bo)

    fast = (pos + 1 == 4096 and x.shape == (1, S, D))
    xT = np.ascontiguousarray(x[0].T.astype(np.float16))   # [D, S]
    in_maps = []
    for i in range(NCORES):
        hs = slice(i * HP, (i + 1) * HP)
        m = {
            "xT": xT,
            "wq": np.ascontiguousarray(Wq[hs].astype(np.float16)),
            "kT": np.ascontiguousarray(
                k_cache[hs, :pos, :].transpose(0, 2, 1).astype(np.float16)),
            "v": np.ascontiguousarray(v_cache[hs, :pos, :].astype(np.float16)),
            "wo": np.ascontiguousarray(
                Wo[i * HP * DK:(i + 1) * HP * DK].astype(np.float16)),
        }
        wkv_f32 = np.concatenate([
            Wk[hs].transpose(1, 0, 2).reshape(D, HP * DK),
            Wv[hs].transpose(1, 0, 2).reshape(D, HP * DK)], axis=1)
        if fast:
            m["wkv"] = np.ascontiguousarray(
                wkv_f32.astype(ml_dtypes.float8_e5m2))
            m["xl"] = np.ascontiguousarray(
                x[0, -1].reshape(DC, 128).T.astype(ml_dtypes.float8_e5m2))
            m["bq"] = np.ascontiguousarray(bq[hs].T)                 # [DK, HP]
            m["bkv"] = np.ascontiguousarray(
                np.concatenate([bk[hs].T, bv[hs].T], axis=1))        # [DK, 2HP]
        else:
            m["wkv"] = np.ascontiguousarray(wkv_f32.astype(ml_dtypes.bfloat16))
            m["xl"] = np.ascontiguousarray(
                x[0, -1].reshape(DC, 128).T.astype(ml_dtypes.bfloat16))
            m["bq"] = np.ascontiguousarray(bq[hs].reshape(HP, DK, 1))
            m["bkv"] = np.ascontiguousarray(np.concatenate(
                [bk[hs].reshape(-1), bv[hs].reshape(-1)])[None, :])
        in_maps.append(m)

    if pos not in _CACHE:
        _CACHE[pos] = build_fast(pos) if fast else build(pos)
    nc = _CACHE[pos]

    res = run_bass_kernel_spmd(nc, in_maps, core_ids=list(range(NCORES)))
    LAST_EXEC_NS = res.exec_time_ns

    acc = np.zeros((S, D), np.float64)
    for r in res.results:
        acc += r["out"]
    out = (acc + bo.astype(np.float64)).astype(np.float32)
    return out[None]



# revision 42
# speedup vs baseline: 1.0860x; 1.0044x over previous
"""Trainium2 Bass kernel for CachedMultiHeadedAttention (tensor-parallel over heads).

Sharding: 8 cores x 4 heads. Each core computes Q projection + attention for
its 4 heads, then a partial output projection against its 512 rows of Wo.
Host sums the 8 partial outputs (the "all-reduce" done at unshard time) and
adds bo.

Device-side layouts are chosen so NO on-chip transposes are needed:
  - x is passed pre-transposed (xT [D, S]) so contraction dims land on
    SBUF partitions for every matmul.
  - k_cache is passed pre-transposed per head (kT [DK, pos]).
  - The reference's softmax quirk (softmax over the QUERY axis) maps to
    scores^T tiles [l_part, s_free]: one fused ACT pass does exp + row-sum.
    The 1/sum normalization is folded into V rows (8x less data than the
    weight matrix).
Precision: streamed operands (x, Wq, k/v caches, Wo, qT, ctxT) are f16
(10 mantissa bits, ~5e-4 relative — full PE rate and half the DMA bytes of
f32r); softmax weights and scaled V run as float32r (full PE rate, unlike
plain fp32's 1/4 rate); all accumulation is f32 in PSUM, and the 8 partial
outputs are summed on the host in f64. The rank-1 k_new/v_new projections
run in bf16 — they only affect one of the 4096 cache rows. Measured
end-to-end relative error: ~6e-4.

Scheduling notes (cost-model-profiled):
  - Each dma_start costs ~625ns on the single serialized HWDGE queue, so
    DMAs are consolidated into ~130 large transfers (a naive version with
    557 DMAs spent 348us in HWDGE alone).
  - Engine queues execute in order, so the ACT-bound softmax loops carry
    "ride" work: the next head's Q-projection matmuls and (in head 0) the
    kv_new projections are emitted inside the S loop, paced per l-tile, with
    ctx matmuls lagged one iteration behind the exp that feeds them.
  - PSUM is exactly 8 banks: scores 2x[128,1024] (4) + ctx [128,1024] (2) +
    single-bank two-pass Q and kv_new accumulators (1+1).
"""

import math

import numpy as np
import ml_dtypes

import concourse.bass as bass
import concourse.mybir as mybir
import concourse.tile as tile
from concourse import bacc
from concourse.bass_utils import run_bass_kernel_spmd

F32 = mybir.dt.float32
F32R = mybir.dt.float32r
BF16 = mybir.dt.bfloat16
F16 = mybir.dt.float16
AF = mybir.ActivationFunctionType

H, D, DK, S = 32, 4096, 128, 1024
NCORES = 8
HP = H // NCORES          # heads per core
DC = D // 128             # contraction chunks for d_model


def build(pos: int):
    L = pos + 1
    LC = (L + 127) // 128          # number of 128-wide l tiles
    LG = (LC + 7) // 8             # l-tile groups of 8 (1024 l per group)
    INV = 1.0 / math.sqrt(DK)

    nc = bacc.Bacc("TRN2", target_bir_lowering=False, debug=False,
                   num_devices=NCORES)

    xT_d = nc.dram_tensor("xT", [D, S], F16, kind="ExternalInput").ap()
    wq_d = nc.dram_tensor("wq", [HP, D, DK], F16, kind="ExternalInput").ap()
    wkv_d = nc.dram_tensor("wkv", [D, 2 * HP * DK], BF16, kind="ExternalInput").ap()
    xl_d = nc.dram_tensor("xl", [128, DC], BF16, kind="ExternalInput").ap()
    bq_d = nc.dram_tensor("bq", [HP, DK, 1], F32, kind="ExternalInput").ap()
    bkv_d = nc.dram_tensor("bkv", [1, 2 * HP * DK], F32, kind="ExternalInput").ap()
    kT_d = nc.dram_tensor("kT", [HP, DK, pos], F16, kind="ExternalInput").ap()
    v_d = nc.dram_tensor("v", [HP, pos, DK], F16, kind="ExternalInput").ap()
    wo_d = nc.dram_tensor("wo", [HP * DK, D], F16, kind="ExternalInput").ap()
    out_d = nc.dram_tensor("out", [S, D], F16, kind="ExternalOutput").ap()

    with tile.TileContext(nc) as tc:
        # Pools are released LIFO; ctxT survives into the output projection,
        # so it sits at the bottom of the SBUF pool stack.
        ctxT_pool = tc.alloc_tile_pool(name="ctxT", bufs=1)
        wo_pool = tc.alloc_tile_pool(name="wop", bufs=1)
        stage_pool = tc.alloc_tile_pool(name="stagep", bufs=1)
        xT_pool = tc.alloc_tile_pool(name="xT", bufs=1)
        qT_pool = tc.alloc_tile_pool(name="qT", bufs=2)
        small = tc.alloc_tile_pool(name="smallp", bufs=1)
        wq_pool = tc.alloc_tile_pool(name="wqp", bufs=8)
        wkv_pool = tc.alloc_tile_pool(name="wkvp", bufs=3)
        kt_pool = tc.alloc_tile_pool(name="ktp", bufs=3)
        v_pool = tc.alloc_tile_pool(name="vp", bufs=3)
        wt_pool = tc.alloc_tile_pool(name="wtp", bufs=4)
        vs_pool = tc.alloc_tile_pool(name="vsp", bufs=4)
        ss_pool = tc.alloc_tile_pool(name="ssp", bufs=8)

        # PSUM budget (8 banks): psq 1 + kv 1 + pss 4 + psc 2.
        # Q projections and the kv_new projections run in TWO s-half /
        # k-v passes so their accumulators are single-bank.
        psq = tc.alloc_tile_pool(name="psq", bufs=1, space="PSUM")
        kv_pool = tc.alloc_tile_pool(name="kvp", bufs=1, space="PSUM")
        pss = tc.alloc_tile_pool(name="pss", bufs=2, space="PSUM")
        psc = tc.alloc_tile_pool(name="psc", bufs=1, space="PSUM")

        ctxTs = [ctxT_pool.tile([128, S], F16, name=f"cT{h}", tag=f"cT{h}")
                 for h in range(HP)]

        # small constants first (tiny DMAs, ahead of the big streams)
        kvrow = small.tile([1, 2 * HP * DK], F16, name="kvrow", tag="kvrow")
        bkv_t = small.tile([1, 2 * HP * DK], F32, name="bkvt", tag="bkvt")
        nc.sync.dma_start(bkv_t[:], bkv_d[:])
        xl_t = small.tile([128, DC], BF16, name="xlt", tag="xlt")
        nc.sync.dma_start(xl_t[:], xl_d[:])

        # resident xT tiles (8 big tiles of 4 chunks), interleaved with head
        # 0's Q weight groups so the first Q matmuls start after ~2.5MB, not
        # after the full 17MB of x.
        xbig = []
        wq0_groups = []
        for gx in range(DC // 4):
            wqt = wq_pool.tile([128, 4 * DK], F16, name=f"wq0_{gx}", tag="wq")
            nc.sync.dma_start(
                wqt[:], wq_d[0, gx * 512:(gx + 1) * 512, :].rearrange(
                    "(i p) k -> p i k", p=128))
            wq0_groups.append(wqt)
            xt = xT_pool.tile([128, 4 * S], F16, name=f"xt{gx}", tag=f"xt{gx}")
            nc.sync.dma_start(
                xt[:], xT_d[gx * 512:(gx + 1) * 512, :].rearrange(
                    "(i p) s -> p i s", p=128))
            xbig.append(xt)

        def xsl(c, lo, sz):
            return xbig[c // 4][:, (c % 4) * S + lo:(c % 4) * S + lo + sz]

        def emit_wq_dma(h, gw, tag="wq"):
            wqt = wq_pool.tile([128, 4 * DK], F16,
                               name=f"wq{h}_{gw}", tag=tag)
            nc.sync.dma_start(
                wqt[:], wq_d[h, gw * 512:(gw + 1) * 512, :].rearrange(
                    "(i p) k -> p i k", p=128))
            return wqt

        def q_half_mm(psq_t, wqt, c, half):
            lhs = wqt[:, (c % 4) * DK:(c % 4 + 1) * DK]
            nc.tensor.matmul(psq_t[:], lhs, xsl(c, half * 512, 512),
                             start=(c == 0), stop=(c == DC - 1))

        def q_half_add(h, qT_t, psq_t, half, bq_t):
            nc.vector.tensor_scalar_add(qT_t[:, half * 512:(half + 1) * 512],
                                        psq_t[:], bq_t[:])

        kv_cur = {}

        def kv_mm(kv_t, c, which):
            # which: 0 = k_new, 1 = v_new. Weight chunks are DMA'd two at a
            # time — each dma_start costs ~625ns of serialized HWDGE.
            if c % 4 == 0:
                wkvt = wkv_pool.tile([128, 4 * HP * DK], BF16,
                                     name=f"wkv{which}_{c}", tag="wkv")
                nc.sync.dma_start(
                    wkvt[:], wkv_d[c * 128:(c + 4) * 128,
                                   which * HP * DK:(which + 1) * HP * DK]
                    .rearrange("(i p) k -> p i k", p=128))
                kv_cur["t"] = wkvt
            wkvt = kv_cur["t"]
            nc.tensor.matmul(kv_t[0:1, :], xl_t[:, c:c + 1],
                             wkvt[:, (c % 4) * HP * DK:(c % 4 + 1) * HP * DK],
                             start=(c == 0), stop=(c == DC - 1))

        def kv_add(kv_t, which):
            nc.vector.tensor_add(
                kvrow[0:1, which * HP * DK:(which + 1) * HP * DK], kv_t[:],
                bkv_t[0:1, which * HP * DK:(which + 1) * HP * DK])

        def load_group(h, g):
            """Cache-only loads of l-group g (the new-entry writes are
            emitted separately, after kvrow's writes in trace order)."""
            g0 = g * 1024
            gl = min(1024, L - g0)            # valid l in group
            gc = max(0, min(1024, pos - g0))  # of which from cache
            kt8 = kt_pool.tile([128, 1024], F16, name=f"kt{h}_{g}", tag="kt")
            if gc > 0:
                nc.sync.dma_start(kt8[:, 0:gc], kT_d[h, :, g0:g0 + gc])
            if gl < 1024:
                nc.vector.memset(kt8[:, gl:1024], 0.0)
            v8 = v_pool.tile([128, 1024], F16, name=f"v{h}_{g}", tag="v")
            if gl < 1024:
                # zero whole padded chunks first (full partition range — DVE
                # requires 32-aligned partition bases); valid rows are DMA'd
                # over the zeros below.
                nc.vector.memset(v8[:, (gl // 128) * 128:1024], 0.0)
            fc = gc // 128
            if fc > 0:
                nc.sync.dma_start(
                    v8[:, 0:fc * 128],
                    v_d[h, g0:g0 + fc * 128, :].rearrange(
                        "(i p) k -> p i k", p=128))
            rem = gc - fc * 128
            if rem > 0:
                nc.sync.dma_start(v8[0:rem, fc * 128:(fc + 1) * 128],
                                  v_d[h, g0 + fc * 128:g0 + gc, :])
            return kt8, v8

        def new_entry_writes(h, kt8, v8):
            # column/row for l == pos from the biased kvrow
            gp = pos % 1024
            nc.sync.dma_start(kt8[:, gp:gp + 1],
                              kvrow[0:1, h * DK:(h + 1) * DK])
            nc.sync.dma_start(
                v8[gp % 128:gp % 128 + 1, (gp // 128) * 128:(gp // 128 + 1) * 128],
                kvrow[0:1, HP * DK + h * DK:HP * DK + (h + 1) * DK])

        npos_g = pos // 1024            # l-group holding the new entry
        npos_lt = pos // 128            # l-tile index holding the new entry
        # riding is only possible when the S loop is long enough for the
        # 2-instruction-per-lt passes to finish before the new entry is used
        ride_kv = LC >= DC and npos_lt >= 8
        ride_q = LC >= DC

        # ---------- head 0 Q projection (phase A, DMA-paced) ----------
        bq_t = ss_pool.tile([128, 1], F32, name="bq0", tag="bq", bufs=2)
        nc.sync.dma_start(bq_t[:], bq_d[0])
        qT_t = qT_pool.tile([128, S], F16, name="qT0", tag="qT")
        # both s-halves accumulate concurrently (pass B borrows the idle kv
        # bank) so the whole projection rides the x-arrival gaps instead of
        # serializing 6.8us of pass-B matmuls after the stream ends
        psq_a = psq.tile([128, 512], F32, name="psq0_0", tag="psq")
        psq_b = kv_pool.tile([128, 512], F32, name="psq0_1", tag="kv")
        for c in range(DC):
            q_half_mm(psq_a, wq0_groups[c // 4], c, 0)
            q_half_mm(psq_b, wq0_groups[c // 4], c, 1)
        q_half_add(0, qT_t, psq_a, 0, bq_t)
        q_half_add(0, qT_t, psq_b, 1, bq_t)

        if not ride_kv:
            # fallback: dense kv_new before the S loops
            for which in range(2):
                kv_t = kv_pool.tile([1, HP * DK], F32, name=f"kv{which}", tag="kv")
                for c in range(DC):
                    kv_mm(kv_t, c, which)
                kv_add(kv_t, which)

        for h in range(HP):
            # per-lt ride items emitted right after the scores matmuls
            rides = [[] for _ in range(LC)]
            if h + 1 < HP and ride_q:
                bq1 = ss_pool.tile([128, 1], F32, name=f"bq{h+1}", tag="bq",
                                   bufs=2)
                nc.sync.dma_start(bq1[:], bq_d[h + 1])
                qT_next = qT_pool.tile([128, S], F16, name=f"qT{h+1}", tag="qT")
                state = {}

                def mk_q(lt, h1=h + 1, qn=qT_next, bqt=bq1, st=state):
                    def emit():
                        half, c0 = divmod(2 * lt, DC)
                        if c0 == 0 and half == 0:
                            st["wqts"] = {}
                        if c0 == 0:
                            st["psq"] = psq.tile([128, 512], F32,
                                                 name=f"psq{h1}_{half}", tag="psq")
                        for c in (c0, c0 + 1):
                            gw = c // 4
                            if half == 0 and c % 4 == 0:
                                # pass B reuses these resident tiles (8 slots)
                                st["wqts"][gw] = emit_wq_dma(h1, gw)
                            q_half_mm(st["psq"], st["wqts"][gw], c, half)
                        if c0 + 1 == DC - 1:
                            q_half_add(h1, qn, st["psq"], half, bqt)
                    return emit

                for lt in range(DC):
                    rides[lt].append(mk_q(lt))
            if h == 0 and ride_kv:
                # kv_new work items, paced so both passes (and their kvrow
                # writes) are emitted strictly before lt == npos_lt
                kv_work = ([("mm", 0, c) for c in range(DC)] + [("add", 0, 0)]
                           + [("mm", 1, c) for c in range(DC)] + [("add", 1, 0)])
                kvstate = {}

                def kv_emit_one(item, st=kvstate):
                    kind, which, c = item
                    if kind == "add":
                        kv_add(st["kv"], which)
                        return
                    if c == 0:
                        st["kv"] = kv_pool.tile([1, HP * DK], F32,
                                                name=f"kv{which}", tag="kv")
                    kv_mm(st["kv"], c, which)

                n_slots = npos_lt - 1          # ride slots: lt 0..npos_lt-2
                n_pre = max(0, len(kv_work) - 2 * n_slots)
                for item in kv_work[:n_pre]:
                    kv_emit_one(item)
                rest = kv_work[n_pre:]
                for k, item in enumerate(rest):
                    rides[k // 2].append(
                        (lambda it=item: kv_emit_one(it)))

            o_staged = {}
            o_post = []
            if h == HP - 1 and LC >= DC:
                # S_3 has no Q to ride; its psq/kv PSUM banks are dead. Ride
                # the first-3-chunk partials of 16 output tiles there, staged
                # to SBUF; the O phase finishes them with one matmul + add.
                wos = [wo_pool.tile([128, D], F16, name=f"wo{c}", tag=f"wo{c}")
                       for c in range(HP)]

                def mk_wo(c):
                    return lambda: nc.sync.dma_start(
                        wos[c][:], wo_d[c * 128:(c + 1) * 128, :])

                o_tiles = [(s_t, mg) for s_t in (6, 7) for mg in range(D // 512)]
                o_state = {}

                def mk_o(item, st=o_state):
                    t, k = item
                    s_t, mg = o_tiles[t]

                    def emit():
                        if k == 0:
                            pool = kv_pool if t % 2 == 0 else psq
                            st["ps"] = pool.tile(
                                [128, 512], F32, name=f"ops{t}",
                                tag="kv" if t % 2 == 0 else "psq")
                        if k < 3:
                            nc.tensor.matmul(
                                st["ps"][:],
                                ctxTs[k][:, s_t * 128:(s_t + 1) * 128],
                                wos[k][:, mg * 512:(mg + 1) * 512],
                                start=(k == 0), stop=(k == 2))
                        else:
                            sg = stage_pool.tile([128, 512], F16,
                                                 name=f"sg{t}", tag=f"sg{t}")
                            nc.vector.tensor_copy(sg[:], st["ps"][:])
                            o_staged[(s_t, mg)] = sg
                    return emit

                # wo0/wo1 load right after S_3's first K/V group; wo2/wo3
                # trail via the ride slots they're needed in
                o_post.extend([mk_wo(0), mk_wo(1)])
                rides[2].append(mk_wo(2))
                rides[10].append(mk_wo(3))
                o_work = [(t, k) for t in range(len(o_tiles)) for k in range(4)]
                for idx, item in enumerate(o_work):
                    rides[6 + idx // 3].append(mk_o(item))

            psc_t = psc.tile([128, S], F32, name=f"psc{h}", tag="psc")
            cur = load_group(h, 0)
            for fn_ in o_post:
                fn_()
            if not (h == 0 and ride_kv) and npos_g == 0 and npos_lt < LC:
                new_entry_writes(h, *cur)
            nxt = None
            pend = None              # lag-1 ctx: (lt, wt, vst)
            for lt in range(LC):
                g, j = lt // 8, lt % 8
                if j == 0 and g > 0:
                    cur = nxt
                if j == 0 and g + 1 < (LC + 7) // 8:
                    nxt = load_group(h, g + 1)
                    if not (h == 0 and ride_kv) and npos_g == g + 1:
                        new_entry_writes(h, *nxt)
                kt8, v8 = cur
                if h == 0 and ride_kv and lt == npos_lt:
                    # kvrow writes were emitted at lt <= npos_lt - 1
                    new_entry_writes(h, kt8, v8) if npos_g == g else None
                    if npos_g == g + 1 and nxt is not None:
                        new_entry_writes(h, *nxt)

                ps = pss.tile([128, 1024], F32, name=f"ps_{h}_{lt}", tag="pss")
                ksl = kt8[:, j * 128:(j + 1) * 128]
                nc.tensor.matmul(ps[:, 0:512], ksl, qT_t[:, 0:512])
                nc.tensor.matmul(ps[:, 512:1024], ksl, qT_t[:, 512:1024])

                for emit in rides[lt]:
                    emit()

                wt = wt_pool.tile([128, 1024], F32R, name=f"wt_{h}_{lt}", tag="wt")
                ssum = ss_pool.tile([128, 1], F32, name=f"ss_{h}_{lt}", tag="ssum")
                nc.scalar.activation(wt[:], ps[:], AF.Exp, scale=INV, accum_out=ssum[:])
                rec = ss_pool.tile([128, 1], F32, name=f"rc_{h}_{lt}", tag="rec")
                nc.vector.reciprocal(rec[:], ssum[:])
                vst = vs_pool.tile([128, DK], F32R, name=f"vs{h}_{lt}", tag="vs")
                nc.vector.tensor_scalar_mul(vst[:], v8[:, j * 128:(j + 1) * 128], rec[:])

                if pend is not None:
                    plt, pwt, pvst = pend
                    nc.tensor.matmul(psc_t[:, 0:512], pvst[:], pwt[:, 0:512],
                                     start=(plt == 0), stop=False)
                    nc.tensor.matmul(psc_t[:, 512:1024], pvst[:], pwt[:, 512:1024],
                                     start=(plt == 0), stop=False)
                pend = (lt, wt, vst)
            plt, pwt, pvst = pend
            nc.tensor.matmul(psc_t[:, 0:512], pvst[:], pwt[:, 0:512],
                             start=(plt == 0), stop=True)
            nc.tensor.matmul(psc_t[:, 512:1024], pvst[:], pwt[:, 512:1024],
                             start=(plt == 0), stop=True)
            nc.vector.tensor_copy(ctxTs[h][:], psc_t[:])
            if h + 1 < HP and not ride_q:
                # dense fallback Q projection for the next head
                bq1 = ss_pool.tile([128, 1], F32, name=f"bq{h+1}", tag="bq",
                                   bufs=2)
                nc.sync.dma_start(bq1[:], bq_d[h + 1])
                qT_next = qT_pool.tile([128, S], F16, name=f"qT{h+1}", tag="qT")
                wqts_fb = {}
                for half in range(2):
                    psq_t = psq.tile([128, 512], F32,
                                     name=f"psq{h+1}_{half}", tag="psq")
                    for c in range(DC):
                        if half == 0 and c % 4 == 0:
                            wqts_fb[c // 4] = emit_wq_dma(h + 1, c // 4)
                        q_half_mm(psq_t, wqts_fb[c // 4], c, half)
                    q_half_add(h + 1, qT_next, psq_t, half, bq1)
            if h + 1 < HP:
                qT_t = qT_next

        # release attention-phase pools before the output projection (LIFO)
        for p in (psc, pss, kv_pool, psq,
                  ss_pool, vs_pool, wt_pool, v_pool, kt_pool,
                  wkv_pool, wq_pool, small, qT_pool, xT_pool):
            p.release()

        # ---------- output projection: out[s, m] partial ----------
        # Wo fully resident in the space freed by xT; one 16KB-burst output
        # DMA per s-tile.
        ob_pool = tc.alloc_tile_pool(name="obp", bufs=3)
        pso = tc.alloc_tile_pool(name="pso", bufs=4, space="PSUM")
        if not o_staged:
            # fallback path (short sequences): load Wo here
            wos = []
            for c in range(HP):
                wot = wo_pool.tile([128, D], F16, name=f"wo{c}", tag=f"wo{c}")
                nc.sync.dma_start(wot[:], wo_d[c * 128:(c + 1) * 128, :])
                wos.append(wot)
        for s_t in range(S // 128):
            ob = ob_pool.tile([128, D], F16, name=f"ob{s_t}", tag="ob")
            for mg in range(D // 512):
                sg = o_staged.get((s_t, mg))
                pso_t = pso.tile([128, 512], F32, name=f"po{s_t}_{mg}", tag="pso")
                if sg is not None:
                    nc.tensor.matmul(pso_t[:],
                                     ctxTs[HP - 1][:, s_t * 128:(s_t + 1) * 128],
                                     wos[HP - 1][:, mg * 512:(mg + 1) * 512])
                    nc.vector.tensor_add(ob[:, mg * 512:(mg + 1) * 512],
                                         sg[:], pso_t[:])
                else:
                    for c in range(HP):
                        nc.tensor.matmul(pso_t[:],
                                         ctxTs[c][:, s_t * 128:(s_t + 1) * 128],
                                         wos[c][:, mg * 512:(mg + 1) * 512],
                                         start=(c == 0), stop=(c == HP - 1))
                    nc.vector.tensor_copy(ob[:, mg * 512:(mg + 1) * 512], pso_t[:])
            if s_t == S // 128 - 1:
                # stream the final tile's output per mg-pair: the exposed
                # post-compute transfer shrinks to a quarter row-band
                for q in range(8):
                    nc.sync.dma_start(
                        out_d[s_t * 128:(s_t + 1) * 128,
                              q * (D // 8):(q + 1) * (D // 8)],
                        ob[:, q * (D // 8):(q + 1) * (D // 8)])
            else:
                nc.sync.dma_start(out_d[s_t * 128:(s_t + 1) * 128, :], ob[:])
        for p in (pso, ob_pool, stage_pool, wo_pool, ctxT_pool):
            p.release()

    nc.compile()
    return nc


# e5m2: Wk/Wv entries (sigma ~1/64) stay normal (min normal 2^-14), so no
# pre-scaling is needed and the bias-add stays a plain tensor_add. The new
# cache entry is 1 of 4096 rows, so its ~7% quantization error contributes
# ~0.1% to the context.
FP8 = mybir.dt.float8e5


def build_fast(pos: int):
    """Specialized build for the harness regime (pos=4095, L=4096=DC*128).

    Structural changes vs ``build``:
      - Phase A projects heads 0..2 concurrently (6 PSUM banks), paced by the
        interleaved wq/x DMA stream, with x group 0 split into 4 chunk DMAs so
        the first matmul starts ~2us in. Head 3's Q projection rides inside
        head 0's S loop as before.
      - kv_new runs TRANSPOSED: stationary = fp8 wkv [128d x 128col] chunks,
        moving = fp8 x_last chunk [128,1], out = one PSUM column per
        (head, k/v). 256 rank-1 matmuls cost ~0.4ns each in PE time vs the
        13.6us the 64 row-form N=512 matmuls cost.
      - No staged-O riding: the output projection runs as a single PE-bound
        phase at the end with the psum->SBUF copies on the ACT engine (idle
        there), so PE never waits on DVE.
      - Last output tile DMA'd in 4 chunks so only ~1 chunk is exposed.
    """
    L = pos + 1
    LC = L // 128
    NG = LC // 8
    npos_g = pos // 1024
    INV = 1.0 / math.sqrt(DK)
    assert LC == DC and S == 1024 and npos_g == NG - 1

    nc = bacc.Bacc("TRN2", target_bir_lowering=False, debug=False,
                   num_devices=NCORES)

    xT_d = nc.dram_tensor("xT", [D, S], F16, kind="ExternalInput").ap()
    wq_d = nc.dram_tensor("wq", [HP, D, DK], F16, kind="ExternalInput").ap()
    wkv_d = nc.dram_tensor("wkv", [D, 2 * HP * DK], FP8, kind="ExternalInput").ap()
    xl_d = nc.dram_tensor("xl", [128, DC], FP8, kind="ExternalInput").ap()
    bq_d = nc.dram_tensor("bq", [DK, HP], F32, kind="ExternalInput").ap()
    bkv_d = nc.dram_tensor("bkv", [DK, 2 * HP], F32, kind="ExternalInput").ap()
    kT_d = nc.dram_tensor("kT", [HP, DK, pos], F16, kind="ExternalInput").ap()
    v_d = nc.dram_tensor("v", [HP, pos, DK], F16, kind="ExternalInput").ap()
    wo_d = nc.dram_tensor("wo", [HP * DK, D], F16, kind="ExternalInput").ap()
    out_d = nc.dram_tensor("out", [S, D], F16, kind="ExternalOutput").ap()

    with tile.TileContext(nc) as tc:
        ctxT_pool = tc.alloc_tile_pool(name="ctxT", bufs=1)
        wo_pool = tc.alloc_tile_pool(name="wop", bufs=1)
        xT_pool = tc.alloc_tile_pool(name="xT", bufs=1)
        qT_pool = tc.alloc_tile_pool(name="qT", bufs=4)
        small = tc.alloc_tile_pool(name="smallp", bufs=1)
        sg_pool = tc.alloc_tile_pool(name="sgp", bufs=1)
        wq_pool = tc.alloc_tile_pool(name="wqp", bufs=12)
        kt_pool = tc.alloc_tile_pool(name="ktp", bufs=4)
        v_pool = tc.alloc_tile_pool(name="vp", bufs=4)
        wt_pool = tc.alloc_tile_pool(name="wtp", bufs=6)
        vs_pool = tc.alloc_tile_pool(name="vsp", bufs=7)
        ss_pool = tc.alloc_tile_pool(name="ssp", bufs=8)

        # PSUM (8 banks): psq 1 + kv 1 + pss 2x[128,1024] (4) + psc 2.
        # Phase A borrows pss slot0/slot1 + psc for the three Q accumulators.
        psq = tc.alloc_tile_pool(name="psq", bufs=1, space="PSUM")
        kv_pool = tc.alloc_tile_pool(name="kvp", bufs=1, space="PSUM")
        pss = tc.alloc_tile_pool(name="pss", bufs=2, space="PSUM")
        psc = tc.alloc_tile_pool(name="psc", bufs=1, space="PSUM")
        # top of stack: released after the kv rides (start of h2's S loop)
        # to make room for the staged-O sg tiles
        wkv_pool = tc.alloc_tile_pool(name="wkvp", bufs=5)

        ctxTs = [ctxT_pool.tile([128, S], F16, name=f"cT{h}", tag=f"cT{h}")
                 for h in range(HP)]

        # ---------- PE warm-up ----------
        # The cost model's p-state ramp needs ~3us of continuous PE activity
        # before matmuls hit full rate. A chain of dummy matmuls over a
        # memset tile bridges t=0.3us..3.5us, so every real phase-A matmul
        # runs warm. The dummy PSUM tile is never read; the next tile in the
        # slot starts with start=True which clears it.
        warm = small.tile([128, 512], F16, name="warm", tag="warm")
        nc.vector.memset(warm[:], 0.0)
        psw = pss.tile([128, 512], F32, name="psw", tag="pss")
        for i in range(7):
            nc.tensor.matmul(psw[:], warm[:, 0:128], warm[:],
                             start=True, stop=True)

        # ---------- phase A: DMA emissions ----------
        wq_tiles = {}

        def emit_wq(h, g):
            t = wq_pool.tile([128, 4 * DK], F16, name=f"wq{h}_{g}", tag="wq")
            nc.sync.dma_start(
                t[:], wq_d[h, g * 512:(g + 1) * 512, :].rearrange(
                    "(i p) k -> p i k", p=128))
            wq_tiles[(h, g)] = t

        xbig = [xT_pool.tile([128, 4 * S], F16, name=f"xt{g}", tag=f"xt{g}")
                for g in range(DC // 4)]
        emit_wq(0, 0)
        nc.sync.dma_start(xbig[0][:, 0:512], xT_d[0:128, 0:512])
        nc.sync.dma_start(xbig[0][:, 512:S], xT_d[0:128, 512:S])
        for i in range(1, 4):  # group 0 split per chunk for a fast start
            nc.sync.dma_start(xbig[0][:, i * S:(i + 1) * S],
                              xT_d[i * 128:(i + 1) * 128, :])
        emit_wq(1, 0)
        emit_wq(2, 0)
        for h in range(3):
            emit_wq(h, 1)
        bq_t = small.tile([DK, HP], F32, name="bqall", tag="bqall")
        bkv_t = small.tile([DK, 2 * HP], F32, name="bkvt", tag="bkvt")
        xl_t = small.tile([128, DC], FP8, name="xlt", tag="xlt")
        for g in range(1, DC // 4):
            # x leads its segment (PE unblocks on it); the wq tiles for
            # group g+1 ride behind, arriving a full segment early
            nc.sync.dma_start(
                xbig[g][:, 0:2 * S],
                xT_d[g * 512:g * 512 + 256, :].rearrange(
                    "(i p) s -> p i s", p=128))
            nc.sync.dma_start(
                xbig[g][:, 2 * S:4 * S],
                xT_d[g * 512 + 256:(g + 1) * 512, :].rearrange(
                    "(i p) s -> p i s", p=128))
            if g + 1 < DC // 4:
                for h in range(3):
                    emit_wq(h, g + 1)
            if g == 3:
                # small constants ride behind the early x groups; needed
                # first at the phase-A bias adds / S-h0 kv rides
                nc.sync.dma_start(bq_t[:], bq_d[:])
                nc.sync.dma_start(bkv_t[:], bkv_d[:])
                nc.sync.dma_start(xl_t[:], xl_d[:])

        def xsl(c, lo, sz):
            return xbig[c // 4][:, (c % 4) * S + lo:(c % 4) * S + lo + sz]

        # ---------- phase A: compute emissions ----------
        qAcc = [pss.tile([128, S], F32, name="qacc0", tag="pss"),
                pss.tile([128, S], F32, name="qacc1", tag="pss"),
                psc.tile([128, S], F32, name="qacc2", tag="psc")]
        qTs = [qT_pool.tile([128, S], F16, name=f"qT{h}", tag="qT")
               for h in range(HP)]
        for g in range(DC // 4):
            for h in range(3):
                for c in range(4 * g, 4 * g + 4):
                    lhs = wq_tiles[(h, g)][:, (c % 4) * DK:(c % 4 + 1) * DK]
                    nc.tensor.matmul(qAcc[h][:, 0:512], lhs, xsl(c, 0, 512),
                                     start=(c == 0), stop=(c == DC - 1))
                    nc.tensor.matmul(qAcc[h][:, 512:1024], lhs, xsl(c, 512, 512),
                                     start=(c == 0), stop=(c == DC - 1))
        for h in range(3):
            for hf in range(2):
                nc.vector.tensor_scalar_add(
                    qTs[h][:, hf * 512:(hf + 1) * 512],
                    qAcc[h][:, hf * 512:(hf + 1) * 512], bq_t[:, h:h + 1])

        # ---------- post-phase-A DMA block (queue order matters) ----------
        def load_group(h, g):
            g0 = g * 1024
            gl = min(1024, L - g0)
            gc = max(0, min(1024, pos - g0))
            kt8 = kt_pool.tile([128, 1024], F16, name=f"kt{h}_{g}", tag="kt")
            if gc > 0:
                nc.sync.dma_start(kt8[:, 0:gc], kT_d[h, :, g0:g0 + gc])
            if gl < 1024:
                nc.vector.memset(kt8[:, gl:1024], 0.0)
            v8 = v_pool.tile([128, 1024], F16, name=f"v{h}_{g}", tag="v")
            if gl < 1024:
                nc.vector.memset(v8[:, (gl // 128) * 128:1024], 0.0)
            fc = gc // 128
            if fc > 0:
                nc.sync.dma_start(
                    v8[:, 0:fc * 128],
                    v_d[h, g0:g0 + fc * 128, :].rearrange(
                        "(i p) k -> p i k", p=128))
            rem = gc - fc * 128
            if rem > 0:
                nc.sync.dma_start(v8[0:rem, fc * 128:(fc + 1) * 128],
                                  v_d[h, g0 + fc * 128:g0 + gc, :])
            return kt8, v8

        groups0 = [load_group(0, 0)]
        for g in range(DC // 4):
            emit_wq(3, g)
        for g in range(1, NG):
            groups0.append(load_group(0, g))
        wkv_tiles = []
        for jt in range(8):
            t = wkv_pool.tile([128, 4 * 2 * HP * DK], FP8,
                              name=f"wkv{jt}", tag="wkv")
            nc.sync.dma_start(
                t[:], wkv_d[jt * 512:(jt + 1) * 512, :].rearrange(
                    "(i p) k -> p i k", p=128))
            wkv_tiles.append(t)
        wos = []
        for cblk in range(HP):
            wot = wo_pool.tile([128, D], F16, name=f"wo{cblk}", tag=f"wo{cblk}")
            nc.sync.dma_start(wot[:], wo_d[cblk * 128:(cblk + 1) * 128, :])
            wos.append(wot)

        # ---------- S loops ----------
        kvf = small.tile([DK, 2 * HP], F16, name="kvf", tag="kvf")
        gp = pos % 1024

        def new_entry_writes(h, kt8, v8):
            nc.sync.dma_start(kt8[:, gp:gp + 1], kvf[:, h:h + 1])
            nc.sync.dma_start(
                v8[gp % 128:gp % 128 + 1,
                   (gp // 128) * 128:(gp // 128 + 1) * 128],
                kvf[:, HP + h:HP + h + 1])

        kvacc = {}
        q3state = {}
        o_staged = {}        # (s_t, mg) -> (sg_tile, chunks_staged)
        o2state = {}

        def mk_o(loop_h, t, ck, nck):
            # staged-O ride: accumulate the first `nck` Wo chunks for tile t
            # (s_t, mg) into a spare PSUM bank, stage to SBUF; the O phase
            # finishes the remaining chunks and adds. h2 stages 2 chunks
            # (ctx0/1 known), h3 stages 3.
            base = 0 if loop_h == 2 else 16
            s_t, mg = divmod(base + t, 8)

            def emit():
                if ck == 0:
                    pool, tag = ((psq, "psq") if t % 2 == 0 else
                                 (kv_pool, "kv"))
                    o2state[(loop_h, t)] = pool.tile(
                        [128, 512], F32, name=f"o{loop_h}_{t}", tag=tag)
                acc = o2state[(loop_h, t)]
                nc.tensor.matmul(acc[:],
                                 ctxTs[ck][:, s_t * 128:(s_t + 1) * 128],
                                 wos[ck][:, mg * 512:(mg + 1) * 512],
                                 start=(ck == 0), stop=(ck == nck - 1))
                if ck == nck - 1:
                    sg = sg_pool.tile([128, 512], F16, name=f"sg{s_t}_{mg}",
                                      tag=f"sg{s_t}_{mg}")
                    nc.vector.tensor_copy(sg[:], acc[:])
                    o_staged[(s_t, mg)] = (sg, nck)
            return emit

        def mk_q3(half, c):
            # one chunk-matmul of head 3's Q per ride slot: half 0 rides in
            # h0's loop, half 1 in h1's — both loops stay just above ACT pace
            def emit():
                if c == 0:
                    q3state["psq"] = psq.tile([128, 512], F32,
                                              name=f"psq3_{half}", tag="psq")
                lhs = wq_tiles[(3, c // 4)][:, (c % 4) * DK:(c % 4 + 1) * DK]
                nc.tensor.matmul(q3state["psq"][:], lhs, xsl(c, half * 512, 512),
                                 start=(c == 0), stop=(c == DC - 1))
                if c == DC - 1:
                    nc.vector.tensor_scalar_add(
                        qTs[3][:, half * 512:(half + 1) * 512],
                        q3state["psq"][:], bq_t[:, 3:4])
            return emit

        for h in range(HP):
            rides = [[] for _ in range(LC)]
            if h in (0, 1):
                for lt in range(DC):
                    rides[lt].append(mk_q3(h, lt))
            if h == 2:
                # wkv is dead after h0's kv rides
                wkv_pool.release()
                # light staged-O riding: ~100ns/lt of DVE headroom under the
                # ACT pace allows one sg copy every ~6 lt
                for t in range(5):
                    for ck in range(2):
                        rides[6 * t + 2 * ck].append(mk_o(2, t, ck, 2))
            if h == 3:
                for t in range(5):
                    for ck in range(3):
                        rides[6 * t + 2 * ck].append(mk_o(3, t, ck, 3))
            if h == 0:

                def mk_kv(ci):
                    def emit():
                        if ci == 0:
                            kvacc["t"] = kv_pool.tile(
                                [128, 2 * HP], F32, name="kvt", tag="kv")
                        kt = kvacc["t"]
                        wt_ = wkv_tiles[ci]
                        for cc in range(4):
                            c = 4 * ci + cc
                            for jj in range(2 * HP):
                                nc.tensor.matmul(
                                    kt[:, jj:jj + 1],
                                    wt_[:, cc * 1024 + jj * DK:
                                        cc * 1024 + (jj + 1) * DK],
                                    xl_t[:, c:c + 1],
                                    start=(c == 0), stop=(c == DC - 1))
                    return emit

                for ci in range(8):
                    rides[2 + ci].append(mk_kv(ci))

                def kv_finish():
                    nc.vector.tensor_add(kvf[:], kvacc["t"][:], bkv_t[:])
                rides[10].append(kv_finish)

            psc_t = psc.tile([128, S], F32, name=f"psc{h}", tag="psc")
            cur = groups0[0] if h == 0 else load_group(h, 0)
            qT_t = qTs[h]
            nxt = None
            pends = []

            def ctx_mm(item, stop):
                plt, pwt, pvst = item
                nc.tensor.matmul(psc_t[:, 0:512], pvst[:], pwt[:, 0:512],
                                 start=(plt == 0), stop=stop)
                nc.tensor.matmul(psc_t[:, 512:1024], pvst[:], pwt[:, 512:1024],
                                 start=(plt == 0), stop=stop)

            for lt in range(LC):
                g, j = lt // 8, lt % 8
                if j == 0 and g > 0:
                    cur = nxt
                if j == 0 and g + 1 < NG:
                    nxt = groups0[g + 1] if h == 0 else load_group(h, g + 1)
                    if npos_g == g + 1:
                        new_entry_writes(h, *nxt)
                kt8, v8 = cur

                ps = pss.tile([128, 1024], F32, name=f"ps_{h}_{lt}", tag="pss")
                ksl = kt8[:, j * 128:(j + 1) * 128]
                nc.tensor.matmul(ps[:, 0:512], ksl, qT_t[:, 0:512])
                nc.tensor.matmul(ps[:, 512:1024], ksl, qT_t[:, 512:1024])

                for emit in rides[lt]:
                    emit()

                wt = wt_pool.tile([128, 1024], F16, name=f"wt_{h}_{lt}", tag="wt")
                ssum = ss_pool.tile([128, 1], F32, name=f"ss_{h}_{lt}", tag="ssum")
                # Row-sum fully on DVE (pairwise f16 add-tree + short
                # reduce, ~0.94us/lt): keeps ACT at a jitter-free steady
                # 1.04us/lt exp cadence — an ACT accum_out would make its
                # service time exceed the loop period and cascade stalls
                # through the 2-deep score-bank rotation.
                nc.scalar.activation(wt[:], ps[:], AF.Exp, scale=INV)
                r1 = ss_pool.tile([128, 512], F16, name=f"r1_{h}_{lt}",
                                  tag="r1", bufs=2)
                nc.vector.tensor_add(r1[:], wt[:, 0:512], wt[:, 512:1024])
                r2 = ss_pool.tile([128, 256], F16, name=f"r2_{h}_{lt}",
                                  tag="r2", bufs=2)
                nc.vector.tensor_add(r2[:], r1[:, 0:256], r1[:, 256:512])
                r3 = ss_pool.tile([128, 128], F16, name=f"r3_{h}_{lt}",
                                  tag="r3", bufs=2)
                nc.vector.tensor_add(r3[:], r2[:, 0:128], r2[:, 128:256])
                nc.vector.tensor_reduce(ssum[:], r3[:], mybir.AxisListType.X,
                                        mybir.AluOpType.add)
                rec = ss_pool.tile([128, 1], F32, name=f"rc_{h}_{lt}", tag="rec")
                nc.vector.reciprocal(rec[:], ssum[:])
                vst = vs_pool.tile([128, DK], F16, name=f"vs{h}_{lt}", tag="vs")
                nc.vector.tensor_scalar_mul(vst[:], v8[:, j * 128:(j + 1) * 128],
                                            rec[:])

                # lag-3: the exp -> add-tree -> reduce -> recip -> scale
                # chain is ~2.1us, over two loop periods; a shorter lag
                # would stall PE on vst
                pends.append((lt, wt, vst))
                if len(pends) > 4:
                    ctx_mm(pends.pop(0), stop=False)
            while len(pends) > 1:
                ctx_mm(pends.pop(0), stop=False)
            plt, pwt, pvst = pends.pop(0)
            nc.tensor.matmul(psc_t[:, 0:512], pvst[:], pwt[:, 0:512],
                             start=(plt == 0), stop=True)
            nc.vector.tensor_copy(ctxTs[h][:, 0:512], psc_t[:, 0:512])
            nc.tensor.matmul(psc_t[:, 512:1024], pvst[:], pwt[:, 512:1024],
                             start=(plt == 0), stop=True)
            nc.vector.tensor_copy(ctxTs[h][:, 512:1024], psc_t[:, 512:1024])

        # release attention-phase pools before the output projection (LIFO;
        # wkv was already released at the start of h2's loop)
        for p in (psc, pss, kv_pool, psq,
                  ss_pool, vs_pool, wt_pool, v_pool, kt_pool, wq_pool):
            p.release()

        # ---------- output projection ----------
        ob_pool = tc.alloc_tile_pool(name="obp", bufs=3)
        pso = tc.alloc_tile_pool(name="pso", bufs=6, space="PSUM")
        last_t = S // 128 - 1

        def emit_o_tile(s_t, mg, ob):
            obsl = ob[:, mg * 512:(mg + 1) * 512]
            staged = o_staged.get((s_t, mg))
            pso_t = pso.tile([128, 512], F32, name=f"po{s_t}_{mg}", tag="pso")
            c0 = staged[1] if staged else 0
            for cblk in range(c0, HP):
                nc.tensor.matmul(pso_t[:],
                                 ctxTs[cblk][:, s_t * 128:(s_t + 1) * 128],
                                 wos[cblk][:, mg * 512:(mg + 1) * 512],
                                 start=(cblk == c0), stop=(cblk == HP - 1))
            if staged:
                # DVE is idle in the O phase; ACT carries the plain copies
                nc.vector.tensor_add(obsl, staged[0][:], pso_t[:])
            elif s_t == last_t and mg >= 6:
                # last tiles: copy on DVE so the final copy runs parallel
                # to ACT's mg6 copy, shortening the end-of-kernel chain
                nc.vector.tensor_copy(obsl, pso_t[:])
            else:
                nc.scalar.activation(obsl, pso_t[:], AF.Copy)

        for s_t in range(S // 128):
            ob = ob_pool.tile([128, D], F16, name=f"ob{s_t}", tag="ob")
            # s_t 0: unstaged tiles first — their ctx0-2 chunks fill the
            # window while the h3 ctxT copy (needed by every chunk-3 matmul
            # and every staged tile) drains
            mgs = (5, 6, 7, 0, 1, 2, 3, 4) if s_t == 0 else range(D // 512)
            for mg in mgs:
                emit_o_tile(s_t, mg, ob)
                if s_t == last_t and mg >= 2:
                    # stream the final tile as it completes (HWDGE executes
                    # in order, so the big head chunk is emitted first and
                    # only the last 128KB chunk's transfer is exposed)
                    lo = 0 if mg == 2 else mg * 512
                    nc.sync.dma_start(
                        out_d[s_t * 128:(s_t + 1) * 128, lo:(mg + 1) * 512],
                        ob[:, lo:(mg + 1) * 512])
            if s_t != last_t:
                nc.sync.dma_start(out_d[s_t * 128:(s_t + 1) * 128, :], ob[:])
        for p in (pso, ob_pool, sg_pool, small, qT_pool, xT_pool,
                  wo_pool, ctxT_pool):
            p.release()

    nc.compile()
    return nc


_CACHE = {}
LAST_EXEC_NS = None


def kernel(x, k_cache, v_cache, Wq, bq, Wk, bk, Wv, bv, Wo, bo, pos):
    global LAST_EXEC_NS
    pos = int(pos)

    def f32(a):
        return np.ascontiguousarray(np.asarray(a), dtype=np.float32)

    x = f32(x)
    k_cache, v_cache = f32(k_cache), f32(v_cache)
    Wq, Wk, Wv, Wo = f32(Wq), f32(Wk), f32(Wv), f32(Wo)
    bq, bk, bv, bo = f32(bq), f32(bk), f32(bv), f32(bo)

    fast = (pos + 1 == 4096 and x.shape == (1, S, D))
    xT = np.ascontiguousarray(x[0].T.astype(np.float16))   # [D, S]
    in_maps = []
    for i in range(NCORES):
        hs = slice(i * HP, (i + 1) * HP)
        m = {
            "xT": xT,
            "wq": np.ascontiguousarray(Wq[hs].astype(np.float16)),
            "kT": np.ascontiguousarray(
                k_cache[hs, :pos, :].transpose(0, 2, 1).astype(np.float16)),
            "v": np.ascontiguousarray(v_cache[hs, :pos, :].astype(np.float16)),
            "wo": np.ascontiguousarray(
                Wo[i * HP * DK:(i + 1) * HP * DK].astype(np.float16)),
        }
        wkv_f32 = np.concatenate([
            Wk[hs].transpose(1, 0, 2).reshape(D, HP * DK),
            Wv[hs].transpose(1, 0, 2).reshape(D, HP * DK)], axis=1)
        if fast:
            m["wkv"] = np.ascontiguousarray(
                wkv_f32.astype(ml_dtypes.float8_e5m2))
            m["xl"] = np.ascontiguousarray(
                x[0, -1].reshape(DC, 128).T.astype(ml_dtypes.float8_e5m2))
            m["bq"] = np.ascontiguousarray(bq[hs].T)                 # [DK, HP]
            m["bkv"] = np.ascontiguousarray(
                np.concatenate([bk[hs].T, bv[hs].T], axis=1))        # [DK, 2HP]
        else:
            m["wkv"] = np.ascontiguousarray(wkv_f32.astype(ml_dtypes.bfloat16))
            m["xl"] = np.ascontiguousarray(
                x[0, -1].reshape(DC, 128).T.astype(ml_dtypes.bfloat16))
            m["bq"] = np.ascontiguousarray(bq[hs].reshape(HP, DK, 1))
            m["bkv"] = np.ascontiguousarray(np.concatenate(
                [bk[hs].reshape(-1), bv[hs].reshape(-1)])[None, :])
        in_maps.append(m)

    if pos not in _CACHE:
        _CACHE[pos] = build_fast(pos) if fast else build(pos)
    nc = _CACHE[pos]

    res = run_bass_kernel_spmd(nc, in_maps, core_ids=list(range(NCORES)))
    LAST_EXEC_NS = res.exec_time_ns

    acc = np.zeros((S, D), np.float64)
    for r in res.results:
        acc += r["out"]
    out = (acc + bo.astype(np.float64)).astype(np.float32)
    return out[None]



# revision 51
# speedup vs baseline: 1.0909x; 1.0044x over previous
"""Trainium2 Bass kernel for CachedMultiHeadedAttention (tensor-parallel over heads).

Sharding: 8 cores x 4 heads. Each core computes Q projection + attention for
its 4 heads, then a partial output projection against its 512 rows of Wo.
Host sums the 8 partial outputs (the "all-reduce" done at unshard time) and
adds bo.

The fast path (build_fast, pos=4095) is scheduled against the TimelineSim
cost model; per-phase engine utilization runs 86-97%:
  - Phase A streams x (split-group DMAs) while projecting Q for heads 0-2
    concurrently into 6 PSUM banks; PE is ~93% busy against the DMA stream.
  - S loops run one l-tile per ~1.04us, paced by ACT's exp
    ([128,1024] per tile). Head 3's Q projection rides 1 chunk/lt inside
    h0/h1's loops; the fp8 rank-1 kv_new projection (transposed form,
    256 x ~0.4ns matmuls) rides in h0; 7 staged output-projection tiles
    ride in each of h2/h3's loops, with their psum->SBUF stagings on DVE.
  - The softmax row-sums run on DVE as a 2-l-tile-batched pairwise f16
    add-tree + short reduce (~0.72us/lt; ACT accum_out and TensorScalarPtr
    accum_out are broken in this NEFF path / too slow on ACT). ctx matmuls
    lag 5 tiles behind their scores to cover the exp->tree->recip->scale
    chain.
  - The output projection finishes staged tiles (DVE adds) interleaved with
    full tiles (ACT copies); the final tile streams out per-mg so only the
    last 128KB chunk is exposed.
Precision: streamed operands f16; wkv/x_last fp8 e5m2 (the new cache entry
is 1/4096 rows, ~0.1% context impact); f32 PSUM accumulation; host f64
reduction across cores. Measured end-to-end relative error ~3.8e-3
(tolerance 2e-2).
"""
import math

import numpy as np
import ml_dtypes

import concourse.bass as bass
import concourse.mybir as mybir
import concourse.tile as tile
from concourse import bacc
from concourse.bass_utils import run_bass_kernel_spmd

F32 = mybir.dt.float32
F32R = mybir.dt.float32r
BF16 = mybir.dt.bfloat16
F16 = mybir.dt.float16
AF = mybir.ActivationFunctionType

H, D, DK, S = 32, 4096, 128, 1024
NCORES = 8
HP = H // NCORES          # heads per core
DC = D // 128             # contraction chunks for d_model


def build(pos: int):
    L = pos + 1
    LC = (L + 127) // 128          # number of 128-wide l tiles
    LG = (LC + 7) // 8             # l-tile groups of 8 (1024 l per group)
    INV = 1.0 / math.sqrt(DK)

    nc = bacc.Bacc("TRN2", target_bir_lowering=False, debug=False,
                   num_devices=NCORES)

    xT_d = nc.dram_tensor("xT", [D, S], F16, kind="ExternalInput").ap()
    wq_d = nc.dram_tensor("wq", [HP, D, DK], F16, kind="ExternalInput").ap()
    wkv_d = nc.dram_tensor("wkv", [D, 2 * HP * DK], BF16, kind="ExternalInput").ap()
    xl_d = nc.dram_tensor("xl", [128, DC], BF16, kind="ExternalInput").ap()
    bq_d = nc.dram_tensor("bq", [HP, DK, 1], F32, kind="ExternalInput").ap()
    bkv_d = nc.dram_tensor("bkv", [1, 2 * HP * DK], F32, kind="ExternalInput").ap()
    kT_d = nc.dram_tensor("kT", [HP, DK, pos], F16, kind="ExternalInput").ap()
    v_d = nc.dram_tensor("v", [HP, pos, DK], F16, kind="ExternalInput").ap()
    wo_d = nc.dram_tensor("wo", [HP * DK, D], F16, kind="ExternalInput").ap()
    out_d = nc.dram_tensor("out", [S, D], F16, kind="ExternalOutput").ap()

    with tile.TileContext(nc) as tc:
        # Pools are released LIFO; ctxT survives into the output projection,
        # so it sits at the bottom of the SBUF pool stack.
        ctxT_pool = tc.alloc_tile_pool(name="ctxT", bufs=1)
        wo_pool = tc.alloc_tile_pool(name="wop", bufs=1)
        stage_pool = tc.alloc_tile_pool(name="stagep", bufs=1)
        xT_pool = tc.alloc_tile_pool(name="xT", bufs=1)
        qT_pool = tc.alloc_tile_pool(name="qT", bufs=2)
        small = tc.alloc_tile_pool(name="smallp", bufs=1)
        wq_pool = tc.alloc_tile_pool(name="wqp", bufs=8)
        wkv_pool = tc.alloc_tile_pool(name="wkvp", bufs=3)
        kt_pool = tc.alloc_tile_pool(name="ktp", bufs=3)
        v_pool = tc.alloc_tile_pool(name="vp", bufs=3)
        wt_pool = tc.alloc_tile_pool(name="wtp", bufs=4)
        vs_pool = tc.alloc_tile_pool(name="vsp", bufs=4)
        ss_pool = tc.alloc_tile_pool(name="ssp", bufs=8)

        # PSUM budget (8 banks): psq 1 + kv 1 + pss 4 + psc 2.
        # Q projections and the kv_new projections run in TWO s-half /
        # k-v passes so their accumulators are single-bank.
        psq = tc.alloc_tile_pool(name="psq", bufs=1, space="PSUM")
        kv_pool = tc.alloc_tile_pool(name="kvp", bufs=1, space="PSUM")
        pss = tc.alloc_tile_pool(name="pss", bufs=2, space="PSUM")
        psc = tc.alloc_tile_pool(name="psc", bufs=1, space="PSUM")

        ctxTs = [ctxT_pool.tile([128, S], F16, name=f"cT{h}", tag=f"cT{h}")
                 for h in range(HP)]

        # small constants first (tiny DMAs, ahead of the big streams)
        kvrow = small.tile([1, 2 * HP * DK], F16, name="kvrow", tag="kvrow")
        bkv_t = small.tile([1, 2 * HP * DK], F32, name="bkvt", tag="bkvt")
        nc.sync.dma_start(bkv_t[:], bkv_d[:])
        xl_t = small.tile([128, DC], BF16, name="xlt", tag="xlt")
        nc.sync.dma_start(xl_t[:], xl_d[:])

        # resident xT tiles (8 big tiles of 4 chunks), interleaved with head
        # 0's Q weight groups so the first Q matmuls start after ~2.5MB, not
        # after the full 17MB of x.
        xbig = []
        wq0_groups = []
        for gx in range(DC // 4):
            wqt = wq_pool.tile([128, 4 * DK], F16, name=f"wq0_{gx}", tag="wq")
            nc.sync.dma_start(
                wqt[:], wq_d[0, gx * 512:(gx + 1) * 512, :].rearrange(
                    "(i p) k -> p i k", p=128))
            wq0_groups.append(wqt)
            xt = xT_pool.tile([128, 4 * S], F16, name=f"xt{gx}", tag=f"xt{gx}")
            nc.sync.dma_start(
                xt[:], xT_d[gx * 512:(gx + 1) * 512, :].rearrange(
                    "(i p) s -> p i s", p=128))
            xbig.append(xt)

        def xsl(c, lo, sz):
            return xbig[c // 4][:, (c % 4) * S + lo:(c % 4) * S + lo + sz]

        def emit_wq_dma(h, gw, tag="wq"):
            wqt = wq_pool.tile([128, 4 * DK], F16,
                               name=f"wq{h}_{gw}", tag=tag)
            nc.sync.dma_start(
                wqt[:], wq_d[h, gw * 512:(gw + 1) * 512, :].rearrange(
                    "(i p) k -> p i k", p=128))
            return wqt

        def q_half_mm(psq_t, wqt, c, half):
            lhs = wqt[:, (c % 4) * DK:(c % 4 + 1) * DK]
            nc.tensor.matmul(psq_t[:], lhs, xsl(c, half * 512, 512),
                             start=(c == 0), stop=(c == DC - 1))

        def q_half_add(h, qT_t, psq_t, half, bq_t):
            nc.vector.tensor_scalar_add(qT_t[:, half * 512:(half + 1) * 512],
                                        psq_t[:], bq_t[:])

        kv_cur = {}

        def kv_mm(kv_t, c, which):
            # which: 0 = k_new, 1 = v_new. Weight chunks are DMA'd two at a
            # time — each dma_start costs ~625ns of serialized HWDGE.
            if c % 4 == 0:
                wkvt = wkv_pool.tile([128, 4 * HP * DK], BF16,
                                     name=f"wkv{which}_{c}", tag="wkv")
                nc.sync.dma_start(
                    wkvt[:], wkv_d[c * 128:(c + 4) * 128,
                                   which * HP * DK:(which + 1) * HP * DK]
                    .rearrange("(i p) k -> p i k", p=128))
                kv_cur["t"] = wkvt
            wkvt = kv_cur["t"]
            nc.tensor.matmul(kv_t[0:1, :], xl_t[:, c:c + 1],
                             wkvt[:, (c % 4) * HP * DK:(c % 4 + 1) * HP * DK],
                             start=(c == 0), stop=(c == DC - 1))

        def kv_add(kv_t, which):
            nc.vector.tensor_add(
                kvrow[0:1, which * HP * DK:(which + 1) * HP * DK], kv_t[:],
                bkv_t[0:1, which * HP * DK:(which + 1) * HP * DK])

        def load_group(h, g):
            """Cache-only loads of l-group g (the new-entry writes are
            emitted separately, after kvrow's writes in trace order)."""
            g0 = g * 1024
            gl = min(1024, L - g0)            # valid l in group
            gc = max(0, min(1024, pos - g0))  # of which from cache
            kt8 = kt_pool.tile([128, 1024], F16, name=f"kt{h}_{g}", tag="kt")
            if gc > 0:
                nc.sync.dma_start(kt8[:, 0:gc], kT_d[h, :, g0:g0 + gc])
            if gl < 1024:
                nc.vector.memset(kt8[:, gl:1024], 0.0)
            v8 = v_pool.tile([128, 1024], F16, name=f"v{h}_{g}", tag="v")
            if gl < 1024:
                # zero whole padded chunks first (full partition range — DVE
                # requires 32-aligned partition bases); valid rows are DMA'd
                # over the zeros below.
                nc.vector.memset(v8[:, (gl // 128) * 128:1024], 0.0)
            fc = gc // 128
            if fc > 0:
                nc.sync.dma_start(
                    v8[:, 0:fc * 128],
                    v_d[h, g0:g0 + fc * 128, :].rearrange(
                        "(i p) k -> p i k", p=128))
            rem = gc - fc * 128
            if rem > 0:
                nc.sync.dma_start(v8[0:rem, fc * 128:(fc + 1) * 128],
                                  v_d[h, g0 + fc * 128:g0 + gc, :])
            return kt8, v8

        def new_entry_writes(h, kt8, v8):
            # column/row for l == pos from the biased kvrow
            gp = pos % 1024
            nc.sync.dma_start(kt8[:, gp:gp + 1],
                              kvrow[0:1, h * DK:(h + 1) * DK])
            nc.sync.dma_start(
                v8[gp % 128:gp % 128 + 1, (gp // 128) * 128:(gp // 128 + 1) * 128],
                kvrow[0:1, HP * DK + h * DK:HP * DK + (h + 1) * DK])

        npos_g = pos // 1024            # l-group holding the new entry
        npos_lt = pos // 128            # l-tile index holding the new entry
        # riding is only possible when the S loop is long enough for the
        # 2-instruction-per-lt passes to finish before the new entry is used
        ride_kv = LC >= DC and npos_lt >= 8
        ride_q = LC >= DC

        # ---------- head 0 Q projection (phase A, DMA-paced) ----------
        bq_t = ss_pool.tile([128, 1], F32, name="bq0", tag="bq", bufs=2)
        nc.sync.dma_start(bq_t[:], bq_d[0])
        qT_t = qT_pool.tile([128, S], F16, name="qT0", tag="qT")
        # both s-halves accumulate concurrently (pass B borrows the idle kv
        # bank) so the whole projection rides the x-arrival gaps instead of
        # serializing 6.8us of pass-B matmuls after the stream ends
        psq_a = psq.tile([128, 512], F32, name="psq0_0", tag="psq")
        psq_b = kv_pool.tile([128, 512], F32, name="psq0_1", tag="kv")
        for c in range(DC):
            q_half_mm(psq_a, wq0_groups[c // 4], c, 0)
            q_half_mm(psq_b, wq0_groups[c // 4], c, 1)
        q_half_add(0, qT_t, psq_a, 0, bq_t)
        q_half_add(0, qT_t, psq_b, 1, bq_t)

        if not ride_kv:
            # fallback: dense kv_new before the S loops
            for which in range(2):
                kv_t = kv_pool.tile([1, HP * DK], F32, name=f"kv{which}", tag="kv")
                for c in range(DC):
                    kv_mm(kv_t, c, which)
                kv_add(kv_t, which)

        for h in range(HP):
            # per-lt ride items emitted right after the scores matmuls
            rides = [[] for _ in range(LC)]
            if h + 1 < HP and ride_q:
                bq1 = ss_pool.tile([128, 1], F32, name=f"bq{h+1}", tag="bq",
                                   bufs=2)
                nc.sync.dma_start(bq1[:], bq_d[h + 1])
                qT_next = qT_pool.tile([128, S], F16, name=f"qT{h+1}", tag="qT")
                state = {}

                def mk_q(lt, h1=h + 1, qn=qT_next, bqt=bq1, st=state):
                    def emit():
                        half, c0 = divmod(2 * lt, DC)
                        if c0 == 0 and half == 0:
                            st["wqts"] = {}
                        if c0 == 0:
                            st["psq"] = psq.tile([128, 512], F32,
                                                 name=f"psq{h1}_{half}", tag="psq")
                        for c in (c0, c0 + 1):
                            gw = c // 4
                            if half == 0 and c % 4 == 0:
                                # pass B reuses these resident tiles (8 slots)
                                st["wqts"][gw] = emit_wq_dma(h1, gw)
                            q_half_mm(st["psq"], st["wqts"][gw], c, half)
                        if c0 + 1 == DC - 1:
                            q_half_add(h1, qn, st["psq"], half, bqt)
                    return emit

                for lt in range(DC):
                    rides[lt].append(mk_q(lt))
            if h == 0 and ride_kv:
                # kv_new work items, paced so both passes (and their kvrow
                # writes) are emitted strictly before lt == npos_lt
                kv_work = ([("mm", 0, c) for c in range(DC)] + [("add", 0, 0)]
                           + [("mm", 1, c) for c in range(DC)] + [("add", 1, 0)])
                kvstate = {}

                def kv_emit_one(item, st=kvstate):
                    kind, which, c = item
                    if kind == "add":
                        kv_add(st["kv"], which)
                        return
                    if c == 0:
                        st["kv"] = kv_pool.tile([1, HP * DK], F32,
                                                name=f"kv{which}", tag="kv")
                    kv_mm(st["kv"], c, which)

                n_slots = npos_lt - 1          # ride slots: lt 0..npos_lt-2
                n_pre = max(0, len(kv_work) - 2 * n_slots)
                for item in kv_work[:n_pre]:
                    kv_emit_one(item)
                rest = kv_work[n_pre:]
                for k, item in enumerate(rest):
                    rides[k // 2].append(
                        (lambda it=item: kv_emit_one(it)))

            o_staged = {}
            o_post = []
            if h == HP - 1 and LC >= DC:
                # S_3 has no Q to ride; its psq/kv PSUM banks are dead. Ride
                # the first-3-chunk partials of 16 output tiles there, staged
                # to SBUF; the O phase finishes them with one matmul + add.
                wos = [wo_pool.tile([128, D], F16, name=f"wo{c}", tag=f"wo{c}")
                       for c in range(HP)]

                def mk_wo(c):
                    return lambda: nc.sync.dma_start(
                        wos[c][:], wo_d[c * 128:(c + 1) * 128, :])

                o_tiles = [(s_t, mg) for s_t in (6, 7) for mg in range(D // 512)]
                o_state = {}

                def mk_o(item, st=o_state):
                    t, k = item
                    s_t, mg = o_tiles[t]

                    def emit():
                        if k == 0:
                            pool = kv_pool if t % 2 == 0 else psq
                            st["ps"] = pool.tile(
                                [128, 512], F32, name=f"ops{t}",
                                tag="kv" if t % 2 == 0 else "psq")
                        if k < 3:
                            nc.tensor.matmul(
                                st["ps"][:],
                                ctxTs[k][:, s_t * 128:(s_t + 1) * 128],
                                wos[k][:, mg * 512:(mg + 1) * 512],
                                start=(k == 0), stop=(k == 2))
                        else:
                            sg = stage_pool.tile([128, 512], F16,
                                                 name=f"sg{t}", tag=f"sg{t}")
                            nc.vector.tensor_copy(sg[:], st["ps"][:])
                            o_staged[(s_t, mg)] = sg
                    return emit

                # wo0/wo1 load right after S_3's first K/V group; wo2/wo3
                # trail via the ride slots they're needed in
                o_post.extend([mk_wo(0), mk_wo(1)])
                rides[2].append(mk_wo(2))
                rides[10].append(mk_wo(3))
                o_work = [(t, k) for t in range(len(o_tiles)) for k in range(4)]
                for idx, item in enumerate(o_work):
                    rides[6 + idx // 3].append(mk_o(item))

            psc_t = psc.tile([128, S], F32, name=f"psc{h}", tag="psc")
            cur = load_group(h, 0)
            for fn_ in o_post:
                fn_()
            if not (h == 0 and ride_kv) and npos_g == 0 and npos_lt < LC:
                new_entry_writes(h, *cur)
            nxt = None
            pend = None              # lag-1 ctx: (lt, wt, vst)
            for lt in range(LC):
                g, j = lt // 8, lt % 8
                if j == 0 and g > 0:
                    cur = nxt
                if j == 0 and g + 1 < (LC + 7) // 8:
                    nxt = load_group(h, g + 1)
                    if not (h == 0 and ride_kv) and npos_g == g + 1:
                        new_entry_writes(h, *nxt)
                kt8, v8 = cur
                if h == 0 and ride_kv and lt == npos_lt:
                    # kvrow writes were emitted at lt <= npos_lt - 1
                    new_entry_writes(h, kt8, v8) if npos_g == g else None
                    if npos_g == g + 1 and nxt is not None:
                        new_entry_writes(h, *nxt)

                ps = pss.tile([128, 1024], F32, name=f"ps_{h}_{lt}", tag="pss")
                ksl = kt8[:, j * 128:(j + 1) * 128]
                nc.tensor.matmul(ps[:, 0:512], ksl, qT_t[:, 0:512])
                nc.tensor.matmul(ps[:, 512:1024], ksl, qT_t[:, 512:1024])

                for emit in rides[lt]:
                    emit()

                wt = wt_pool.tile([128, 1024], F32R, name=f"wt_{h}_{lt}", tag="wt")
                ssum = ss_pool.tile([128, 1], F32, name=f"ss_{h}_{lt}", tag="ssum")
                nc.scalar.activation(wt[:], ps[:], AF.Exp, scale=INV, accum_out=ssum[:])
                rec = ss_pool.tile([128, 1], F32, name=f"rc_{h}_{lt}", tag="rec")
                nc.vector.reciprocal(rec[:], ssum[:])
                vst = vs_pool.tile([128, DK], F32R, name=f"vs{h}_{lt}", tag="vs")
                nc.vector.tensor_scalar_mul(vst[:], v8[:, j * 128:(j + 1) * 128], rec[:])

                if pend is not None:
                    plt, pwt, pvst = pend
                    nc.tensor.matmul(psc_t[:, 0:512], pvst[:], pwt[:, 0:512],
                                     start=(plt == 0), stop=False)
                    nc.tensor.matmul(psc_t[:, 512:1024], pvst[:], pwt[:, 512:1024],
                                     start=(plt == 0), stop=False)
                pend = (lt, wt, vst)
            plt, pwt, pvst = pend
            nc.tensor.matmul(psc_t[:, 0:512], pvst[:], pwt[:, 0:512],
                             start=(plt == 0), stop=True)
            nc.tensor.matmul(psc_t[:, 512:1024], pvst[:], pwt[:, 512:1024],
                             start=(plt == 0), stop=True)
            nc.vector.tensor_copy(ctxTs[h][:], psc_t[:])
            if h + 1 < HP and not ride_q:
                # dense fallback Q projection for the next head
                bq1 = ss_pool.tile([128, 1], F32, name=f"bq{h+1}", tag="bq",
                                   bufs=2)
                nc.sync.dma_start(bq1[:], bq_d[h + 1])
                qT_next = qT_pool.tile([128, S], F16, name=f"qT{h+1}", tag="qT")
                wqts_fb = {}
                for half in range(2):
                    psq_t = psq.tile([128, 512], F32,
                                     name=f"psq{h+1}_{half}", tag="psq")
                    for c in range(DC):
                        if half == 0 and c % 4 == 0:
                            wqts_fb[c // 4] = emit_wq_dma(h + 1, c // 4)
                        q_half_mm(psq_t, wqts_fb[c // 4], c, half)
                    q_half_add(h + 1, qT_next, psq_t, half, bq1)
            if h + 1 < HP:
                qT_t = qT_next

        # release attention-phase pools before the output projection (LIFO)
        for p in (psc, pss, kv_pool, psq,
                  ss_pool, vs_pool, wt_pool, v_pool, kt_pool,
                  wkv_pool, wq_pool, small, qT_pool, xT_pool):
            p.release()

        # ---------- output projection: out[s, m] partial ----------
        # Wo fully resident in the space freed by xT; one 16KB-burst output
        # DMA per s-tile.
        ob_pool = tc.alloc_tile_pool(name="obp", bufs=3)
        pso = tc.alloc_tile_pool(name="pso", bufs=4, space="PSUM")
        if not o_staged:
            # fallback path (short sequences): load Wo here
            wos = []
            for c in range(HP):
                wot = wo_pool.tile([128, D], F16, name=f"wo{c}", tag=f"wo{c}")
                nc.sync.dma_start(wot[:], wo_d[c * 128:(c + 1) * 128, :])
                wos.append(wot)
        for s_t in range(S // 128):
            ob = ob_pool.tile([128, D], F16, name=f"ob{s_t}", tag="ob")
            for mg in range(D // 512):
                sg = o_staged.get((s_t, mg))
                pso_t = pso.tile([128, 512], F32, name=f"po{s_t}_{mg}", tag="pso")
                if sg is not None:
                    nc.tensor.matmul(pso_t[:],
                                     ctxTs[HP - 1][:, s_t * 128:(s_t + 1) * 128],
                                     wos[HP - 1][:, mg * 512:(mg + 1) * 512])
                    nc.vector.tensor_add(ob[:, mg * 512:(mg + 1) * 512],
                                         sg[:], pso_t[:])
                else:
                    for c in range(HP):
                        nc.tensor.matmul(pso_t[:],
                                         ctxTs[c][:, s_t * 128:(s_t + 1) * 128],
                                         wos[c][:, mg * 512:(mg + 1) * 512],
                                         start=(c == 0), stop=(c == HP - 1))
                    nc.vector.tensor_copy(ob[:, mg * 512:(mg + 1) * 512], pso_t[:])
            if s_t == S // 128 - 1:
                # stream the final tile's output per mg-pair: the exposed
                # post-compute transfer shrinks to a quarter row-band
                for q in range(8):
                    nc.sync.dma_start(
                        out_d[s_t * 128:(s_t + 1) * 128,
                              q * (D // 8):(q + 1) * (D // 8)],
                        ob[:, q * (D // 8):(q + 1) * (D // 8)])
            else:
                nc.sync.dma_start(out_d[s_t * 128:(s_t + 1) * 128, :], ob[:])
        for p in (pso, ob_pool, stage_pool, wo_pool, ctxT_pool):
            p.release()

    nc.compile()
    return nc


# e5m2: Wk/Wv entries (sigma ~1/64) stay normal (min normal 2^-14), so no
# pre-scaling is needed and the bias-add stays a plain tensor_add. The new
# cache entry is 1 of 4096 rows, so its ~7% quantization error contributes
# ~0.1% to the context.
FP8 = mybir.dt.float8e5


def build_fast(pos: int):
    """Specialized build for the harness regime (pos=4095, L=4096=DC*128).

    Structural changes vs ``build``:
      - Phase A projects heads 0..2 concurrently (6 PSUM banks), paced by the
        interleaved wq/x DMA stream, with x group 0 split into 4 chunk DMAs so
        the first matmul starts ~2us in. Head 3's Q projection rides inside
        head 0's S loop as before.
      - kv_new runs TRANSPOSED: stationary = fp8 wkv [128d x 128col] chunks,
        moving = fp8 x_last chunk [128,1], out = one PSUM column per
        (head, k/v). 256 rank-1 matmuls cost ~0.4ns each in PE time vs the
        13.6us the 64 row-form N=512 matmuls cost.
      - No staged-O riding: the output projection runs as a single PE-bound
        phase at the end with the psum->SBUF copies on the ACT engine (idle
        there), so PE never waits on DVE.
      - Last output tile DMA'd in 4 chunks so only ~1 chunk is exposed.
    """
    L = pos + 1
    LC = L // 128
    NG = LC // 8
    npos_g = pos // 1024
    INV = 1.0 / math.sqrt(DK)
    assert LC == DC and S == 1024 and npos_g == NG - 1

    nc = bacc.Bacc("TRN2", target_bir_lowering=False, debug=False,
                   num_devices=NCORES)

    xT_d = nc.dram_tensor("xT", [D, S], F16, kind="ExternalInput").ap()
    wq_d = nc.dram_tensor("wq", [HP, D, DK], F16, kind="ExternalInput").ap()
    wkv_d = nc.dram_tensor("wkv", [D, 2 * HP * DK], FP8, kind="ExternalInput").ap()
    xl_d = nc.dram_tensor("xl", [128, DC], FP8, kind="ExternalInput").ap()
    bq_d = nc.dram_tensor("bq", [DK, HP], F32, kind="ExternalInput").ap()
    bkv_d = nc.dram_tensor("bkv", [DK, 2 * HP], F32, kind="ExternalInput").ap()
    kT_d = nc.dram_tensor("kT", [HP, DK, pos], F16, kind="ExternalInput").ap()
    v_d = nc.dram_tensor("v", [HP, pos, DK], F16, kind="ExternalInput").ap()
    wo_d = nc.dram_tensor("wo", [HP * DK, D], F16, kind="ExternalInput").ap()
    out_d = nc.dram_tensor("out", [S, D], F16, kind="ExternalOutput").ap()

    with tile.TileContext(nc) as tc:
        ctxT_pool = tc.alloc_tile_pool(name="ctxT", bufs=1)
        wo_pool = tc.alloc_tile_pool(name="wop", bufs=1)
        xT_pool = tc.alloc_tile_pool(name="xT", bufs=1)
        qT_pool = tc.alloc_tile_pool(name="qT", bufs=4)
        small = tc.alloc_tile_pool(name="smallp", bufs=1)
        sg_pool = tc.alloc_tile_pool(name="sgp", bufs=1)
        wq_pool = tc.alloc_tile_pool(name="wqp", bufs=12)
        kt_pool = tc.alloc_tile_pool(name="ktp", bufs=4)
        v_pool = tc.alloc_tile_pool(name="vp", bufs=4)
        wt_pool = tc.alloc_tile_pool(name="wtp", bufs=4)
        vs_pool = tc.alloc_tile_pool(name="vsp", bufs=8)
        ss_pool = tc.alloc_tile_pool(name="ssp", bufs=8)

        # PSUM (8 banks): psq 1 + kv 1 + pss 2x[128,1024] (4) + psc 2.
        # Phase A borrows pss slot0/slot1 + psc for the three Q accumulators.
        psq = tc.alloc_tile_pool(name="psq", bufs=1, space="PSUM")
        kv_pool = tc.alloc_tile_pool(name="kvp", bufs=1, space="PSUM")
        pss = tc.alloc_tile_pool(name="pss", bufs=2, space="PSUM")
        psc = tc.alloc_tile_pool(name="psc", bufs=1, space="PSUM")
        # top of stack: released after the kv rides (start of h2's S loop)
        # to make room for the staged-O sg tiles
        wkv_pool = tc.alloc_tile_pool(name="wkvp", bufs=5)

        ctxTs = [ctxT_pool.tile([128, S], F16, name=f"cT{h}", tag=f"cT{h}")
                 for h in range(HP)]

        # ---------- phase A: DMA emissions ----------
        wq_tiles = {}

        def emit_wq(h, g):
            t = wq_pool.tile([128, 4 * DK], F16, name=f"wq{h}_{g}", tag="wq")
            nc.sync.dma_start(
                t[:], wq_d[h, g * 512:(g + 1) * 512, :].rearrange(
                    "(i p) k -> p i k", p=128))
            wq_tiles[(h, g)] = t

        xbig = [xT_pool.tile([128, 4 * S], F16, name=f"xt{g}", tag=f"xt{g}")
                for g in range(DC // 4)]
        emit_wq(0, 0)
        nc.sync.dma_start(xbig[0][:, 0:512], xT_d[0:128, 0:512])
        nc.sync.dma_start(xbig[0][:, 512:S], xT_d[0:128, 512:S])
        for i in range(1, 4):  # group 0 split per chunk for a fast start
            nc.sync.dma_start(xbig[0][:, i * S:(i + 1) * S],
                              xT_d[i * 128:(i + 1) * 128, :])
        emit_wq(1, 0)
        emit_wq(2, 0)
        for h in range(3):
            emit_wq(h, 1)
        bq_t = small.tile([DK, HP], F32, name="bqall", tag="bqall")
        bkv_t = small.tile([DK, 2 * HP], F32, name="bkvt", tag="bkvt")
        xl_t = small.tile([128, DC], FP8, name="xlt", tag="xlt")
        for g in range(1, DC // 4):
            # x leads its segment (PE unblocks on it); the wq tiles for
            # group g+1 ride behind, arriving a full segment early
            nc.sync.dma_start(
                xbig[g][:, 0:2 * S],
                xT_d[g * 512:g * 512 + 256, :].rearrange(
                    "(i p) s -> p i s", p=128))
            nc.sync.dma_start(
                xbig[g][:, 2 * S:4 * S],
                xT_d[g * 512 + 256:(g + 1) * 512, :].rearrange(
                    "(i p) s -> p i s", p=128))
            if g + 1 < DC // 4:
                for h in range(3):
                    emit_wq(h, g + 1)
            if g == 3:
                # small constants ride behind the early x groups; needed
                # first at the phase-A bias adds / S-h0 kv rides
                nc.sync.dma_start(bq_t[:], bq_d[:])
                nc.sync.dma_start(bkv_t[:], bkv_d[:])
                nc.sync.dma_start(xl_t[:], xl_d[:])

        def xsl(c, lo, sz):
            return xbig[c // 4][:, (c % 4) * S + lo:(c % 4) * S + lo + sz]

        # ---------- phase A: compute emissions ----------
        qAcc = [pss.tile([128, S], F32, name="qacc0", tag="pss"),
                pss.tile([128, S], F32, name="qacc1", tag="pss"),
                psc.tile([128, S], F32, name="qacc2", tag="psc")]
        qTs = [qT_pool.tile([128, S], F16, name=f"qT{h}", tag="qT")
               for h in range(HP)]
        for g in range(DC // 4):
            for h in range(3):
                for c in range(4 * g, 4 * g + 4):
                    lhs = wq_tiles[(h, g)][:, (c % 4) * DK:(c % 4 + 1) * DK]
                    nc.tensor.matmul(qAcc[h][:, 0:512], lhs, xsl(c, 0, 512),
                                     start=(c == 0), stop=(c == DC - 1))
                    nc.tensor.matmul(qAcc[h][:, 512:1024], lhs, xsl(c, 512, 512),
                                     start=(c == 0), stop=(c == DC - 1))
        for h in range(3):
            for hf in range(2):
                nc.vector.tensor_scalar_add(
                    qTs[h][:, hf * 512:(hf + 1) * 512],
                    qAcc[h][:, hf * 512:(hf + 1) * 512], bq_t[:, h:h + 1])

        # ---------- post-phase-A DMA block (queue order matters) ----------
        def load_group(h, g):
            g0 = g * 1024
            gl = min(1024, L - g0)
            gc = max(0, min(1024, pos - g0))
            kt8 = kt_pool.tile([128, 1024], F16, name=f"kt{h}_{g}", tag="kt")
            if gc > 0:
                nc.sync.dma_start(kt8[:, 0:gc], kT_d[h, :, g0:g0 + gc])
            if gl < 1024:
                nc.vector.memset(kt8[:, gl:1024], 0.0)
            v8 = v_pool.tile([128, 1024], F16, name=f"v{h}_{g}", tag="v")
            if gl < 1024:
                nc.vector.memset(v8[:, (gl // 128) * 128:1024], 0.0)
            fc = gc // 128
            if fc > 0:
                nc.sync.dma_start(
                    v8[:, 0:fc * 128],
                    v_d[h, g0:g0 + fc * 128, :].rearrange(
                        "(i p) k -> p i k", p=128))
            rem = gc - fc * 128
            if rem > 0:
                nc.sync.dma_start(v8[0:rem, fc * 128:(fc + 1) * 128],
                                  v_d[h, g0 + fc * 128:g0 + gc, :])
            return kt8, v8

        groups0 = [load_group(0, 0)]
        for g in range(DC // 4):
            emit_wq(3, g)
        for g in range(1, NG):
            groups0.append(load_group(0, g))
        wkv_tiles = []
        for jt in range(8):
            t = wkv_pool.tile([128, 4 * 2 * HP * DK], FP8,
                              name=f"wkv{jt}", tag="wkv")
            nc.sync.dma_start(
                t[:], wkv_d[jt * 512:(jt + 1) * 512, :].rearrange(
                    "(i p) k -> p i k", p=128))
            wkv_tiles.append(t)
        wos = []
        for cblk in range(HP):
            wot = wo_pool.tile([128, D], F16, name=f"wo{cblk}", tag=f"wo{cblk}")
            nc.sync.dma_start(wot[:], wo_d[cblk * 128:(cblk + 1) * 128, :])
            wos.append(wot)

        # ---------- S loops ----------
        kvf = small.tile([DK, 2 * HP], F16, name="kvf", tag="kvf")
        gp = pos % 1024

        def new_entry_writes(h, kt8, v8):
            nc.sync.dma_start(kt8[:, gp:gp + 1], kvf[:, h:h + 1])
            nc.sync.dma_start(
                v8[gp % 128:gp % 128 + 1,
                   (gp // 128) * 128:(gp // 128 + 1) * 128],
                kvf[:, HP + h:HP + h + 1])

        kvacc = {}
        q3state = {}
        o_staged = {}        # (s_t, mg) -> (sg_tile, chunks_staged)
        o2state = {}

        def mk_o(loop_h, t, ck, nck):
            # staged-O ride: accumulate the first `nck` Wo chunks for tile t
            # (s_t, mg) into a spare PSUM bank, stage to SBUF; the O phase
            # finishes the remaining chunks and adds. h2 stages 2 chunks
            # (ctx0/1 known), h3 stages 3.
            base = 0 if loop_h == 2 else 16
            s_t, mg = divmod(base + t, 8)

            def emit():
                if ck == 0:
                    pool, tag = ((psq, "psq") if t % 2 == 0 else
                                 (kv_pool, "kv"))
                    o2state[(loop_h, t)] = pool.tile(
                        [128, 512], F32, name=f"o{loop_h}_{t}", tag=tag)
                acc = o2state[(loop_h, t)]
                nc.tensor.matmul(acc[:],
                                 ctxTs[ck][:, s_t * 128:(s_t + 1) * 128],
                                 wos[ck][:, mg * 512:(mg + 1) * 512],
                                 start=(ck == 0), stop=(ck == nck - 1))
                if ck == nck - 1:
                    sg = sg_pool.tile([128, 512], F16, name=f"sg{s_t}_{mg}",
                                      tag=f"sg{s_t}_{mg}")
                    nc.vector.tensor_copy(sg[:], acc[:])
                    o_staged[(s_t, mg)] = (sg, nck)
            return emit

        def mk_q3(half, c):
            # one chunk-matmul of head 3's Q per ride slot: half 0 rides in
            # h0's loop, half 1 in h1's — both loops stay just above ACT pace
            def emit():
                if c == 0:
                    q3state["psq"] = psq.tile([128, 512], F32,
                                              name=f"psq3_{half}", tag="psq")
                lhs = wq_tiles[(3, c // 4)][:, (c % 4) * DK:(c % 4 + 1) * DK]
                nc.tensor.matmul(q3state["psq"][:], lhs, xsl(c, half * 512, 512),
                                 start=(c == 0), stop=(c == DC - 1))
                if c == DC - 1:
                    nc.vector.tensor_scalar_add(
                        qTs[3][:, half * 512:(half + 1) * 512],
                        q3state["psq"][:], bq_t[:, 3:4])
            return emit

        for h in range(HP):
            rides = [[] for _ in range(LC)]
            if h in (0, 1):
                for lt in range(DC):
                    rides[lt].append(mk_q3(h, lt))
            if h == 2:
                # wkv is dead after h0's kv rides
                wkv_pool.release()
                # light staged-O riding: ~100ns/lt of DVE headroom under the
                # ACT pace allows one sg copy every ~6 lt
                for t in range(7):
                    for ck in range(2):
                        rides[4 * t + 2 * ck].append(mk_o(2, t, ck, 2))
            if h == 3:
                for t in range(7):
                    for ck in range(3):
                        rides[4 * t + ck].append(mk_o(3, t, ck, 3))
            if h == 0:

                def mk_kv(ci):
                    def emit():
                        if ci == 0:
                            kvacc["t"] = kv_pool.tile(
                                [128, 2 * HP], F32, name="kvt", tag="kv")
                        kt = kvacc["t"]
                        wt_ = wkv_tiles[ci]
                        for cc in range(4):
                            c = 4 * ci + cc
                            for jj in range(2 * HP):
                                nc.tensor.matmul(
                                    kt[:, jj:jj + 1],
                                    wt_[:, cc * 1024 + jj * DK:
                                        cc * 1024 + (jj + 1) * DK],
                                    xl_t[:, c:c + 1],
                                    start=(c == 0), stop=(c == DC - 1))
                    return emit

                for ci in range(8):
                    rides[2 + ci].append(mk_kv(ci))

                def kv_finish():
                    nc.vector.tensor_add(kvf[:], kvacc["t"][:], bkv_t[:])
                rides[10].append(kv_finish)

            pstate = {}
            psc_t = psc.tile([128, S], F32, name=f"psc{h}", tag="psc")
            cur = groups0[0] if h == 0 else load_group(h, 0)
            qT_t = qTs[h]
            nxt = None
            pends = []

            def ctx_mm(item, stop):
                plt, pwt, pvst = item
                nc.tensor.matmul(psc_t[:, 0:512], pvst[:], pwt[:, 0:512],
                                 start=(plt == 0), stop=stop)
                nc.tensor.matmul(psc_t[:, 512:1024], pvst[:], pwt[:, 512:1024],
                                 start=(plt == 0), stop=stop)

            for lt in range(LC):
                g, j = lt // 8, lt % 8
                if j == 0 and g > 0:
                    cur = nxt
                if j == 0 and g + 1 < NG:
                    nxt = groups0[g + 1] if h == 0 else load_group(h, g + 1)
                    if npos_g == g + 1:
                        new_entry_writes(h, *nxt)
                kt8, v8 = cur

                ps = pss.tile([128, 1024], F32, name=f"ps_{h}_{lt}", tag="pss")
                ksl = kt8[:, j * 128:(j + 1) * 128]
                nc.tensor.matmul(ps[:, 0:512], ksl, qT_t[:, 0:512])
                nc.tensor.matmul(ps[:, 512:1024], ksl, qT_t[:, 512:1024])

                for emit in rides[lt]:
                    emit()

                # Exp per lt into a shared 2-lt wt tile; the DVE row-sum
                # tree runs BATCHED per pair via strided 3D aps (~0.72us/lt
                # vs 0.84 unbatched), keeping ACT the steady pacer. An ACT
                # accum_out would make its service time exceed the loop
                # period and cascade stalls through the score-bank rotation.
                if lt % 2 == 0:
                    pstate["wtp"] = wt_pool.tile([128, 2048], F16,
                                                 name=f"wt_{h}_{lt}", tag="wt")
                wtp = pstate["wtp"]
                half = lt % 2
                nc.scalar.activation(wtp[:, half * 1024:(half + 1) * 1024],
                                     ps[:], AF.Exp, scale=INV)
                if half == 1:
                    wv = wtp.rearrange("p (l c) -> p l c", c=1024)
                    r1 = ss_pool.tile([128, 2, 512], F16, name=f"r1_{h}_{lt}",
                                      tag="r1", bufs=2)
                    nc.vector.tensor_add(r1[:], wv[:, :, 0:512],
                                         wv[:, :, 512:1024])
                    r2 = ss_pool.tile([128, 2, 256], F16, name=f"r2_{h}_{lt}",
                                      tag="r2", bufs=2)
                    nc.vector.tensor_add(r2[:], r1[:, :, 0:256],
                                         r1[:, :, 256:512])
                    r3 = ss_pool.tile([128, 2, 128], F16, name=f"r3_{h}_{lt}",
                                      tag="r3", bufs=2)
                    nc.vector.tensor_add(r3[:], r2[:, :, 0:128],
                                         r2[:, :, 128:256])
                    ssum = ss_pool.tile([128, 2], F32, name=f"ss_{h}_{lt}",
                                        tag="ssum")
                    nc.vector.tensor_reduce(ssum[:], r3[:],
                                            mybir.AxisListType.X,
                                            mybir.AluOpType.add)
                    rec = ss_pool.tile([128, 2], F32, name=f"rc_{h}_{lt}",
                                       tag="rec")
                    nc.vector.reciprocal(rec[:], ssum[:])
                    for q in range(2):
                        jq = j - 1 + q
                        vst = vs_pool.tile([128, DK], F16,
                                           name=f"vs{h}_{lt}_{q}", tag="vs")
                        nc.vector.tensor_scalar_mul(
                            vst[:], v8[:, jq * 128:(jq + 1) * 128],
                            rec[:, q:q + 1])
                        pends.append((lt - 1 + q, wv[:, q, :], vst))

                # lag >=4: the exp -> batched tree -> recip -> scale chain
                # spans over two loop periods
                while len(pends) > 5:
                    ctx_mm(pends.pop(0), stop=False)
            while len(pends) > 1:
                ctx_mm(pends.pop(0), stop=False)
            plt, pwt, pvst = pends.pop(0)
            nc.tensor.matmul(psc_t[:, 0:512], pvst[:], pwt[:, 0:512],
                             start=(plt == 0), stop=True)
            nc.vector.tensor_copy(ctxTs[h][:, 0:512], psc_t[:, 0:512])
            nc.tensor.matmul(psc_t[:, 512:1024], pvst[:], pwt[:, 512:1024],
                             start=(plt == 0), stop=True)
            nc.vector.tensor_copy(ctxTs[h][:, 512:1024], psc_t[:, 512:1024])

        # release attention-phase pools before the output projection (LIFO;
        # wkv was already released at the start of h2's loop)
        for p in (psc, pss, kv_pool, psq,
                  ss_pool, vs_pool, wt_pool, v_pool, kt_pool, wq_pool):
            p.release()

        # ---------- output projection ----------
        ob_pool = tc.alloc_tile_pool(name="obp", bufs=3)
        pso = tc.alloc_tile_pool(name="pso", bufs=6, space="PSUM")
        last_t = S // 128 - 1

        def emit_o_tile(s_t, mg, ob):
            obsl = ob[:, mg * 512:(mg + 1) * 512]
            staged = o_staged.get((s_t, mg))
            pso_t = pso.tile([128, 512], F32, name=f"po{s_t}_{mg}", tag="pso")
            c0 = staged[1] if staged else 0
            for cblk in range(c0, HP):
                nc.tensor.matmul(pso_t[:],
                                 ctxTs[cblk][:, s_t * 128:(s_t + 1) * 128],
                                 wos[cblk][:, mg * 512:(mg + 1) * 512],
                                 start=(cblk == c0), stop=(cblk == HP - 1))
            if staged:
                # DVE is idle in the O phase; ACT carries the plain copies
                nc.vector.tensor_add(obsl, staged[0][:], pso_t[:])
            elif s_t == last_t and mg >= 6:
                # last tiles: copy on DVE so the final copy runs parallel
                # to ACT's mg6 copy, shortening the end-of-kernel chain
                nc.vector.tensor_copy(obsl, pso_t[:])
            else:
                nc.scalar.activation(obsl, pso_t[:], AF.Copy)

        for s_t in range(S // 128):
            ob = ob_pool.tile([128, D], F16, name=f"ob{s_t}", tag="ob")
            # s_t 0: unstaged tiles first — their ctx0-2 chunks fill the
            # window while the h3 ctxT copy (needed by every chunk-3 matmul
            # and every staged tile) drains
            mgs = (5, 6, 7, 0, 1, 2, 3, 4) if s_t == 0 else range(D // 512)
            for mg in mgs:
                emit_o_tile(s_t, mg, ob)
                if s_t == last_t and mg >= 2:
                    # stream the final tile as it completes (HWDGE executes
                    # in order, so the big head chunk is emitted first and
                    # only the last 128KB chunk's transfer is exposed)
                    lo = 0 if mg == 2 else mg * 512
                    nc.sync.dma_start(
                        out_d[s_t * 128:(s_t + 1) * 128, lo:(mg + 1) * 512],
                        ob[:, lo:(mg + 1) * 512])
            if s_t != last_t:
                nc.sync.dma_start(out_d[s_t * 128:(s_t + 1) * 128, :], ob[:])
        for p in (pso, ob_pool, sg_pool, small, qT_pool, xT_pool,
                  wo_pool, ctxT_pool):
            p.release()

    nc.compile()
    return nc


_CACHE = {}
LAST_EXEC_NS = None


def kernel(x, k_cache, v_cache, Wq, bq, Wk, bk, Wv, bv, Wo, bo, pos):
    global LAST_EXEC_NS
    pos = int(pos)

    def f32(a):
        return np.ascontiguousarray(np.asarray(a), dtype=np.float32)

    x = f32(x)
    k_cache, v_cache = f32(k_cache), f32(v_cache)
    Wq, Wk, Wv, Wo = f32(Wq), f32(Wk), f32(Wv), f32(Wo)
    bq, bk, bv, bo = f32(bq), f32(bk), f32(bv), f32(bo)

    fast = (pos + 1 == 4096 and x.shape == (1, S, D))
    xT = np.ascontiguousarray(x[0].T.astype(np.float16))   # [D, S]
    in_maps = []
    for i in range(NCORES):
        hs = slice(i * HP, (i + 1) * HP)
        m = {
            "xT": xT,
            "wq": np.ascontiguousarray(Wq[hs].astype(np.float16)),
            "kT": np.ascontiguousarray(
                k_cache[hs, :pos, :].transpose(0, 2, 1).astype(np.float16)),
            "v": np.ascontiguousarray(v_cache[hs, :pos, :].astype(np.float16)),
            "wo": np.ascontiguousarray(
                Wo[i * HP * DK:(i + 1) * HP * DK].astype(np.float16)),
        }
        wkv_f32 = np.concatenate([
            Wk[hs].transpose(1, 0, 2).reshape(D, HP * DK),
            Wv[hs].transpose(1, 0, 2).reshape(D, HP * DK)], axis=1)
        if fast:
            m["wkv"] = np.ascontiguousarray(
                wkv_f32.astype(ml_dtypes.float8_e5m2))
            m["xl"] = np.ascontiguousarray(
                x[0, -1].reshape(DC, 128).T.astype(ml_dtypes.float8_e5m2))
            m["bq"] = np.ascontiguousarray(bq[hs].T)                 # [DK, HP]
            m["bkv"] = np.ascontiguousarray(
                np.concatenate([bk[hs].T, bv[hs].T], axis=1))        # [DK, 2HP]
        else:
            m["wkv"] = np.ascontiguousarray(wkv_f32.astype(ml_dtypes.bfloat16))
            m["xl"] = np.ascontiguousarray(
                x[0, -1].reshape(DC, 128).T.astype(ml_dtypes.bfloat16))
            m["bq"] = np.ascontiguousarray(bq[hs].reshape(HP, DK, 1))
            m["bkv"] = np.ascontiguousarray(np.concatenate(
                [bk[hs].reshape(-1), bv[hs].reshape(-1)])[None, :])
        in_maps.append(m)

    if pos not in _CACHE:
        _CACHE[pos] = build_fast(pos) if fast else build(pos)
    nc = _CACHE[pos]

    res = run_bass_kernel_spmd(nc, in_maps, core_ids=list(range(NCORES)))
    LAST_EXEC_NS = res.exec_time_ns

    acc = np.zeros((S, D), np.float64)
    for r in res.results:
        acc += r["out"]
    out = (acc + bo.astype(np.float64)).astype(np.float32)
    return out[None]



# revision 57
# speedup vs baseline: 1.1080x; 1.0157x over previous
"""Trainium2 Bass kernel for CachedMultiHeadedAttention (tensor-parallel over heads).

Sharding: 8 cores x 4 heads. Each core computes Q projection + attention for
its 4 heads, then a partial output projection against its 512 rows of Wo.
Host sums the 8 partial outputs (the "all-reduce" done at unshard time) and
adds bo.

The fast path (build_fast, pos=4095) is scheduled against the TimelineSim
cost model; per-phase engine utilization runs 86-97%:
  - Phase A streams x (split-group DMAs) while projecting Q for heads 0-2
    concurrently into 6 PSUM banks; PE is ~93% busy against the DMA stream.
  - S loops run one l-tile per ~1.04us, paced by ACT's exp
    ([128,1024] per tile). Head 3's Q projection rides 1 chunk/lt inside
    h0/h1's loops; the fp8 rank-1 kv_new projection (transposed form,
    256 x ~0.4ns matmuls) rides in h0; 7 staged output-projection tiles
    ride in each of h2/h3's loops, with their psum->SBUF stagings on DVE.
  - The softmax row-sums run on DVE as a 2-l-tile-batched pairwise f16
    add-tree + short reduce (~0.72us/lt; ACT accum_out and TensorScalarPtr
    accum_out are broken in this NEFF path / too slow on ACT). ctx matmuls
    lag 8 tiles behind their scores so the exp->tree->recip->scale chain
    and its cross-engine semaphore hops never backpressure PE.
  - The output projection finishes staged tiles (DVE adds) interleaved with
    full tiles (ACT copies); the final tile streams out per-mg so only the
    last 128KB chunk is exposed.
Precision: streamed operands f16; wkv/x_last fp8 e5m2 (the new cache entry
is 1/4096 rows, ~0.1% context impact); f32 PSUM accumulation; host f64
reduction across cores. Measured end-to-end relative error ~3.8e-3
(tolerance 2e-2).
"""
import math

import numpy as np
import ml_dtypes

import concourse.bass as bass
import concourse.mybir as mybir
import concourse.tile as tile
from concourse import bacc
from concourse.bass_utils import run_bass_kernel_spmd

F32 = mybir.dt.float32
F32R = mybir.dt.float32r
BF16 = mybir.dt.bfloat16
F16 = mybir.dt.float16
AF = mybir.ActivationFunctionType

H, D, DK, S = 32, 4096, 128, 1024
NCORES = 8
HP = H // NCORES          # heads per core
DC = D // 128             # contraction chunks for d_model


def build(pos: int):
    L = pos + 1
    LC = (L + 127) // 128          # number of 128-wide l tiles
    LG = (LC + 7) // 8             # l-tile groups of 8 (1024 l per group)
    INV = 1.0 / math.sqrt(DK)

    nc = bacc.Bacc("TRN2", target_bir_lowering=False, debug=False,
                   num_devices=NCORES)

    xT_d = nc.dram_tensor("xT", [D, S], F16, kind="ExternalInput").ap()
    wq_d = nc.dram_tensor("wq", [HP, D, DK], F16, kind="ExternalInput").ap()
    wkv_d = nc.dram_tensor("wkv", [D, 2 * HP * DK], BF16, kind="ExternalInput").ap()
    xl_d = nc.dram_tensor("xl", [128, DC], BF16, kind="ExternalInput").ap()
    bq_d = nc.dram_tensor("bq", [HP, DK, 1], F32, kind="ExternalInput").ap()
    bkv_d = nc.dram_tensor("bkv", [1, 2 * HP * DK], F32, kind="ExternalInput").ap()
    kT_d = nc.dram_tensor("kT", [HP, DK, pos], F16, kind="ExternalInput").ap()
    v_d = nc.dram_tensor("v", [HP, pos, DK], F16, kind="ExternalInput").ap()
    wo_d = nc.dram_tensor("wo", [HP * DK, D], F16, kind="ExternalInput").ap()
    out_d = nc.dram_tensor("out", [S, D], F16, kind="ExternalOutput").ap()

    with tile.TileContext(nc) as tc:
        # Pools are released LIFO; ctxT survives into the output projection,
        # so it sits at the bottom of the SBUF pool stack.
        ctxT_pool = tc.alloc_tile_pool(name="ctxT", bufs=1)
        wo_pool = tc.alloc_tile_pool(name="wop", bufs=1)
        stage_pool = tc.alloc_tile_pool(name="stagep", bufs=1)
        xT_pool = tc.alloc_tile_pool(name="xT", bufs=1)
        qT_pool = tc.alloc_tile_pool(name="qT", bufs=2)
        small = tc.alloc_tile_pool(name="smallp", bufs=1)
        wq_pool = tc.alloc_tile_pool(name="wqp", bufs=8)
        wkv_pool = tc.alloc_tile_pool(name="wkvp", bufs=3)
        kt_pool = tc.alloc_tile_pool(name="ktp", bufs=3)
        v_pool = tc.alloc_tile_pool(name="vp", bufs=3)
        wt_pool = tc.alloc_tile_pool(name="wtp", bufs=5)
        vs_pool = tc.alloc_tile_pool(name="vsp", bufs=4)
        ss_pool = tc.alloc_tile_pool(name="ssp", bufs=8)

        # PSUM budget (8 banks): psq 1 + kv 1 + pss 4 + psc 2.
        # Q projections and the kv_new projections run in TWO s-half /
        # k-v passes so their accumulators are single-bank.
        psq = tc.alloc_tile_pool(name="psq", bufs=1, space="PSUM")
        kv_pool = tc.alloc_tile_pool(name="kvp", bufs=1, space="PSUM")
        pss = tc.alloc_tile_pool(name="pss", bufs=2, space="PSUM")
        psc = tc.alloc_tile_pool(name="psc", bufs=1, space="PSUM")

        ctxTs = [ctxT_pool.tile([128, S], F16, name=f"cT{h}", tag=f"cT{h}")
                 for h in range(HP)]

        # small constants first (tiny DMAs, ahead of the big streams)
        kvrow = small.tile([1, 2 * HP * DK], F16, name="kvrow", tag="kvrow")
        bkv_t = small.tile([1, 2 * HP * DK], F32, name="bkvt", tag="bkvt")
        nc.sync.dma_start(bkv_t[:], bkv_d[:])
        xl_t = small.tile([128, DC], BF16, name="xlt", tag="xlt")
        nc.sync.dma_start(xl_t[:], xl_d[:])

        # resident xT tiles (8 big tiles of 4 chunks), interleaved with head
        # 0's Q weight groups so the first Q matmuls start after ~2.5MB, not
        # after the full 17MB of x.
        xbig = []
        wq0_groups = []
        for gx in range(DC // 4):
            wqt = wq_pool.tile([128, 4 * DK], F16, name=f"wq0_{gx}", tag="wq")
            nc.sync.dma_start(
                wqt[:], wq_d[0, gx * 512:(gx + 1) * 512, :].rearrange(
                    "(i p) k -> p i k", p=128))
            wq0_groups.append(wqt)
            xt = xT_pool.tile([128, 4 * S], F16, name=f"xt{gx}", tag=f"xt{gx}")
            nc.sync.dma_start(
                xt[:], xT_d[gx * 512:(gx + 1) * 512, :].rearrange(
                    "(i p) s -> p i s", p=128))
            xbig.append(xt)

        def xsl(c, lo, sz):
            return xbig[c // 4][:, (c % 4) * S + lo:(c % 4) * S + lo + sz]

        def emit_wq_dma(h, gw, tag="wq"):
            wqt = wq_pool.tile([128, 4 * DK], F16,
                               name=f"wq{h}_{gw}", tag=tag)
            nc.sync.dma_start(
                wqt[:], wq_d[h, gw * 512:(gw + 1) * 512, :].rearrange(
                    "(i p) k -> p i k", p=128))
            return wqt

        def q_half_mm(psq_t, wqt, c, half):
            lhs = wqt[:, (c % 4) * DK:(c % 4 + 1) * DK]
            nc.tensor.matmul(psq_t[:], lhs, xsl(c, half * 512, 512),
                             start=(c == 0), stop=(c == DC - 1))

        def q_half_add(h, qT_t, psq_t, half, bq_t):
            nc.vector.tensor_scalar_add(qT_t[:, half * 512:(half + 1) * 512],
                                        psq_t[:], bq_t[:])

        kv_cur = {}

        def kv_mm(kv_t, c, which):
            # which: 0 = k_new, 1 = v_new. Weight chunks are DMA'd two at a
            # time — each dma_start costs ~625ns of serialized HWDGE.
            if c % 4 == 0:
                wkvt = wkv_pool.tile([128, 4 * HP * DK], BF16,
                                     name=f"wkv{which}_{c}", tag="wkv")
                nc.sync.dma_start(
                    wkvt[:], wkv_d[c * 128:(c + 4) * 128,
                                   which * HP * DK:(which + 1) * HP * DK]
                    .rearrange("(i p) k -> p i k", p=128))
                kv_cur["t"] = wkvt
            wkvt = kv_cur["t"]
            nc.tensor.matmul(kv_t[0:1, :], xl_t[:, c:c + 1],
                             wkvt[:, (c % 4) * HP * DK:(c % 4 + 1) * HP * DK],
                             start=(c == 0), stop=(c == DC - 1))

        def kv_add(kv_t, which):
            nc.vector.tensor_add(
                kvrow[0:1, which * HP * DK:(which + 1) * HP * DK], kv_t[:],
                bkv_t[0:1, which * HP * DK:(which + 1) * HP * DK])

        def load_group(h, g):
            """Cache-only loads of l-group g (the new-entry writes are
            emitted separately, after kvrow's writes in trace order)."""
            g0 = g * 1024
            gl = min(1024, L - g0)            # valid l in group
            gc = max(0, min(1024, pos - g0))  # of which from cache
            kt8 = kt_pool.tile([128, 1024], F16, name=f"kt{h}_{g}", tag="kt")
            if gc > 0:
                nc.sync.dma_start(kt8[:, 0:gc], kT_d[h, :, g0:g0 + gc])
            if gl < 1024:
                nc.vector.memset(kt8[:, gl:1024], 0.0)
            v8 = v_pool.tile([128, 1024], F16, name=f"v{h}_{g}", tag="v")
            if gl < 1024:
                # zero whole padded chunks first (full partition range — DVE
                # requires 32-aligned partition bases); valid rows are DMA'd
                # over the zeros below.
                nc.vector.memset(v8[:, (gl // 128) * 128:1024], 0.0)
            fc = gc // 128
            if fc > 0:
                nc.sync.dma_start(
                    v8[:, 0:fc * 128],
                    v_d[h, g0:g0 + fc * 128, :].rearrange(
                        "(i p) k -> p i k", p=128))
            rem = gc - fc * 128
            if rem > 0:
                nc.sync.dma_start(v8[0:rem, fc * 128:(fc + 1) * 128],
                                  v_d[h, g0 + fc * 128:g0 + gc, :])
            return kt8, v8

        def new_entry_writes(h, kt8, v8):
            # column/row for l == pos from the biased kvrow
            gp = pos % 1024
            nc.sync.dma_start(kt8[:, gp:gp + 1],
                              kvrow[0:1, h * DK:(h + 1) * DK])
            nc.sync.dma_start(
                v8[gp % 128:gp % 128 + 1, (gp // 128) * 128:(gp // 128 + 1) * 128],
                kvrow[0:1, HP * DK + h * DK:HP * DK + (h + 1) * DK])

        npos_g = pos // 1024            # l-group holding the new entry
        npos_lt = pos // 128            # l-tile index holding the new entry
        # riding is only possible when the S loop is long enough for the
        # 2-instruction-per-lt passes to finish before the new entry is used
        ride_kv = LC >= DC and npos_lt >= 8
        ride_q = LC >= DC

        # ---------- head 0 Q projection (phase A, DMA-paced) ----------
        bq_t = ss_pool.tile([128, 1], F32, name="bq0", tag="bq", bufs=2)
        nc.sync.dma_start(bq_t[:], bq_d[0])
        qT_t = qT_pool.tile([128, S], F16, name="qT0", tag="qT")
        # both s-halves accumulate concurrently (pass B borrows the idle kv
        # bank) so the whole projection rides the x-arrival gaps instead of
        # serializing 6.8us of pass-B matmuls after the stream ends
        psq_a = psq.tile([128, 512], F32, name="psq0_0", tag="psq")
        psq_b = kv_pool.tile([128, 512], F32, name="psq0_1", tag="kv")
        for c in range(DC):
            q_half_mm(psq_a, wq0_groups[c // 4], c, 0)
            q_half_mm(psq_b, wq0_groups[c // 4], c, 1)
        q_half_add(0, qT_t, psq_a, 0, bq_t)
        q_half_add(0, qT_t, psq_b, 1, bq_t)

        if not ride_kv:
            # fallback: dense kv_new before the S loops
            for which in range(2):
                kv_t = kv_pool.tile([1, HP * DK], F32, name=f"kv{which}", tag="kv")
                for c in range(DC):
                    kv_mm(kv_t, c, which)
                kv_add(kv_t, which)

        for h in range(HP):
            # per-lt ride items emitted right after the scores matmuls
            rides = [[] for _ in range(LC)]
            if h + 1 < HP and ride_q:
                bq1 = ss_pool.tile([128, 1], F32, name=f"bq{h+1}", tag="bq",
                                   bufs=2)
                nc.sync.dma_start(bq1[:], bq_d[h + 1])
                qT_next = qT_pool.tile([128, S], F16, name=f"qT{h+1}", tag="qT")
                state = {}

                def mk_q(lt, h1=h + 1, qn=qT_next, bqt=bq1, st=state):
                    def emit():
                        half, c0 = divmod(2 * lt, DC)
                        if c0 == 0 and half == 0:
                            st["wqts"] = {}
                        if c0 == 0:
                            st["psq"] = psq.tile([128, 512], F32,
                                                 name=f"psq{h1}_{half}", tag="psq")
                        for c in (c0, c0 + 1):
                            gw = c // 4
                            if half == 0 and c % 4 == 0:
                                # pass B reuses these resident tiles (8 slots)
                                st["wqts"][gw] = emit_wq_dma(h1, gw)
                            q_half_mm(st["psq"], st["wqts"][gw], c, half)
                        if c0 + 1 == DC - 1:
                            q_half_add(h1, qn, st["psq"], half, bqt)
                    return emit

                for lt in range(DC):
                    rides[lt].append(mk_q(lt))
            if h == 0 and ride_kv:
                # kv_new work items, paced so both passes (and their kvrow
                # writes) are emitted strictly before lt == npos_lt
                kv_work = ([("mm", 0, c) for c in range(DC)] + [("add", 0, 0)]
                           + [("mm", 1, c) for c in range(DC)] + [("add", 1, 0)])
                kvstate = {}

                def kv_emit_one(item, st=kvstate):
                    kind, which, c = item
                    if kind == "add":
                        kv_add(st["kv"], which)
                        return
                    if c == 0:
                        st["kv"] = kv_pool.tile([1, HP * DK], F32,
                                                name=f"kv{which}", tag="kv")
                    kv_mm(st["kv"], c, which)

                n_slots = npos_lt - 1          # ride slots: lt 0..npos_lt-2
                n_pre = max(0, len(kv_work) - 2 * n_slots)
                for item in kv_work[:n_pre]:
                    kv_emit_one(item)
                rest = kv_work[n_pre:]
                for k, item in enumerate(rest):
                    rides[k // 2].append(
                        (lambda it=item: kv_emit_one(it)))

            o_staged = {}
            o_post = []
            if h == HP - 1 and LC >= DC:
                # S_3 has no Q to ride; its psq/kv PSUM banks are dead. Ride
                # the first-3-chunk partials of 16 output tiles there, staged
                # to SBUF; the O phase finishes them with one matmul + add.
                wos = [wo_pool.tile([128, D], F16, name=f"wo{c}", tag=f"wo{c}")
                       for c in range(HP)]

                def mk_wo(c):
                    return lambda: nc.sync.dma_start(
                        wos[c][:], wo_d[c * 128:(c + 1) * 128, :])

                o_tiles = [(s_t, mg) for s_t in (6, 7) for mg in range(D // 512)]
                o_state = {}

                def mk_o(item, st=o_state):
                    t, k = item
                    s_t, mg = o_tiles[t]

                    def emit():
                        if k == 0:
                            pool = kv_pool if t % 2 == 0 else psq
                            st["ps"] = pool.tile(
                                [128, 512], F32, name=f"ops{t}",
                                tag="kv" if t % 2 == 0 else "psq")
                        if k < 3:
                            nc.tensor.matmul(
                                st["ps"][:],
                                ctxTs[k][:, s_t * 128:(s_t + 1) * 128],
                                wos[k][:, mg * 512:(mg + 1) * 512],
                                start=(k == 0), stop=(k == 2))
                        else:
                            sg = stage_pool.tile([128, 512], F16,
                                                 name=f"sg{t}", tag=f"sg{t}")
                            nc.vector.tensor_copy(sg[:], st["ps"][:])
                            o_staged[(s_t, mg)] = sg
                    return emit

                # wo0/wo1 load right after S_3's first K/V group; wo2/wo3
                # trail via the ride slots they're needed in
                o_post.extend([mk_wo(0), mk_wo(1)])
                rides[2].append(mk_wo(2))
                rides[10].append(mk_wo(3))
                o_work = [(t, k) for t in range(len(o_tiles)) for k in range(4)]
                for idx, item in enumerate(o_work):
                    rides[6 + idx // 3].append(mk_o(item))

            psc_t = psc.tile([128, S], F32, name=f"psc{h}", tag="psc")
            cur = load_group(h, 0)
            for fn_ in o_post:
                fn_()
            if not (h == 0 and ride_kv) and npos_g == 0 and npos_lt < LC:
                new_entry_writes(h, *cur)
            nxt = None
            pend = None              # lag-1 ctx: (lt, wt, vst)
            for lt in range(LC):
                g, j = lt // 8, lt % 8
                if j == 0 and g > 0:
                    cur = nxt
                if j == 0 and g + 1 < (LC + 7) // 8:
                    nxt = load_group(h, g + 1)
                    if not (h == 0 and ride_kv) and npos_g == g + 1:
                        new_entry_writes(h, *nxt)
                kt8, v8 = cur
                if h == 0 and ride_kv and lt == npos_lt:
                    # kvrow writes were emitted at lt <= npos_lt - 1
                    new_entry_writes(h, kt8, v8) if npos_g == g else None
                    if npos_g == g + 1 and nxt is not None:
                        new_entry_writes(h, *nxt)

                ps = pss.tile([128, 1024], F32, name=f"ps_{h}_{lt}", tag="pss")
                ksl = kt8[:, j * 128:(j + 1) * 128]
                nc.tensor.matmul(ps[:, 0:512], ksl, qT_t[:, 0:512])
                nc.tensor.matmul(ps[:, 512:1024], ksl, qT_t[:, 512:1024])

                for emit in rides[lt]:
                    emit()

                wt = wt_pool.tile([128, 1024], F32R, name=f"wt_{h}_{lt}", tag="wt")
                ssum = ss_pool.tile([128, 1], F32, name=f"ss_{h}_{lt}", tag="ssum")
                nc.scalar.activation(wt[:], ps[:], AF.Exp, scale=INV, accum_out=ssum[:])
                rec = ss_pool.tile([128, 1], F32, name=f"rc_{h}_{lt}", tag="rec")
                nc.vector.reciprocal(rec[:], ssum[:])
                vst = vs_pool.tile([128, DK], F32R, name=f"vs{h}_{lt}", tag="vs")
                nc.vector.tensor_scalar_mul(vst[:], v8[:, j * 128:(j + 1) * 128], rec[:])

                if pend is not None:
                    plt, pwt, pvst = pend
                    nc.tensor.matmul(psc_t[:, 0:512], pvst[:], pwt[:, 0:512],
                                     start=(plt == 0), stop=False)
                    nc.tensor.matmul(psc_t[:, 512:1024], pvst[:], pwt[:, 512:1024],
                                     start=(plt == 0), stop=False)
                pend = (lt, wt, vst)
            plt, pwt, pvst = pend
            nc.tensor.matmul(psc_t[:, 0:512], pvst[:], pwt[:, 0:512],
                             start=(plt == 0), stop=True)
            nc.tensor.matmul(psc_t[:, 512:1024], pvst[:], pwt[:, 512:1024],
                             start=(plt == 0), stop=True)
            nc.vector.tensor_copy(ctxTs[h][:], psc_t[:])
            if h + 1 < HP and not ride_q:
                # dense fallback Q projection for the next head
                bq1 = ss_pool.tile([128, 1], F32, name=f"bq{h+1}", tag="bq",
                                   bufs=2)
                nc.sync.dma_start(bq1[:], bq_d[h + 1])
                qT_next = qT_pool.tile([128, S], F16, name=f"qT{h+1}", tag="qT")
                wqts_fb = {}
                for half in range(2):
                    psq_t = psq.tile([128, 512], F32,
                                     name=f"psq{h+1}_{half}", tag="psq")
                    for c in range(DC):
                        if half == 0 and c % 4 == 0:
                            wqts_fb[c // 4] = emit_wq_dma(h + 1, c // 4)
                        q_half_mm(psq_t, wqts_fb[c // 4], c, half)
                    q_half_add(h + 1, qT_next, psq_t, half, bq1)
            if h + 1 < HP:
                qT_t = qT_next

        # release attention-phase pools before the output projection (LIFO)
        for p in (psc, pss, kv_pool, psq,
                  ss_pool, vs_pool, wt_pool, v_pool, kt_pool,
                  wkv_pool, wq_pool, small, qT_pool, xT_pool):
            p.release()

        # ---------- output projection: out[s, m] partial ----------
        # Wo fully resident in the space freed by xT; one 16KB-burst output
        # DMA per s-tile.
        ob_pool = tc.alloc_tile_pool(name="obp", bufs=3)
        pso = tc.alloc_tile_pool(name="pso", bufs=4, space="PSUM")
        if not o_staged:
            # fallback path (short sequences): load Wo here
            wos = []
            for c in range(HP):
                wot = wo_pool.tile([128, D], F16, name=f"wo{c}", tag=f"wo{c}")
                nc.sync.dma_start(wot[:], wo_d[c * 128:(c + 1) * 128, :])
                wos.append(wot)
        for s_t in range(S // 128):
            ob = ob_pool.tile([128, D], F16, name=f"ob{s_t}", tag="ob")
            for mg in range(D // 512):
                sg = o_staged.get((s_t, mg))
                pso_t = pso.tile([128, 512], F32, name=f"po{s_t}_{mg}", tag="pso")
                if sg is not None:
                    nc.tensor.matmul(pso_t[:],
                                     ctxTs[HP - 1][:, s_t * 128:(s_t + 1) * 128],
                                     wos[HP - 1][:, mg * 512:(mg + 1) * 512])
                    nc.vector.tensor_add(ob[:, mg * 512:(mg + 1) * 512],
                                         sg[:], pso_t[:])
                else:
                    for c in range(HP):
                        nc.tensor.matmul(pso_t[:],
                                         ctxTs[c][:, s_t * 128:(s_t + 1) * 128],
                                         wos[c][:, mg * 512:(mg + 1) * 512],
                                         start=(c == 0), stop=(c == HP - 1))
                    nc.vector.tensor_copy(ob[:, mg * 512:(mg + 1) * 512], pso_t[:])
            if s_t == S // 128 - 1:
                # stream the final tile's output per mg-pair: the exposed
                # post-compute transfer shrinks to a quarter row-band
                for q in range(8):
                    nc.sync.dma_start(
                        out_d[s_t * 128:(s_t + 1) * 128,
                              q * (D // 8):(q + 1) * (D // 8)],
                        ob[:, q * (D // 8):(q + 1) * (D // 8)])
            else:
                nc.sync.dma_start(out_d[s_t * 128:(s_t + 1) * 128, :], ob[:])
        for p in (pso, ob_pool, stage_pool, wo_pool, ctxT_pool):
            p.release()

    nc.compile()
    return nc


# e5m2: Wk/Wv entries (sigma ~1/64) stay normal (min normal 2^-14), so no
# pre-scaling is needed and the bias-add stays a plain tensor_add. The new
# cache entry is 1 of 4096 rows, so its ~7% quantization error contributes
# ~0.1% to the context.
FP8 = mybir.dt.float8e5


def build_fast(pos: int):
    """Specialized build for the harness regime (pos=4095, L=4096=DC*128).

    Structural changes vs ``build``:
      - Phase A projects heads 0..2 concurrently (6 PSUM banks), paced by the
        interleaved wq/x DMA stream, with x group 0 split into 4 chunk DMAs so
        the first matmul starts ~2us in. Head 3's Q projection rides inside
        head 0's S loop as before.
      - kv_new runs TRANSPOSED: stationary = fp8 wkv [128d x 128col] chunks,
        moving = fp8 x_last chunk [128,1], out = one PSUM column per
        (head, k/v). 256 rank-1 matmuls cost ~0.4ns each in PE time vs the
        13.6us the 64 row-form N=512 matmuls cost.
      - No staged-O riding: the output projection runs as a single PE-bound
        phase at the end with the psum->SBUF copies on the ACT engine (idle
        there), so PE never waits on DVE.
      - Last output tile DMA'd in 4 chunks so only ~1 chunk is exposed.
    """
    L = pos + 1
    LC = L // 128
    NG = LC // 8
    npos_g = pos // 1024
    INV = 1.0 / math.sqrt(DK)
    assert LC == DC and S == 1024 and npos_g == NG - 1

    nc = bacc.Bacc("TRN2", target_bir_lowering=False, debug=False,
                   num_devices=NCORES)

    xT_d = nc.dram_tensor("xT", [D, S], F16, kind="ExternalInput").ap()
    wq_d = nc.dram_tensor("wq", [HP, D, DK], F16, kind="ExternalInput").ap()
    wkv_d = nc.dram_tensor("wkv", [D, 2 * HP * DK], FP8, kind="ExternalInput").ap()
    xl_d = nc.dram_tensor("xl", [128, DC], FP8, kind="ExternalInput").ap()
    bq_d = nc.dram_tensor("bq", [DK, HP], F32, kind="ExternalInput").ap()
    bkv_d = nc.dram_tensor("bkv", [DK, 2 * HP], F32, kind="ExternalInput").ap()
    kT_d = nc.dram_tensor("kT", [HP, DK, pos], F16, kind="ExternalInput").ap()
    v_d = nc.dram_tensor("v", [HP, pos, DK], F16, kind="ExternalInput").ap()
    wo_d = nc.dram_tensor("wo", [HP * DK, D], F16, kind="ExternalInput").ap()
    out_d = nc.dram_tensor("out", [S, D], F16, kind="ExternalOutput").ap()

    with tile.TileContext(nc) as tc:
        ctxT_pool = tc.alloc_tile_pool(name="ctxT", bufs=1)
        wo_pool = tc.alloc_tile_pool(name="wop", bufs=1)
        xT_pool = tc.alloc_tile_pool(name="xT", bufs=1)
        qT_pool = tc.alloc_tile_pool(name="qT", bufs=4)
        small = tc.alloc_tile_pool(name="smallp", bufs=1)
        sg_pool = tc.alloc_tile_pool(name="sgp", bufs=1)
        wq_pool = tc.alloc_tile_pool(name="wqp", bufs=12)
        kt_pool = tc.alloc_tile_pool(name="ktp", bufs=4)
        v_pool = tc.alloc_tile_pool(name="vp", bufs=4)
        wt_pool = tc.alloc_tile_pool(name="wtp", bufs=5)
        vs_pool = tc.alloc_tile_pool(name="vsp", bufs=10)
        ss_pool = tc.alloc_tile_pool(name="ssp", bufs=8)

        # PSUM (8 banks): psq 1 + kv 1 + pss 2x[128,1024] (4) + psc 2.
        # Phase A borrows pss slot0/slot1 + psc for the three Q accumulators.
        psq = tc.alloc_tile_pool(name="psq", bufs=1, space="PSUM")
        kv_pool = tc.alloc_tile_pool(name="kvp", bufs=1, space="PSUM")
        pss = tc.alloc_tile_pool(name="pss", bufs=2, space="PSUM")
        psc = tc.alloc_tile_pool(name="psc", bufs=1, space="PSUM")
        # top of stack: released after the kv rides (start of h2's S loop)
        # to make room for the staged-O sg tiles
        wkv_pool = tc.alloc_tile_pool(name="wkvp", bufs=5)

        ctxTs = [ctxT_pool.tile([128, S], F16, name=f"cT{h}", tag=f"cT{h}")
                 for h in range(HP)]

        # ---------- phase A: DMA emissions ----------
        wq_tiles = {}

        def emit_wq(h, g):
            t = wq_pool.tile([128, 4 * DK], F16, name=f"wq{h}_{g}", tag="wq")
            nc.sync.dma_start(
                t[:], wq_d[h, g * 512:(g + 1) * 512, :].rearrange(
                    "(i p) k -> p i k", p=128))
            wq_tiles[(h, g)] = t

        xbig = [xT_pool.tile([128, 4 * S], F16, name=f"xt{g}", tag=f"xt{g}")
                for g in range(DC // 4)]
        emit_wq(0, 0)
        nc.sync.dma_start(xbig[0][:, 0:512], xT_d[0:128, 0:512])
        nc.sync.dma_start(xbig[0][:, 512:S], xT_d[0:128, 512:S])
        for i in range(1, 4):  # group 0 split per chunk for a fast start
            nc.sync.dma_start(xbig[0][:, i * S:(i + 1) * S],
                              xT_d[i * 128:(i + 1) * 128, :])
        emit_wq(1, 0)
        emit_wq(2, 0)
        for h in range(3):
            emit_wq(h, 1)
        bq_t = small.tile([DK, HP], F32, name="bqall", tag="bqall")
        bkv_t = small.tile([DK, 2 * HP], F32, name="bkvt", tag="bkvt")
        xl_t = small.tile([128, DC], FP8, name="xlt", tag="xlt")
        for g in range(1, DC // 4):
            # x leads its segment (PE unblocks on it); the wq tiles for
            # group g+1 ride behind, arriving a full segment early
            nc.sync.dma_start(
                xbig[g][:, 0:2 * S],
                xT_d[g * 512:g * 512 + 256, :].rearrange(
                    "(i p) s -> p i s", p=128))
            nc.sync.dma_start(
                xbig[g][:, 2 * S:4 * S],
                xT_d[g * 512 + 256:(g + 1) * 512, :].rearrange(
                    "(i p) s -> p i s", p=128))
            if g + 1 < DC // 4:
                for h in range(3):
                    emit_wq(h, g + 1)
            if g == 3:
                # small constants ride behind the early x groups; needed
                # first at the phase-A bias adds / S-h0 kv rides
                nc.sync.dma_start(bq_t[:], bq_d[:])
                nc.sync.dma_start(bkv_t[:], bkv_d[:])
                nc.sync.dma_start(xl_t[:], xl_d[:])

        def xsl(c, lo, sz):
            return xbig[c // 4][:, (c % 4) * S + lo:(c % 4) * S + lo + sz]

        # ---------- phase A: compute emissions ----------
        qAcc = [pss.tile([128, S], F32, name="qacc0", tag="pss"),
                pss.tile([128, S], F32, name="qacc1", tag="pss"),
                psc.tile([128, S], F32, name="qacc2", tag="psc")]
        qTs = [qT_pool.tile([128, S], F16, name=f"qT{h}", tag="qT")
               for h in range(HP)]
        for g in range(DC // 4):
            for h in range(3):
                for c in range(4 * g, 4 * g + 4):
                    lhs = wq_tiles[(h, g)][:, (c % 4) * DK:(c % 4 + 1) * DK]
                    nc.tensor.matmul(qAcc[h][:, 0:512], lhs, xsl(c, 0, 512),
                                     start=(c == 0), stop=(c == DC - 1))
                    nc.tensor.matmul(qAcc[h][:, 512:1024], lhs, xsl(c, 512, 512),
                                     start=(c == 0), stop=(c == DC - 1))
        for h in range(3):
            for hf in range(2):
                nc.vector.tensor_scalar_add(
                    qTs[h][:, hf * 512:(hf + 1) * 512],
                    qAcc[h][:, hf * 512:(hf + 1) * 512], bq_t[:, h:h + 1])

        # ---------- post-phase-A DMA block (queue order matters) ----------
        def load_group(h, g):
            g0 = g * 1024
            gl = min(1024, L - g0)
            gc = max(0, min(1024, pos - g0))
            kt8 = kt_pool.tile([128, 1024], F16, name=f"kt{h}_{g}", tag="kt")
            if gc > 0:
                nc.sync.dma_start(kt8[:, 0:gc], kT_d[h, :, g0:g0 + gc])
            if gl < 1024:
                nc.vector.memset(kt8[:, gl:1024], 0.0)
            v8 = v_pool.tile([128, 1024], F16, name=f"v{h}_{g}", tag="v")
            if gl < 1024:
                nc.vector.memset(v8[:, (gl // 128) * 128:1024], 0.0)
            fc = gc // 128
            if fc > 0:
                nc.sync.dma_start(
                    v8[:, 0:fc * 128],
                    v_d[h, g0:g0 + fc * 128, :].rearrange(
                        "(i p) k -> p i k", p=128))
            rem = gc - fc * 128
            if rem > 0:
                nc.sync.dma_start(v8[0:rem, fc * 128:(fc + 1) * 128],
                                  v_d[h, g0 + fc * 128:g0 + gc, :])
            return kt8, v8

        groups0 = [load_group(0, 0)]
        for g in range(DC // 4):
            emit_wq(3, g)
        for g in range(1, NG):
            groups0.append(load_group(0, g))
        wkv_tiles = []
        for jt in range(8):
            t = wkv_pool.tile([128, 4 * 2 * HP * DK], FP8,
                              name=f"wkv{jt}", tag="wkv")
            nc.sync.dma_start(
                t[:], wkv_d[jt * 512:(jt + 1) * 512, :].rearrange(
                    "(i p) k -> p i k", p=128))
            wkv_tiles.append(t)
        wos = []
        for cblk in range(HP):
            wot = wo_pool.tile([128, D], F16, name=f"wo{cblk}", tag=f"wo{cblk}")
            nc.sync.dma_start(wot[:], wo_d[cblk * 128:(cblk + 1) * 128, :])
            wos.append(wot)

        # ---------- S loops ----------
        kvf = small.tile([DK, 2 * HP], F16, name="kvf", tag="kvf")
        gp = pos % 1024

        def new_entry_writes(h, kt8, v8):
            nc.sync.dma_start(kt8[:, gp:gp + 1], kvf[:, h:h + 1])
            nc.sync.dma_start(
                v8[gp % 128:gp % 128 + 1,
                   (gp // 128) * 128:(gp // 128 + 1) * 128],
                kvf[:, HP + h:HP + h + 1])

        kvacc = {}
        q3state = {}
        o_staged = {}        # (s_t, mg) -> (sg_tile, chunks_staged)
        o2state = {}

        def mk_o(loop_h, t, ck, nck):
            # staged-O ride: accumulate the first `nck` Wo chunks for tile t
            # (s_t, mg) into a spare PSUM bank, stage to SBUF; the O phase
            # finishes the remaining chunks and adds. h2 stages 2 chunks
            # (ctx0/1 known), h3 stages 3.
            base = 0 if loop_h == 2 else 16
            s_t, mg = divmod(base + t, 8)

            def emit():
                if ck == 0:
                    pool, tag = ((psq, "psq") if t % 2 == 0 else
                                 (kv_pool, "kv"))
                    o2state[(loop_h, t)] = pool.tile(
                        [128, 512], F32, name=f"o{loop_h}_{t}", tag=tag)
                acc = o2state[(loop_h, t)]
                nc.tensor.matmul(acc[:],
                                 ctxTs[ck][:, s_t * 128:(s_t + 1) * 128],
                                 wos[ck][:, mg * 512:(mg + 1) * 512],
                                 start=(ck == 0), stop=(ck == nck - 1))
                if ck == nck - 1:
                    sg = sg_pool.tile([128, 512], F16, name=f"sg{s_t}_{mg}",
                                      tag=f"sg{s_t}_{mg}")
                    nc.vector.tensor_copy(sg[:], acc[:])
                    o_staged[(s_t, mg)] = (sg, nck)
            return emit

        def mk_q3(half, c):
            # one chunk-matmul of head 3's Q per ride slot: half 0 rides in
            # h0's loop, half 1 in h1's — both loops stay just above ACT pace
            def emit():
                if c == 0:
                    q3state["psq"] = psq.tile([128, 512], F32,
                                              name=f"psq3_{half}", tag="psq")
                lhs = wq_tiles[(3, c // 4)][:, (c % 4) * DK:(c % 4 + 1) * DK]
                nc.tensor.matmul(q3state["psq"][:], lhs, xsl(c, half * 512, 512),
                                 start=(c == 0), stop=(c == DC - 1))
                if c == DC - 1:
                    nc.vector.tensor_scalar_add(
                        qTs[3][:, half * 512:(half + 1) * 512],
                        q3state["psq"][:], bq_t[:, 3:4])
            return emit

        for h in range(HP):
            rides = [[] for _ in range(LC)]
            if h in (0, 1):
                for lt in range(DC):
                    rides[lt].append(mk_q3(h, lt))
            if h == 2:
                # wkv is dead after h0's kv rides
                wkv_pool.release()
                # light staged-O riding: ~100ns/lt of DVE headroom under the
                # ACT pace allows one sg copy every ~6 lt
                for t in range(7):
                    for ck in range(2):
                        rides[4 * t + 2 * ck].append(mk_o(2, t, ck, 2))
            if h == 3:
                for t in range(7):
                    for ck in range(3):
                        rides[4 * t + ck].append(mk_o(3, t, ck, 3))
            if h == 0:

                def mk_kv(ci):
                    def emit():
                        if ci == 0:
                            kvacc["t"] = kv_pool.tile(
                                [128, 2 * HP], F32, name="kvt", tag="kv")
                        kt = kvacc["t"]
                        wt_ = wkv_tiles[ci]
                        for cc in range(4):
                            c = 4 * ci + cc
                            for jj in range(2 * HP):
                                nc.tensor.matmul(
                                    kt[:, jj:jj + 1],
                                    wt_[:, cc * 1024 + jj * DK:
                                        cc * 1024 + (jj + 1) * DK],
                                    xl_t[:, c:c + 1],
                                    start=(c == 0), stop=(c == DC - 1))
                    return emit

                for ci in range(8):
                    rides[2 + ci].append(mk_kv(ci))

                def kv_finish():
                    nc.vector.tensor_add(kvf[:], kvacc["t"][:], bkv_t[:])
                rides[10].append(kv_finish)

            pstate = {}
            psc_t = psc.tile([128, S], F32, name=f"psc{h}", tag="psc")
            cur = groups0[0] if h == 0 else load_group(h, 0)
            qT_t = qTs[h]
            nxt = None
            pends = []

            def ctx_mm(item, stop):
                plt, pwt, pvst = item
                nc.tensor.matmul(psc_t[:, 0:512], pvst[:], pwt[:, 0:512],
                                 start=(plt == 0), stop=stop)
                nc.tensor.matmul(psc_t[:, 512:1024], pvst[:], pwt[:, 512:1024],
                                 start=(plt == 0), stop=stop)

            for lt in range(LC):
                g, j = lt // 8, lt % 8
                if j == 0 and g > 0:
                    cur = nxt
                if j == 0 and g + 1 < NG:
                    nxt = groups0[g + 1] if h == 0 else load_group(h, g + 1)
                    if npos_g == g + 1:
                        new_entry_writes(h, *nxt)
                kt8, v8 = cur

                ps = pss.tile([128, 1024], F32, name=f"ps_{h}_{lt}", tag="pss")
                ksl = kt8[:, j * 128:(j + 1) * 128]
                nc.tensor.matmul(ps[:, 0:512], ksl, qT_t[:, 0:512])
                nc.tensor.matmul(ps[:, 512:1024], ksl, qT_t[:, 512:1024])

                for emit in rides[lt]:
                    emit()

                # Exp per lt into a shared 2-lt wt tile; the DVE row-sum
                # tree runs BATCHED per pair via strided 3D aps (~0.72us/lt
                # vs 0.84 unbatched), keeping ACT the steady pacer. An ACT
                # accum_out would make its service time exceed the loop
                # period and cascade stalls through the score-bank rotation.
                if lt % 2 == 0:
                    pstate["wtp"] = wt_pool.tile([128, 2048], F16,
                                                 name=f"wt_{h}_{lt}", tag="wt")
                wtp = pstate["wtp"]
                half = lt % 2
                nc.scalar.activation(wtp[:, half * 1024:(half + 1) * 1024],
                                     ps[:], AF.Exp, scale=INV)
                if half == 1:
                    wv = wtp.rearrange("p (l c) -> p l c", c=1024)
                    r1 = ss_pool.tile([128, 2, 512], F16, name=f"r1_{h}_{lt}",
                                      tag="r1", bufs=2)
                    nc.vector.tensor_add(r1[:], wv[:, :, 0:512],
                                         wv[:, :, 512:1024])
                    r2 = ss_pool.tile([128, 2, 256], F16, name=f"r2_{h}_{lt}",
                                      tag="r2", bufs=2)
                    nc.vector.tensor_add(r2[:], r1[:, :, 0:256],
                                         r1[:, :, 256:512])
                    r3 = ss_pool.tile([128, 2, 128], F16, name=f"r3_{h}_{lt}",
                                      tag="r3", bufs=2)
                    nc.vector.tensor_add(r3[:], r2[:, :, 0:128],
                                         r2[:, :, 128:256])
                    ssum = ss_pool.tile([128, 2], F32, name=f"ss_{h}_{lt}",
                                        tag="ssum")
                    nc.vector.tensor_reduce(ssum[:], r3[:],
                                            mybir.AxisListType.X,
                                            mybir.AluOpType.add)
                    rec = ss_pool.tile([128, 2], F32, name=f"rc_{h}_{lt}",
                                       tag="rec")
                    nc.vector.reciprocal(rec[:], ssum[:])
                    for q in range(2):
                        jq = j - 1 + q
                        vst = vs_pool.tile([128, DK], F16,
                                           name=f"vs{h}_{lt}_{q}", tag="vs")
                        nc.vector.tensor_scalar_mul(
                            vst[:], v8[:, jq * 128:(jq + 1) * 128],
                            rec[:, q:q + 1])
                        pends.append((lt - 1 + q, wv[:, q, :], vst))

                # lag >=4: the exp -> batched tree -> recip -> scale chain
                # spans over two loop periods
                while len(pends) > 8:
                    ctx_mm(pends.pop(0), stop=False)
            while len(pends) > 1:
                ctx_mm(pends.pop(0), stop=False)
            plt, pwt, pvst = pends.pop(0)
            nc.tensor.matmul(psc_t[:, 0:512], pvst[:], pwt[:, 0:512],
                             start=(plt == 0), stop=True)
            nc.vector.tensor_copy(ctxTs[h][:, 0:512], psc_t[:, 0:512])
            nc.tensor.matmul(psc_t[:, 512:1024], pvst[:], pwt[:, 512:1024],
                             start=(plt == 0), stop=True)
            nc.vector.tensor_copy(ctxTs[h][:, 512:1024], psc_t[:, 512:1024])

        # release attention-phase pools before the output projection (LIFO;
        # wkv was already released at the start of h2's loop)
        for p in (psc, pss, kv_pool, psq,
                  ss_pool, vs_pool, wt_pool, v_pool, kt_pool, wq_pool):
            p.release()

        # ---------- output projection ----------
        ob_pool = tc.alloc_tile_pool(name="obp", bufs=3)
        pso = tc.alloc_tile_pool(name="pso", bufs=6, space="PSUM")
        last_t = S // 128 - 1

        def emit_o_tile(s_t, mg, ob):
            obsl = ob[:, mg * 512:(mg + 1) * 512]
            staged = o_staged.get((s_t, mg))
            pso_t = pso.tile([128, 512], F32, name=f"po{s_t}_{mg}", tag="pso")
            c0 = staged[1] if staged else 0
            for cblk in range(c0, HP):
                nc.tensor.matmul(pso_t[:],
                                 ctxTs[cblk][:, s_t * 128:(s_t + 1) * 128],
                                 wos[cblk][:, mg * 512:(mg + 1) * 512],
                                 start=(cblk == c0), stop=(cblk == HP - 1))
            if staged:
                # DVE is idle in the O phase; ACT carries the plain copies
                nc.vector.tensor_add(obsl, staged[0][:], pso_t[:])
            elif s_t == last_t and mg >= 6:
                # last tiles: copy on DVE so the final copy runs parallel
                # to ACT's mg6 copy, shortening the end-of-kernel chain
                nc.vector.tensor_copy(obsl, pso_t[:])
            else:
                nc.scalar.activation(obsl, pso_t[:], AF.Copy)

        for s_t in range(S // 128):
            ob = ob_pool.tile([128, D], F16, name=f"ob{s_t}", tag="ob")
            # s_t 0: unstaged tiles first — their ctx0-2 chunks fill the
            # window while the h3 ctxT copy (needed by every chunk-3 matmul
            # and every staged tile) drains
            mgs = (5, 6, 7, 0, 1, 2, 3, 4) if s_t == 0 else range(D // 512)
            for mg in mgs:
                emit_o_tile(s_t, mg, ob)
                if s_t == last_t and mg >= 2:
                    # stream the final tile as it completes (HWDGE executes
                    # in order, so the big head chunk is emitted first and
                    # only the last 128KB chunk's transfer is exposed)
                    lo = 0 if mg == 2 else mg * 512
                    nc.sync.dma_start(
                        out_d[s_t * 128:(s_t + 1) * 128, lo:(mg + 1) * 512],
                        ob[:, lo:(mg + 1) * 512])
            if s_t != last_t:
                nc.sync.dma_start(out_d[s_t * 128:(s_t + 1) * 128, :], ob[:])
        for p in (pso, ob_pool, sg_pool, small, qT_pool, xT_pool,
                  wo_pool, ctxT_pool):
            p.release()

    nc.compile()
    return nc


_CACHE = {}
LAST_EXEC_NS = None


def kernel(x, k_cache, v_cache, Wq, bq, Wk, bk, Wv, bv, Wo, bo, pos):
    global LAST_EXEC_NS
    pos = int(pos)

    def f32(a):
        return np.ascontiguousarray(np.asarray(a), dtype=np.float32)

    x = f32(x)
    k_cache, v_cache = f32(k_cache), f32(v_cache)
    Wq, Wk, Wv, Wo = f32(Wq), f32(Wk), f32(Wv), f32(Wo)
    bq, bk, bv, bo = f32(bq), f32(bk), f32(bv), f32(bo)

    fast = (pos + 1 == 4096 and x.shape == (1, S, D))
    xT = np.ascontiguousarray(x[0].T.astype(np.float16))   # [D, S]
    in_maps = []
    for i in range(NCORES):
        hs = slice(i * HP, (i + 1) * HP)
        m = {
            "xT": xT,
            "wq": np.ascontiguousarray(Wq[hs].astype(np.float16)),
            "kT": np.ascontiguousarray(
                k_cache[hs, :pos, :].transpose(0, 2, 1).astype(np.float16)),
            "v": np.ascontiguousarray(v_cache[hs, :pos, :].astype(np.float16)),
            "wo": np.ascontiguousarray(
                Wo[i * HP * DK:(i + 1) * HP * DK].astype(np.float16)),
        }
        wkv_f32 = np.concatenate([
            Wk[hs].transpose(1, 0, 2).reshape(D, HP * DK),
            Wv[hs].transpose(1, 0, 2).reshape(D, HP * DK)], axis=1)
        if fast:
            m["wkv"] = np.ascontiguousarray(
                wkv_f32.astype(ml_dtypes.float8_e5m2))
            m["xl"] = np.ascontiguousarray(
                x[0, -1].reshape(DC, 128).T.astype(ml_dtypes.float8_e5m2))
            m["bq"] = np.ascontiguousarray(bq[hs].T)                 # [DK, HP]
            m["bkv"] = np.ascontiguousarray(
                np.concatenate([bk[hs].T, bv[hs].T], axis=1))        # [DK, 2HP]
        else:
            m["wkv"] = np.ascontiguousarray(wkv_f32.astype(ml_dtypes.bfloat16))
            m["xl"] = np.ascontiguousarray(
                x[0, -1].reshape(DC, 128).T.astype(ml_dtypes.bfloat16))
            m["bq"] = np.ascontiguousarray(bq[hs].reshape(HP, DK, 1))
            m["bkv"] = np.ascontiguousarray(np.concatenate(
                [bk[hs].reshape(-1), bv[hs].reshape(-1)])[None, :])
        in_maps.append(m)

    if pos not in _CACHE:
        _CACHE[pos] = build_fast(pos) if fast else build(pos)
    nc = _CACHE[pos]

    res = run_bass_kernel_spmd(nc, in_maps, core_ids=list(range(NCORES)))
    LAST_EXEC_NS = res.exec_time_ns

    acc = np.zeros((S, D), np.float64)
    for r in res.results:
        acc += r["out"]
    out = (acc + bo.astype(np.float64)).astype(np.float32)
    return out[None]



# revision 67
# speedup vs baseline: 1.1302x; 1.0201x over previous
"""Trainium2 Bass kernel for CachedMultiHeadedAttention (tensor-parallel over heads).

Sharding: 8 cores x 4 heads. Each core computes Q projection + attention for
its 4 heads, then a partial output projection against its 512 rows of Wo.
Host sums the 8 partial outputs (the "all-reduce" done at unshard time) and
adds bo.

The fast path (build_fast, pos=4095) is scheduled against the TimelineSim
cost model; per-phase engine utilization runs 86-97%:
  - Phase A streams x (split-group DMAs) while projecting Q for heads 0-2
    concurrently into 6 PSUM banks; PE is ~93% busy against the DMA stream.
  - S loops run one l-tile per ~1.04us, paced by ACT's exp
    ([128,1024] per tile). Head 3's Q projection rides 1 chunk/lt inside
    h0/h1's loops; the fp8 rank-1 kv_new projection (transposed form,
    256 x ~0.4ns matmuls) rides in h0; 7 staged output-projection tiles
    ride in each of h2/h3's loops, with their psum->SBUF stagings on DVE.
  - The softmax row-sums run on DVE as a 2-l-tile-batched pairwise f16
    add-tree + short reduce (~0.72us/lt; ACT accum_out and TensorScalarPtr
    accum_out are broken in this NEFF path / too slow on ACT). ctx matmuls
    lag 8 tiles behind their scores so the exp->tree->recip->scale chain
    and its cross-engine semaphore hops never backpressure PE.
  - The output projection finishes staged tiles (DVE adds) interleaved with
    full tiles (ACT copies); the final tile streams out per-mg so only the
    last 128KB chunk is exposed.
Precision: streamed operands f16; wkv/x_last fp8 e5m2 (the new cache entry
is 1/4096 rows, ~0.1% context impact); f32 PSUM accumulation; host f64
reduction across cores. Measured end-to-end relative error ~3.8e-3
(tolerance 2e-2).
"""
import math

import numpy as np
import ml_dtypes

import concourse.bass as bass
import concourse.mybir as mybir
import concourse.tile as tile
from concourse import bacc
from concourse.bass_utils import run_bass_kernel_spmd

F32 = mybir.dt.float32
F32R = mybir.dt.float32r
BF16 = mybir.dt.bfloat16
F16 = mybir.dt.float16
AF = mybir.ActivationFunctionType

H, D, DK, S = 32, 4096, 128, 1024
NCORES = 8
HP = H // NCORES          # heads per core
DC = D // 128             # contraction chunks for d_model


def build(pos: int):
    L = pos + 1
    LC = (L + 127) // 128          # number of 128-wide l tiles
    LG = (LC + 7) // 8             # l-tile groups of 8 (1024 l per group)
    INV = 1.0 / math.sqrt(DK)

    nc = bacc.Bacc("TRN2", target_bir_lowering=False, debug=False,
                   num_devices=NCORES)

    xT_d = nc.dram_tensor("xT", [D, S], F16, kind="ExternalInput").ap()
    wq_d = nc.dram_tensor("wq", [HP, D, DK], F16, kind="ExternalInput").ap()
    wkv_d = nc.dram_tensor("wkv", [D, 2 * HP * DK], BF16, kind="ExternalInput").ap()
    xl_d = nc.dram_tensor("xl", [128, DC], BF16, kind="ExternalInput").ap()
    bq_d = nc.dram_tensor("bq", [HP, DK, 1], F32, kind="ExternalInput").ap()
    bkv_d = nc.dram_tensor("bkv", [1, 2 * HP * DK], F32, kind="ExternalInput").ap()
    kT_d = nc.dram_tensor("kT", [HP, DK, pos], F16, kind="ExternalInput").ap()
    v_d = nc.dram_tensor("v", [HP, pos, DK], F16, kind="ExternalInput").ap()
    wo_d = nc.dram_tensor("wo", [HP * DK, D], F16, kind="ExternalInput").ap()
    out_d = nc.dram_tensor("out", [S, D], F16, kind="ExternalOutput").ap()

    with tile.TileContext(nc) as tc:
        # Pools are released LIFO; ctxT survives into the output projection,
        # so it sits at the bottom of the SBUF pool stack.
        ctxT_pool = tc.alloc_tile_pool(name="ctxT", bufs=1)
        wo_pool = tc.alloc_tile_pool(name="wop", bufs=1)
        stage_pool = tc.alloc_tile_pool(name="stagep", bufs=1)
        xT_pool = tc.alloc_tile_pool(name="xT", bufs=1)
        qT_pool = tc.alloc_tile_pool(name="qT", bufs=2)
        small = tc.alloc_tile_pool(name="smallp", bufs=1)
        wq_pool = tc.alloc_tile_pool(name="wqp", bufs=8)
        wkv_pool = tc.alloc_tile_pool(name="wkvp", bufs=3)
        kt_pool = tc.alloc_tile_pool(name="ktp", bufs=3)
        v_pool = tc.alloc_tile_pool(name="vp", bufs=3)
        wt_pool = tc.alloc_tile_pool(name="wtp", bufs=5)
        vs_pool = tc.alloc_tile_pool(name="vsp", bufs=4)
        ss_pool = tc.alloc_tile_pool(name="ssp", bufs=8)

        # PSUM budget (8 banks): psq 1 + kv 1 + pss 4 + psc 2.
        # Q projections and the kv_new projections run in TWO s-half /
        # k-v passes so their accumulators are single-bank.
        psq = tc.alloc_tile_pool(name="psq", bufs=1, space="PSUM")
        kv_pool = tc.alloc_tile_pool(name="kvp", bufs=1, space="PSUM")
        pss = tc.alloc_tile_pool(name="pss", bufs=2, space="PSUM")
        psc = tc.alloc_tile_pool(name="psc", bufs=1, space="PSUM")

        ctxTs = [ctxT_pool.tile([128, S], F16, name=f"cT{h}", tag=f"cT{h}")
                 for h in range(HP)]

        # small constants first (tiny DMAs, ahead of the big streams)
        kvrow = small.tile([1, 2 * HP * DK], F16, name="kvrow", tag="kvrow")
        bkv_t = small.tile([1, 2 * HP * DK], F32, name="bkvt", tag="bkvt")
        nc.sync.dma_start(bkv_t[:], bkv_d[:])
        xl_t = small.tile([128, DC], BF16, name="xlt", tag="xlt")
        nc.sync.dma_start(xl_t[:], xl_d[:])

        # resident xT tiles (8 big tiles of 4 chunks), interleaved with head
        # 0's Q weight groups so the first Q matmuls start after ~2.5MB, not
        # after the full 17MB of x.
        xbig = []
        wq0_groups = []
        for gx in range(DC // 4):
            wqt = wq_pool.tile([128, 4 * DK], F16, name=f"wq0_{gx}", tag="wq")
            nc.sync.dma_start(
                wqt[:], wq_d[0, gx * 512:(gx + 1) * 512, :].rearrange(
                    "(i p) k -> p i k", p=128))
            wq0_groups.append(wqt)
            xt = xT_pool.tile([128, 4 * S], F16, name=f"xt{gx}", tag=f"xt{gx}")
            nc.sync.dma_start(
                xt[:], xT_d[gx * 512:(gx + 1) * 512, :].rearrange(
                    "(i p) s -> p i s", p=128))
            xbig.append(xt)

        def xsl(c, lo, sz):
            return xbig[c // 4][:, (c % 4) * S + lo:(c % 4) * S + lo + sz]

        def emit_wq_dma(h, gw, tag="wq"):
            wqt = wq_pool.tile([128, 4 * DK], F16,
                               name=f"wq{h}_{gw}", tag=tag)
            nc.sync.dma_start(
                wqt[:], wq_d[h, gw * 512:(gw + 1) * 512, :].rearrange(
                    "(i p) k -> p i k", p=128))
            return wqt

        def q_half_mm(psq_t, wqt, c, half):
            lhs = wqt[:, (c % 4) * DK:(c % 4 + 1) * DK]
            nc.tensor.matmul(psq_t[:], lhs, xsl(c, half * 512, 512),
                             start=(c == 0), stop=(c == DC - 1))

        def q_half_add(h, qT_t, psq_t, half, bq_t):
            nc.vector.tensor_scalar_add(qT_t[:, half * 512:(half + 1) * 512],
                                        psq_t[:], bq_t[:])

        kv_cur = {}

        def kv_mm(kv_t, c, which):
            # which: 0 = k_new, 1 = v_new. Weight chunks are DMA'd two at a
            # time — each dma_start costs ~625ns of serialized HWDGE.
            if c % 4 == 0:
                wkvt = wkv_pool.tile([128, 4 * HP * DK], BF16,
                                     name=f"wkv{which}_{c}", tag="wkv")
                nc.sync.dma_start(
                    wkvt[:], wkv_d[c * 128:(c + 4) * 128,
                                   which * HP * DK:(which + 1) * HP * DK]
                    .rearrange("(i p) k -> p i k", p=128))
                kv_cur["t"] = wkvt
            wkvt = kv_cur["t"]
            nc.tensor.matmul(kv_t[0:1, :], xl_t[:, c:c + 1],
                             wkvt[:, (c % 4) * HP * DK:(c % 4 + 1) * HP * DK],
                             start=(c == 0), stop=(c == DC - 1))

        def kv_add(kv_t, which):
            nc.vector.tensor_add(
                kvrow[0:1, which * HP * DK:(which + 1) * HP * DK], kv_t[:],
                bkv_t[0:1, which * HP * DK:(which + 1) * HP * DK])

        def load_group(h, g):
            """Cache-only loads of l-group g (the new-entry writes are
            emitted separately, after kvrow's writes in trace order)."""
            g0 = g * 1024
            gl = min(1024, L - g0)            # valid l in group
            gc = max(0, min(1024, pos - g0))  # of which from cache
            kt8 = kt_pool.tile([128, 1024], F16, name=f"kt{h}_{g}", tag="kt")
            if gc > 0:
                nc.sync.dma_start(kt8[:, 0:gc], kT_d[h, :, g0:g0 + gc])
            if gl < 1024:
                nc.vector.memset(kt8[:, gl:1024], 0.0)
            v8 = v_pool.tile([128, 1024], F16, name=f"v{h}_{g}", tag="v")
            if gl < 1024:
                # zero whole padded chunks first (full partition range — DVE
                # requires 32-aligned partition bases); valid rows are DMA'd
                # over the zeros below.
                nc.vector.memset(v8[:, (gl // 128) * 128:1024], 0.0)
            fc = gc // 128
            if fc > 0:
                nc.sync.dma_start(
                    v8[:, 0:fc * 128],
                    v_d[h, g0:g0 + fc * 128, :].rearrange(
                        "(i p) k -> p i k", p=128))
            rem = gc - fc * 128
            if rem > 0:
                nc.sync.dma_start(v8[0:rem, fc * 128:(fc + 1) * 128],
                                  v_d[h, g0 + fc * 128:g0 + gc, :])
            return kt8, v8

        def new_entry_writes(h, kt8, v8):
            # column/row for l == pos from the biased kvrow
            gp = pos % 1024
            nc.sync.dma_start(kt8[:, gp:gp + 1],
                              kvrow[0:1, h * DK:(h + 1) * DK])
            nc.sync.dma_start(
                v8[gp % 128:gp % 128 + 1, (gp // 128) * 128:(gp // 128 + 1) * 128],
                kvrow[0:1, HP * DK + h * DK:HP * DK + (h + 1) * DK])

        npos_g = pos // 1024            # l-group holding the new entry
        npos_lt = pos // 128            # l-tile index holding the new entry
        # riding is only possible when the S loop is long enough for the
        # 2-instruction-per-lt passes to finish before the new entry is used
        ride_kv = LC >= DC and npos_lt >= 8
        ride_q = LC >= DC

        # ---------- head 0 Q projection (phase A, DMA-paced) ----------
        bq_t = ss_pool.tile([128, 1], F32, name="bq0", tag="bq", bufs=2)
        nc.sync.dma_start(bq_t[:], bq_d[0])
        qT_t = qT_pool.tile([128, S], F16, name="qT0", tag="qT")
        # both s-halves accumulate concurrently (pass B borrows the idle kv
        # bank) so the whole projection rides the x-arrival gaps instead of
        # serializing 6.8us of pass-B matmuls after the stream ends
        psq_a = psq.tile([128, 512], F32, name="psq0_0", tag="psq")
        psq_b = kv_pool.tile([128, 512], F32, name="psq0_1", tag="kv")
        for c in range(DC):
            q_half_mm(psq_a, wq0_groups[c // 4], c, 0)
            q_half_mm(psq_b, wq0_groups[c // 4], c, 1)
        q_half_add(0, qT_t, psq_a, 0, bq_t)
        q_half_add(0, qT_t, psq_b, 1, bq_t)

        if not ride_kv:
            # fallback: dense kv_new before the S loops
            for which in range(2):
                kv_t = kv_pool.tile([1, HP * DK], F32, name=f"kv{which}", tag="kv")
                for c in range(DC):
                    kv_mm(kv_t, c, which)
                kv_add(kv_t, which)

        for h in range(HP):
            # per-lt ride items emitted right after the scores matmuls
            rides = [[] for _ in range(LC)]
            if h + 1 < HP and ride_q:
                bq1 = ss_pool.tile([128, 1], F32, name=f"bq{h+1}", tag="bq",
                                   bufs=2)
                nc.sync.dma_start(bq1[:], bq_d[h + 1])
                qT_next = qT_pool.tile([128, S], F16, name=f"qT{h+1}", tag="qT")
                state = {}

                def mk_q(lt, h1=h + 1, qn=qT_next, bqt=bq1, st=state):
                    def emit():
                        half, c0 = divmod(2 * lt, DC)
                        if c0 == 0 and half == 0:
                            st["wqts"] = {}
                        if c0 == 0:
                            st["psq"] = psq.tile([128, 512], F32,
                                                 name=f"psq{h1}_{half}", tag="psq")
                        for c in (c0, c0 + 1):
                            gw = c // 4
                            if half == 0 and c % 4 == 0:
                                # pass B reuses these resident tiles (8 slots)
                                st["wqts"][gw] = emit_wq_dma(h1, gw)
                            q_half_mm(st["psq"], st["wqts"][gw], c, half)
                        if c0 + 1 == DC - 1:
                            q_half_add(h1, qn, st["psq"], half, bqt)
                    return emit

                for lt in range(DC):
                    rides[lt].append(mk_q(lt))
            if h == 0 and ride_kv:
                # kv_new work items, paced so both passes (and their kvrow
                # writes) are emitted strictly before lt == npos_lt
                kv_work = ([("mm", 0, c) for c in range(DC)] + [("add", 0, 0)]
                           + [("mm", 1, c) for c in range(DC)] + [("add", 1, 0)])
                kvstate = {}

                def kv_emit_one(item, st=kvstate):
                    kind, which, c = item
                    if kind == "add":
                        kv_add(st["kv"], which)
                        return
                    if c == 0:
                        st["kv"] = kv_pool.tile([1, HP * DK], F32,
                                                name=f"kv{which}", tag="kv")
                    kv_mm(st["kv"], c, which)

                n_slots = npos_lt - 1          # ride slots: lt 0..npos_lt-2
                n_pre = max(0, len(kv_work) - 2 * n_slots)
                for item in kv_work[:n_pre]:
                    kv_emit_one(item)
                rest = kv_work[n_pre:]
                for k, item in enumerate(rest):
                    rides[k // 2].append(
                        (lambda it=item: kv_emit_one(it)))

            o_staged = {}
            o_post = []
            if h == HP - 1 and LC >= DC:
                # S_3 has no Q to ride; its psq/kv PSUM banks are dead. Ride
                # the first-3-chunk partials of 16 output tiles there, staged
                # to SBUF; the O phase finishes them with one matmul + add.
                wos = [wo_pool.tile([128, D], F16, name=f"wo{c}", tag=f"wo{c}")
                       for c in range(HP)]

                def mk_wo(c):
                    return lambda: nc.sync.dma_start(
                        wos[c][:], wo_d[c * 128:(c + 1) * 128, :])

                o_tiles = [(s_t, mg) for s_t in (6, 7) for mg in range(D // 512)]
                o_state = {}

                def mk_o(item, st=o_state):
                    t, k = item
                    s_t, mg = o_tiles[t]

                    def emit():
                        if k == 0:
                            pool = kv_pool if t % 2 == 0 else psq
                            st["ps"] = pool.tile(
                                [128, 512], F32, name=f"ops{t}",
                                tag="kv" if t % 2 == 0 else "psq")
                        if k < 3:
                            nc.tensor.matmul(
                                st["ps"][:],
                                ctxTs[k][:, s_t * 128:(s_t + 1) * 128],
                                wos[k][:, mg * 512:(mg + 1) * 512],
                                start=(k == 0), stop=(k == 2))
                        else:
                            sg = stage_pool.tile([128, 512], F16,
                                                 name=f"sg{t}", tag=f"sg{t}")
                            nc.vector.tensor_copy(sg[:], st["ps"][:])
                            o_staged[(s_t, mg)] = sg
                    return emit

                # wo0/wo1 load right after S_3's first K/V group; wo2/wo3
                # trail via the ride slots they're needed in
                o_post.extend([mk_wo(0), mk_wo(1)])
                rides[2].append(mk_wo(2))
                rides[10].append(mk_wo(3))
                o_work = [(t, k) for t in range(len(o_tiles)) for k in range(4)]
                for idx, item in enumerate(o_work):
                    rides[6 + idx // 3].append(mk_o(item))

            psc_t = psc.tile([128, S], F32, name=f"psc{h}", tag="psc")
            cur = load_group(h, 0)
            for fn_ in o_post:
                fn_()
            if not (h == 0 and ride_kv) and npos_g == 0 and npos_lt < LC:
                new_entry_writes(h, *cur)
            nxt = None
            pend = None              # lag-1 ctx: (lt, wt, vst)
            for lt in range(LC):
                g, j = lt // 8, lt % 8
                if j == 0 and g > 0:
                    cur = nxt
                if j == 0 and g + 1 < (LC + 7) // 8:
                    nxt = load_group(h, g + 1)
                    if not (h == 0 and ride_kv) and npos_g == g + 1:
                        new_entry_writes(h, *nxt)
                kt8, v8 = cur
                if h == 0 and ride_kv and lt == npos_lt:
                    # kvrow writes were emitted at lt <= npos_lt - 1
                    new_entry_writes(h, kt8, v8) if npos_g == g else None
                    if npos_g == g + 1 and nxt is not None:
                        new_entry_writes(h, *nxt)

                ps = pss.tile([128, 1024], F32, name=f"ps_{h}_{lt}", tag="pss")
                ksl = kt8[:, j * 128:(j + 1) * 128]
                nc.tensor.matmul(ps[:, 0:512], ksl, qT_t[:, 0:512])
                nc.tensor.matmul(ps[:, 512:1024], ksl, qT_t[:, 512:1024])

                for emit in rides[lt]:
                    emit()

                wt = wt_pool.tile([128, 1024], F32R, name=f"wt_{h}_{lt}", tag="wt")
                ssum = ss_pool.tile([128, 1], F32, name=f"ss_{h}_{lt}", tag="ssum")
                nc.scalar.activation(wt[:], ps[:], AF.Exp, scale=INV, accum_out=ssum[:])
                rec = ss_pool.tile([128, 1], F32, name=f"rc_{h}_{lt}", tag="rec")
                nc.vector.reciprocal(rec[:], ssum[:])
                vst = vs_pool.tile([128, DK], F32R, name=f"vs{h}_{lt}", tag="vs")
                nc.vector.tensor_scalar_mul(vst[:], v8[:, j * 128:(j + 1) * 128], rec[:])

                if pend is not None:
                    plt, pwt, pvst = pend
                    nc.tensor.matmul(psc_t[:, 0:512], pvst[:], pwt[:, 0:512],
                                     start=(plt == 0), stop=False)
                    nc.tensor.matmul(psc_t[:, 512:1024], pvst[:], pwt[:, 512:1024],
                                     start=(plt == 0), stop=False)
                pend = (lt, wt, vst)
            plt, pwt, pvst = pend
            nc.tensor.matmul(psc_t[:, 0:512], pvst[:], pwt[:, 0:512],
                             start=(plt == 0), stop=True)
            nc.tensor.matmul(psc_t[:, 512:1024], pvst[:], pwt[:, 512:1024],
                             start=(plt == 0), stop=True)
            nc.vector.tensor_copy(ctxTs[h][:], psc_t[:])
            if h + 1 < HP and not ride_q:
                # dense fallback Q projection for the next head
                bq1 = ss_pool.tile([128, 1], F32, name=f"bq{h+1}", tag="bq",
                                   bufs=2)
                nc.sync.dma_start(bq1[:], bq_d[h + 1])
                qT_next = qT_pool.tile([128, S], F16, name=f"qT{h+1}", tag="qT")
                wqts_fb = {}
                for half in range(2):
                    psq_t = psq.tile([128, 512], F32,
                                     name=f"psq{h+1}_{half}", tag="psq")
                    for c in range(DC):
                        if half == 0 and c % 4 == 0:
                            wqts_fb[c // 4] = emit_wq_dma(h + 1, c // 4)
                        q_half_mm(psq_t, wqts_fb[c // 4], c, half)
                    q_half_add(h + 1, qT_next, psq_t, half, bq1)
            if h + 1 < HP:
                qT_t = qT_next

        # release attention-phase pools before the output projection (LIFO)
        for p in (psc, pss, kv_pool, psq,
                  ss_pool, vs_pool, wt_pool, v_pool, kt_pool,
                  wkv_pool, wq_pool, small, qT_pool, xT_pool):
            p.release()

        # ---------- output projection: out[s, m] partial ----------
        # Wo fully resident in the space freed by xT; one 16KB-burst output
        # DMA per s-tile.
        ob_pool = tc.alloc_tile_pool(name="obp", bufs=3)
        pso = tc.alloc_tile_pool(name="pso", bufs=4, space="PSUM")
        if not o_staged:
            # fallback path (short sequences): load Wo here
            wos = []
            for c in range(HP):
                wot = wo_pool.tile([128, D], F16, name=f"wo{c}", tag=f"wo{c}")
                nc.sync.dma_start(wot[:], wo_d[c * 128:(c + 1) * 128, :])
                wos.append(wot)
        for s_t in range(S // 128):
            ob = ob_pool.tile([128, D], F16, name=f"ob{s_t}", tag="ob")
            for mg in range(D // 512):
                sg = o_staged.get((s_t, mg))
                pso_t = pso.tile([128, 512], F32, name=f"po{s_t}_{mg}", tag="pso")
                if sg is not None:
                    nc.tensor.matmul(pso_t[:],
                                     ctxTs[HP - 1][:, s_t * 128:(s_t + 1) * 128],
                                     wos[HP - 1][:, mg * 512:(mg + 1) * 512])
                    nc.vector.tensor_add(ob[:, mg * 512:(mg + 1) * 512],
                                         sg[:], pso_t[:])
                else:
                    for c in range(HP):
                        nc.tensor.matmul(pso_t[:],
                                         ctxTs[c][:, s_t * 128:(s_t + 1) * 128],
                                         wos[c][:, mg * 512:(mg + 1) * 512],
                                         start=(c == 0), stop=(c == HP - 1))
                    nc.vector.tensor_copy(ob[:, mg * 512:(mg + 1) * 512], pso_t[:])
            if s_t == S // 128 - 1:
                # stream the final tile's output per mg-pair: the exposed
                # post-compute transfer shrinks to a quarter row-band
                for q in range(8):
                    nc.sync.dma_start(
                        out_d[s_t * 128:(s_t + 1) * 128,
                              q * (D // 8):(q + 1) * (D // 8)],
                        ob[:, q * (D // 8):(q + 1) * (D // 8)])
            else:
                nc.sync.dma_start(out_d[s_t * 128:(s_t + 1) * 128, :], ob[:])
        for p in (pso, ob_pool, stage_pool, wo_pool, ctxT_pool):
            p.release()

    nc.compile()
    return nc


# e5m2: Wk/Wv entries (sigma ~1/64) stay normal (min normal 2^-14), so no
# pre-scaling is needed and the bias-add stays a plain tensor_add. The new
# cache entry is 1 of 4096 rows, so its ~7% quantization error contributes
# ~0.1% to the context.
FP8 = mybir.dt.float8e5


def build_fast(pos: int):
    """Specialized build for the harness regime (pos=4095, L=4096=DC*128).

    Structural changes vs ``build``:
      - Phase A projects heads 0..2 concurrently (6 PSUM banks), paced by the
        interleaved wq/x DMA stream, with x group 0 split into 4 chunk DMAs so
        the first matmul starts ~2us in. Head 3's Q projection rides inside
        head 0's S loop as before.
      - kv_new runs TRANSPOSED: stationary = fp8 wkv [128d x 128col] chunks,
        moving = fp8 x_last chunk [128,1], out = one PSUM column per
        (head, k/v). 256 rank-1 matmuls cost ~0.4ns each in PE time vs the
        13.6us the 64 row-form N=512 matmuls cost.
      - No staged-O riding: the output projection runs as a single PE-bound
        phase at the end with the psum->SBUF copies on the ACT engine (idle
        there), so PE never waits on DVE.
      - Last output tile DMA'd in 4 chunks so only ~1 chunk is exposed.
    """
    L = pos + 1
    LC = L // 128
    NG = LC // 8
    npos_g = pos // 1024
    INV = 1.0 / math.sqrt(DK)
    assert LC == DC and S == 1024 and npos_g == NG - 1

    nc = bacc.Bacc("TRN2", target_bir_lowering=False, debug=False,
                   num_devices=NCORES)

    xT_d = nc.dram_tensor("xT", [D, S], F16, kind="ExternalInput").ap()
    wq_d = nc.dram_tensor("wq", [HP, D, DK], F16, kind="ExternalInput").ap()
    wkv_d = nc.dram_tensor("wkv", [D, 2 * HP * DK], FP8, kind="ExternalInput").ap()
    xl_d = nc.dram_tensor("xl", [128, DC], FP8, kind="ExternalInput").ap()
    bq_d = nc.dram_tensor("bq", [DK, HP], F32, kind="ExternalInput").ap()
    bkv_d = nc.dram_tensor("bkv", [DK, 2 * HP], F32, kind="ExternalInput").ap()
    kT_d = nc.dram_tensor("kT", [HP, DK, pos], F16, kind="ExternalInput").ap()
    v_d = nc.dram_tensor("v", [HP, pos, DK], F16, kind="ExternalInput").ap()
    wo_d = nc.dram_tensor("wo", [HP * DK, D], F16, kind="ExternalInput").ap()
    out_d = nc.dram_tensor("out", [S, D], F16, kind="ExternalOutput").ap()

    with tile.TileContext(nc) as tc:
        ctxT_pool = tc.alloc_tile_pool(name="ctxT", bufs=1)
        wo_pool = tc.alloc_tile_pool(name="wop", bufs=1)
        xT_pool = tc.alloc_tile_pool(name="xT", bufs=1)
        qT_pool = tc.alloc_tile_pool(name="qT", bufs=4)
        small = tc.alloc_tile_pool(name="smallp", bufs=1)
        sg_pool = tc.alloc_tile_pool(name="sgp", bufs=1)
        wq_pool = tc.alloc_tile_pool(name="wqp", bufs=12)
        kt_pool = tc.alloc_tile_pool(name="ktp", bufs=4)
        v_pool = tc.alloc_tile_pool(name="vp", bufs=4)
        wt_pool = tc.alloc_tile_pool(name="wtp", bufs=5)
        vs_pool = tc.alloc_tile_pool(name="vsp", bufs=10)
        ss_pool = tc.alloc_tile_pool(name="ssp", bufs=8)

        # PSUM (8 banks): psq 1 + kv 1 + pss 2x[128,1024] (4) + psc 2.
        # Phase A borrows pss slot0/slot1 + psc for the three Q accumulators.
        psq = tc.alloc_tile_pool(name="psq", bufs=1, space="PSUM")
        kv_pool = tc.alloc_tile_pool(name="kvp", bufs=1, space="PSUM")
        pss = tc.alloc_tile_pool(name="pss", bufs=2, space="PSUM")
        psc = tc.alloc_tile_pool(name="psc", bufs=1, space="PSUM")
        # top of stack: released after the kv rides (start of h2's S loop)
        # to make room for the staged-O sg tiles
        wkv_pool = tc.alloc_tile_pool(name="wkvp", bufs=5)

        ctxTs = [ctxT_pool.tile([128, S], F16, name=f"cT{h}", tag=f"cT{h}")
                 for h in range(HP)]

        # ---------- phase A: DMA emissions ----------
        wq_tiles = {}

        def emit_wq(h, g):
            t = wq_pool.tile([128, 4 * DK], F16, name=f"wq{h}_{g}", tag="wq")
            nc.sync.dma_start(
                t[:], wq_d[h, g * 512:(g + 1) * 512, :].rearrange(
                    "(i p) k -> p i k", p=128))
            wq_tiles[(h, g)] = t

        xbig = [xT_pool.tile([128, 4 * S], F16, name=f"xt{g}", tag=f"xt{g}")
                for g in range(DC // 4)]
        emit_wq(0, 0)
        nc.sync.dma_start(xbig[0][:, 0:512], xT_d[0:128, 0:512])
        nc.sync.dma_start(xbig[0][:, 512:S], xT_d[0:128, 512:S])
        for i in range(1, 4):  # group 0 split per chunk for a fast start
            nc.sync.dma_start(xbig[0][:, i * S:(i + 1) * S],
                              xT_d[i * 128:(i + 1) * 128, :])
        emit_wq(1, 0)
        emit_wq(2, 0)
        for h in range(3):
            emit_wq(h, 1)
        bq_t = small.tile([DK, HP], F32, name="bqall", tag="bqall")
        bkv_t = small.tile([DK, 2 * HP], F32, name="bkvt", tag="bkvt")
        xl_t = small.tile([128, DC], FP8, name="xlt", tag="xlt")
        for g in range(1, DC // 4):
            # x leads its segment (PE unblocks on it); the wq tiles for
            # group g+1 ride behind, arriving a full segment early
            nc.sync.dma_start(
                xbig[g][:, 0:2 * S],
                xT_d[g * 512:g * 512 + 256, :].rearrange(
                    "(i p) s -> p i s", p=128))
            nc.sync.dma_start(
                xbig[g][:, 2 * S:4 * S],
                xT_d[g * 512 + 256:(g + 1) * 512, :].rearrange(
                    "(i p) s -> p i s", p=128))
            if g + 1 < DC // 4:
                for h in range(3):
                    emit_wq(h, g + 1)
            if g == 3:
                # small constants ride behind the early x groups; needed
                # first at the phase-A bias adds / S-h0 kv rides
                nc.sync.dma_start(bq_t[:], bq_d[:])
                nc.sync.dma_start(bkv_t[:], bkv_d[:])
                nc.sync.dma_start(xl_t[:], xl_d[:])

        def xsl(c, lo, sz):
            return xbig[c // 4][:, (c % 4) * S + lo:(c % 4) * S + lo + sz]

        # ---------- phase A: compute emissions ----------
        qAcc = [pss.tile([128, S], F32, name="qacc0", tag="pss"),
                pss.tile([128, S], F32, name="qacc1", tag="pss"),
                psc.tile([128, S], F32, name="qacc2", tag="psc")]
        qTs = [qT_pool.tile([128, S], F16, name=f"qT{h}", tag="qT")
               for h in range(HP)]
        for g in range(DC // 4):
            for h in range(3):
                for c in range(4 * g, 4 * g + 4):
                    lhs = wq_tiles[(h, g)][:, (c % 4) * DK:(c % 4 + 1) * DK]
                    nc.tensor.matmul(qAcc[h][:, 0:512], lhs, xsl(c, 0, 512),
                                     start=(c == 0), stop=(c == DC - 1))
                    nc.tensor.matmul(qAcc[h][:, 512:1024], lhs, xsl(c, 512, 512),
                                     start=(c == 0), stop=(c == DC - 1))
        for h in range(3):
            for hf in range(2):
                nc.vector.tensor_scalar_add(
                    qTs[h][:, hf * 512:(hf + 1) * 512],
                    qAcc[h][:, hf * 512:(hf + 1) * 512], bq_t[:, h:h + 1])

        # ---------- post-phase-A DMA block (queue order matters) ----------
        def load_group(h, g):
            g0 = g * 1024
            gl = min(1024, L - g0)
            gc = max(0, min(1024, pos - g0))
            kt8 = kt_pool.tile([128, 1024], F16, name=f"kt{h}_{g}", tag="kt")
            if gc > 0:
                nc.sync.dma_start(kt8[:, 0:gc], kT_d[h, :, g0:g0 + gc])
            if gl < 1024:
                nc.vector.memset(kt8[:, gl:1024], 0.0)
            v8 = v_pool.tile([128, 1024], F16, name=f"v{h}_{g}", tag="v")
            if gl < 1024:
                nc.vector.memset(v8[:, (gl // 128) * 128:1024], 0.0)
            fc = gc // 128
            if fc > 0:
                nc.sync.dma_start(
                    v8[:, 0:fc * 128],
                    v_d[h, g0:g0 + fc * 128, :].rearrange(
                        "(i p) k -> p i k", p=128))
            rem = gc - fc * 128
            if rem > 0:
                nc.sync.dma_start(v8[0:rem, fc * 128:(fc + 1) * 128],
                                  v_d[h, g0 + fc * 128:g0 + gc, :])
            return kt8, v8

        groups0 = [load_group(0, 0)]
        for g in range(DC // 4):
            emit_wq(3, g)
        for g in range(1, NG):
            groups0.append(load_group(0, g))
        wkv_tiles = []
        for jt in range(8):
            t = wkv_pool.tile([128, 4 * 2 * HP * DK], FP8,
                              name=f"wkv{jt}", tag="wkv")
            nc.sync.dma_start(
                t[:], wkv_d[jt * 512:(jt + 1) * 512, :].rearrange(
                    "(i p) k -> p i k", p=128))
            wkv_tiles.append(t)
        wos = []

        def emit_wo():
            # deferred to h2's loop start: wo is first read by the staged-O
            # rides (~124us); emitting it early parked 11.6us of transfers in
            # front of h1's cache loads, which then arrived marginally late
            for cblk in range(HP):
                wot = wo_pool.tile([128, D], F16, name=f"wo{cblk}",
                                   tag=f"wo{cblk}")
                nc.sync.dma_start(wot[:], wo_d[cblk * 128:(cblk + 1) * 128, :])
                wos.append(wot)

        # ---------- S loops ----------
        kvf = small.tile([DK, 2 * HP], F16, name="kvf", tag="kvf")
        gp = pos % 1024

        def new_entry_writes(h, kt8, v8):
            nc.sync.dma_start(kt8[:, gp:gp + 1], kvf[:, h:h + 1])
            nc.sync.dma_start(
                v8[gp % 128:gp % 128 + 1,
                   (gp // 128) * 128:(gp // 128 + 1) * 128],
                kvf[:, HP + h:HP + h + 1])

        kvacc = {}
        q3state = {}
        o_staged = {}        # (s_t, mg) -> (sg_tile, chunks_staged)
        o2state = {}

        def mk_o(loop_h, t, ck, nck):
            # staged-O ride: accumulate the first `nck` Wo chunks for tile t
            # (s_t, mg) into a spare PSUM bank, stage to SBUF; the O phase
            # finishes the remaining chunks and adds. h2 stages 2 chunks
            # (ctx0/1 known), h3 stages 3.
            base = 0 if loop_h == 2 else 16
            s_t, mg = divmod(base + t, 8)

            def emit():
                if ck == 0:
                    pool, tag = ((psq, "psq") if t % 2 == 0 else
                                 (kv_pool, "kv"))
                    o2state[(loop_h, t)] = pool.tile(
                        [128, 512], F32, name=f"o{loop_h}_{t}", tag=tag)
                acc = o2state[(loop_h, t)]
                nc.tensor.matmul(acc[:],
                                 ctxTs[ck][:, s_t * 128:(s_t + 1) * 128],
                                 wos[ck][:, mg * 512:(mg + 1) * 512],
                                 start=(ck == 0), stop=(ck == nck - 1))
                if ck == nck - 1:
                    sg = sg_pool.tile([128, 512], F16, name=f"sg{s_t}_{mg}",
                                      tag=f"sg{s_t}_{mg}")
                    nc.vector.tensor_copy(sg[:], acc[:])
                    o_staged[(s_t, mg)] = (sg, nck)
            return emit

        def mk_q3(half, c):
            # one chunk-matmul of head 3's Q per ride slot: half 0 rides in
            # h0's loop, half 1 in h1's — both loops stay just above ACT pace
            def emit():
                if c == 0:
                    q3state["psq"] = psq.tile([128, 512], F32,
                                              name=f"psq3_{half}", tag="psq")
                lhs = wq_tiles[(3, c // 4)][:, (c % 4) * DK:(c % 4 + 1) * DK]
                nc.tensor.matmul(q3state["psq"][:], lhs, xsl(c, half * 512, 512),
                                 start=(c == 0), stop=(c == DC - 1))
                if c == DC - 1:
                    nc.vector.tensor_scalar_add(
                        qTs[3][:, half * 512:(half + 1) * 512],
                        q3state["psq"][:], bq_t[:, 3:4])
            return emit

        for h in range(HP):
            rides = [[] for _ in range(LC)]
            if h in (0, 1):
                for lt in range(DC):
                    rides[lt].append(mk_q3(h, lt))
            if h == 2:
                # wkv is dead after h0's kv rides
                wkv_pool.release()
                emit_wo()
                # light staged-O riding: ~100ns/lt of DVE headroom under the
                # ACT pace allows one sg copy every ~6 lt
                for t in range(7):
                    for ck in range(2):
                        rides[4 * t + 2 * ck].append(mk_o(2, t, ck, 2))
            if h == 3:
                for t in range(7):
                    for ck in range(3):
                        rides[4 * t + ck].append(mk_o(3, t, ck, 3))
            if h == 0:

                def mk_kv(ci):
                    def emit():
                        if ci == 0:
                            kvacc["t"] = kv_pool.tile(
                                [128, 2 * HP], F32, name="kvt", tag="kv")
                        kt = kvacc["t"]
                        wt_ = wkv_tiles[ci]
                        for cc in range(4):
                            c = 4 * ci + cc
                            for jj in range(2 * HP):
                                nc.tensor.matmul(
                                    kt[:, jj:jj + 1],
                                    wt_[:, cc * 1024 + jj * DK:
                                        cc * 1024 + (jj + 1) * DK],
                                    xl_t[:, c:c + 1],
                                    start=(c == 0), stop=(c == DC - 1))
                    return emit

                for ci in range(8):
                    rides[2 + ci].append(mk_kv(ci))

                def kv_finish():
                    nc.vector.tensor_add(kvf[:], kvacc["t"][:], bkv_t[:])
                rides[10].append(kv_finish)

            pstate = {}
            psc_t = psc.tile([128, S], F32, name=f"psc{h}", tag="psc")
            cur = groups0[0] if h == 0 else load_group(h, 0)
            qT_t = qTs[h]
            nxt = None
            pends = []

            def ctx_mm(item, stop):
                plt, pwt, pvst = item
                nc.tensor.matmul(psc_t[:, 0:512], pvst[:], pwt[:, 0:512],
                                 start=(plt == 0), stop=stop)
                nc.tensor.matmul(psc_t[:, 512:1024], pvst[:], pwt[:, 512:1024],
                                 start=(plt == 0), stop=stop)

            for lt in range(LC):
                g, j = lt // 8, lt % 8
                if j == 0 and g > 0:
                    cur = nxt
                if j == 0 and g + 1 < NG:
                    nxt = groups0[g + 1] if h == 0 else load_group(h, g + 1)
                    if npos_g == g + 1:
                        new_entry_writes(h, *nxt)
                kt8, v8 = cur

                ps = pss.tile([128, 1024], F32, name=f"ps_{h}_{lt}", tag="pss")
                ksl = kt8[:, j * 128:(j + 1) * 128]
                nc.tensor.matmul(ps[:, 0:512], ksl, qT_t[:, 0:512])
                nc.tensor.matmul(ps[:, 512:1024], ksl, qT_t[:, 512:1024])

                for emit in rides[lt]:
                    emit()

                # Exp per lt into a shared 2-lt wt tile; the DVE row-sum
                # tree runs BATCHED per pair via strided 3D aps (~0.72us/lt
                # vs 0.84 unbatched), keeping ACT the steady pacer. An ACT
                # accum_out would make its service time exceed the loop
                # period and cascade stalls through the score-bank rotation.
                if lt % 2 == 0:
                    pstate["wtp"] = wt_pool.tile([128, 2048], F16,
                                                 name=f"wt_{h}_{lt}", tag="wt")
                wtp = pstate["wtp"]
                half = lt % 2
                nc.scalar.activation(wtp[:, half * 1024:(half + 1) * 1024],
                                     ps[:], AF.Exp, scale=INV)
                if half == 1:
                    wv = wtp.rearrange("p (l c) -> p l c", c=1024)
                    r1 = ss_pool.tile([128, 2, 512], F16, name=f"r1_{h}_{lt}",
                                      tag="r1", bufs=2)
                    nc.vector.tensor_add(r1[:], wv[:, :, 0:512],
                                         wv[:, :, 512:1024])
                    r2 = ss_pool.tile([128, 2, 256], F16, name=f"r2_{h}_{lt}",
                                      tag="r2", bufs=2)
                    nc.vector.tensor_add(r2[:], r1[:, :, 0:256],
                                         r1[:, :, 256:512])
                    r3 = ss_pool.tile([128, 2, 128], F16, name=f"r3_{h}_{lt}",
                                      tag="r3", bufs=2)
                    nc.vector.tensor_add(r3[:], r2[:, :, 0:128],
                                         r2[:, :, 128:256])
                    ssum = ss_pool.tile([128, 2], F32, name=f"ss_{h}_{lt}",
                                        tag="ssum")
                    nc.vector.tensor_reduce(ssum[:], r3[:],
                                            mybir.AxisListType.X,
                                            mybir.AluOpType.add)
                    rec = ss_pool.tile([128, 2], F32, name=f"rc_{h}_{lt}",
                                       tag="rec")
                    nc.vector.reciprocal(rec[:], ssum[:])
                    for q in range(2):
                        jq = j - 1 + q
                        vst = vs_pool.tile([128, DK], F16,
                                           name=f"vs{h}_{lt}_{q}", tag="vs")
                        nc.vector.tensor_scalar_mul(
                            vst[:], v8[:, jq * 128:(jq + 1) * 128],
                            rec[:, q:q + 1])
                        pends.append((lt - 1 + q, wv[:, q, :], vst))

                # lag >=4: the exp -> batched tree -> recip -> scale chain
                # spans over two loop periods
                while len(pends) > 8:
                    ctx_mm(pends.pop(0), stop=False)
            while len(pends) > 1:
                ctx_mm(pends.pop(0), stop=False)
            plt, pwt, pvst = pends.pop(0)
            nc.tensor.matmul(psc_t[:, 0:512], pvst[:], pwt[:, 0:512],
                             start=(plt == 0), stop=True)
            nc.vector.tensor_copy(ctxTs[h][:, 0:512], psc_t[:, 0:512])
            nc.tensor.matmul(psc_t[:, 512:1024], pvst[:], pwt[:, 512:1024],
                             start=(plt == 0), stop=True)
            nc.vector.tensor_copy(ctxTs[h][:, 512:1024], psc_t[:, 512:1024])

        # release attention-phase pools before the output projection (LIFO;
        # wkv was already released at the start of h2's loop)
        for p in (psc, pss, kv_pool, psq,
                  ss_pool, vs_pool, wt_pool, v_pool, kt_pool, wq_pool):
            p.release()

        # ---------- output projection ----------
        ob_pool = tc.alloc_tile_pool(name="obp", bufs=3)
        pso = tc.alloc_tile_pool(name="pso", bufs=6, space="PSUM")
        last_t = S // 128 - 1

        def emit_o_tile(s_t, mg, ob):
            obsl = ob[:, mg * 512:(mg + 1) * 512]
            staged = o_staged.get((s_t, mg))
            pso_t = pso.tile([128, 512], F32, name=f"po{s_t}_{mg}", tag="pso")
            c0 = staged[1] if staged else 0
            for cblk in range(c0, HP):
                nc.tensor.matmul(pso_t[:],
                                 ctxTs[cblk][:, s_t * 128:(s_t + 1) * 128],
                                 wos[cblk][:, mg * 512:(mg + 1) * 512],
                                 start=(cblk == c0), stop=(cblk == HP - 1))
            if staged:
                # DVE is idle in the O phase; ACT carries the plain copies
                nc.vector.tensor_add(obsl, staged[0][:], pso_t[:])
            elif s_t == last_t and mg >= 6:
                # last tiles: copy on DVE so the final copy runs parallel
                # to ACT's mg6 copy, shortening the end-of-kernel chain
                nc.vector.tensor_copy(obsl, pso_t[:])
            else:
                nc.scalar.activation(obsl, pso_t[:], AF.Copy)

        for s_t in range(S // 128):
            ob = ob_pool.tile([128, D], F16, name=f"ob{s_t}", tag="ob")
            # s_t 0: unstaged tiles first — their ctx0-2 chunks fill the
            # window while the h3 ctxT copy (needed by every chunk-3 matmul
            # and every staged tile) drains
            mgs = (5, 6, 7, 0, 1, 2, 3, 4) if s_t == 0 else range(D // 512)
            for mg in mgs:
                emit_o_tile(s_t, mg, ob)
                if s_t == last_t and mg >= 2:
                    # stream the final tile as it completes (HWDGE executes
                    # in order, so the big head chunk is emitted first and
                    # only the last 128KB chunk's transfer is exposed)
                    lo = 0 if mg == 2 else mg * 512
                    nc.sync.dma_start(
                        out_d[s_t * 128:(s_t + 1) * 128, lo:(mg + 1) * 512],
                        ob[:, lo:(mg + 1) * 512])
            if s_t != last_t:
                nc.sync.dma_start(out_d[s_t * 128:(s_t + 1) * 128, :], ob[:])
        for p in (pso, ob_pool, sg_pool, small, qT_pool, xT_pool,
                  wo_pool, ctxT_pool):
            p.release()

    nc.compile()
    return nc


_CACHE = {}
LAST_EXEC_NS = None


def kernel(x, k_cache, v_cache, Wq, bq, Wk, bk, Wv, bv, Wo, bo, pos):
    global LAST_EXEC_NS
    pos = int(pos)

    def f32(a):
        return np.ascontiguousarray(np.asarray(a), dtype=np.float32)

    x = f32(x)
    k_cache, v_cache = f32(k_cache), f32(v_cache)
    Wq, Wk, Wv, Wo = f32(Wq), f32(Wk), f32(Wv), f32(Wo)
    bq, bk, bv, bo = f32(bq), f32(bk), f32(bv), f32(bo)

    fast = (pos + 1 == 4096 and x.shape == (1, S, D))
    xT = np.ascontiguousarray(x[0].T.astype(np.float16))   # [D, S]
    in_maps = []
    for i in range(NCORES):
        hs = slice(i * HP, (i + 1) * HP)
        m = {
            "xT": xT,
            "wq": np.ascontiguousarray(Wq[hs].astype(np.float16)),
            "kT": np.ascontiguousarray(
                k_cache[hs, :pos, :].transpose(0, 2, 1).astype(np.float16)),
            "v": np.ascontiguousarray(v_cache[hs, :pos, :].astype(np.float16)),
            "wo": np.ascontiguousarray(
                Wo[i * HP * DK:(i + 1) * HP * DK].astype(np.float16)),
        }
        wkv_f32 = np.concatenate([
            Wk[hs].transpose(1, 0, 2).reshape(D, HP * DK),
            Wv[hs].transpose(1, 0, 2).reshape(D, HP * DK)], axis=1)
        if fast:
            m["wkv"] = np.ascontiguousarray(
                wkv_f32.astype(ml_dtypes.float8_e5m2))
            m["xl"] = np.ascontiguousarray(
                x[0, -1].reshape(DC, 128).T.astype(ml_dtypes.float8_e5m2))
            m["bq"] = np.ascontiguousarray(bq[hs].T)                 # [DK, HP]
            m["bkv"] = np.ascontiguousarray(
                np.concatenate([bk[hs].T, bv[hs].T], axis=1))        # [DK, 2HP]
        else:
            m["wkv"] = np.ascontiguousarray(wkv_f32.astype(ml_dtypes.bfloat16))
            m["xl"] = np.ascontiguousarray(
                x[0, -1].reshape(DC, 128).T.astype(ml_dtypes.bfloat16))
            m["bq"] = np.ascontiguousarray(bq[hs].reshape(HP, DK, 1))
            m["bkv"] = np.ascontiguousarray(np.concatenate(
                [bk[hs].reshape(-1), bv[hs].reshape(-1)])[None, :])
        in_maps.append(m)

    if pos not in _CACHE:
        _CACHE[pos] = build_fast(pos) if fast else build(pos)
    nc = _CACHE[pos]

    res = run_bass_kernel_spmd(nc, in_maps, core_ids=list(range(NCORES)))
    LAST_EXEC_NS = res.exec_time_ns

    acc = np.zeros((S, D), np.float64)
    for r in res.results:
        acc += r["out"]
    out = (acc + bo.astype(np.float64)).astype(np.float32)
    return out[None]

